# revision 25
# baseline (speedup 1.0000x reference)
"""AtomAttentionEncoder — hand-written Bass/Tile kernel for 8 trn2 NeuronCores.

Sequence-parallel over atoms: each core owns 192 atoms with a 192-atom halo on
each side (576 local atoms, zero inter-core collectives — the host sums the 8
per-core [384,384] token partials). Per core, bf16 channel-major throughout:

- atom activations [128 ch, 576 atoms]; LN stats via ones-matmuls (the PE
  broadcast comes for free), rstd = exp(-0.5*ln(var+eps)) so the whole main
  phase uses only the natural_log_exp activation-table set (one sigmoid-set
  phase precomputes all s-dependent gates, one Newton-rsqrt handles LN(s)).
- pair tensor [(gc,j)->(j,c) 128, rows=(block, q//4, q%4, k)] block-diagonal
  grouping so the 16-channel pair MLP and LN run as 128-wide matmuls; the
  whole pair pipeline is fused per 512-column chunk (one chunk == one block).
- 32x128 block-local attention on validity-shrinking blocks (14/10/6 per
  layer); no-max softmax (logits are tiny; masked keys get -1e4 before exp,
  exactly matching the reference's -1e9 since exp underflows to 0).
"""
import numpy as np
import ml_dtypes

import concourse.bass as bass
import concourse.mybir as mybir
import concourse.tile as tile
from concourse.alu_op_type import AluOpType
from concourse.bass_utils import run_bass_kernel_spmd

BF = mybir.dt.bfloat16
F32 = mybir.dt.float32
I32 = mybir.dt.int32
NPBF = ml_dtypes.bfloat16
AF = mybir.ActivationFunctionType

NCORES, OWN, MARGIN, LOC, PAD = 8, 192, 192, 576, 48
N_ATOM, N_TOK = 1536, 384
H, CH, CP = 4, 32, 16
NB0 = 14                      # layer-0 valid blocks (2..15)
ROWS = NB0 * 512              # pair rows (g, j, qq, k), q = 4j+qq
BIG = 1e4
EPS = 1e-5
L = 3
BLOCKS = [list(range(2, 16)), list(range(4, 14)), list(range(6, 12))]
RANGES = [(0, 576), (64, 512), (128, 448)]
AMUL, AADD, ASUB = AluOpType.mult, AluOpType.add, AluOpType.subtract
AMAX = AluOpType.max
RSQRT_MAGIC = 0x5F3759DF


# ------------------------------------------------------------------ host prep
def _prep_core(core, i):
    f32 = np.float32
    start = core * OWN - MARGIN
    idx = np.clip(start + np.arange(LOC), 0, N_ATOM - 1)
    pos = i['ref_pos'][0][idx].astype(f32)
    uid = i['ref_space_uid'][0][idx].astype(f32)
    feats = np.concatenate([
        i['ref_element'][0][idx],
        i['ref_atom_name_chars'][0].reshape(N_ATOM, -1)[idx],
        pos, i['ref_mask'][0][idx][:, None], i['ref_charge'][0][idx][:, None],
        uid[:, None], np.zeros((LOC, 122), f32)], axis=1)   # [LOC, 512]

    atom_mask = (i['atom_to_token_index'][0][idx] @ i['token_mask'][0]).astype(f32)
    mb = np.zeros((NB0, 128), f32)
    for n, b in enumerate(BLOCKS[0]):
        kl = 32 * b - PAD + np.arange(128)
        kg = start + kl
        ok = (kg >= 0) & (kg < N_ATOM) & (kl >= 0) & (kl < LOC)
        am = atom_mask[np.clip(kl, 0, LOC - 1)] > 0
        mb[n] = (np.where(ok & am, 1.0, 0.0) - 1.0) * BIG
    mb = np.broadcast_to(mb.reshape(1, -1), (128, NB0 * 128)).copy()

    qat = np.zeros((NB0, 8, 4), np.int64)          # local q-atom of (g, j, qq)
    for n, b in enumerate(BLOCKS[0]):
        qat[n] = 32 * b + (4 * np.arange(8)[:, None] + np.arange(4)[None, :])
    posq = np.zeros((24, NB0 * 4), f32)
    uidq = np.zeros((40, NB0 * 4), f32)
    for j in range(8):
        for gc in range(5):
            uidq[gc * 8 + j] = uid[qat[:, j, :]].reshape(-1)
            if gc < 3:
                posq[gc * 8 + j] = pos[qat[:, j, :], gc].reshape(-1)
    posw = np.zeros((24, LOC), f32)
    uidw = np.zeros((40, LOC), f32)
    for j in range(8):
        for gc in range(5):
            uidw[gc * 8 + j] = uid
            if gc < 3:
                posw[gc * 8 + j] = pos[:, gc]

    # a2t row-major, atoms padded 192 -> 2 chunks of 128 stacked on free axis
    a2t_own = i['atom_to_token_index'][0][core * OWN:(core + 1) * OWN].astype(f32)
    a2t_st = np.zeros((128, 2 * N_TOK), f32)
    a2t_st[:, :N_TOK] = a2t_own[0:128]
    a2t_st[0:64, N_TOK:] = a2t_own[128:192]
    d = {'feats': feats.T, 'posq': posq, 'uidq': uidq, 'posw': posw,
         'uidw': uidw, 'mb': mb, 'a2t': a2t_st}
    return {k: np.ascontiguousarray(v.astype(NPBF)) for k, v in d.items()}


def _prep_shared(i):
    f32 = np.float32
    inv_sqrt = 1.0 / np.sqrt(CH)
    Wf = np.asarray(i['W_feats'], f32)
    Wf2 = np.concatenate([Wf[4:132], Wf[133:389], Wf[0:3], Wf[3:4],
                          Wf[132:133], Wf[389:390],
                          np.zeros((122, 128), f32)], axis=0)       # [512,128]

    W5 = np.concatenate([np.asarray(i['W_ref_offset'], f32),
                         np.asarray(i['W_valid'], f32),
                         np.asarray(i['W_inv_sq'], f32)], 0)        # [5,16]
    bdW5 = np.zeros((40, 128), f32)
    for j in range(8):
        for gc in range(5):
            bdW5[gc * 8 + j, j * 16:(j + 1) * 16] = W5[gc]
    bdones3 = np.zeros((24, 8), f32)
    for j in range(8):
        for gc in range(3):
            bdones3[gc * 8 + j, j] = 1.0

    def bd8(w):
        n = w.shape[1]
        o = np.zeros((128, 8 * n), f32)
        for j in range(8):
            o[j * 16:(j + 1) * 16, j * n:(j + 1) * n] = w
        return o

    bdmlp = np.concatenate([bd8(np.asarray(i[f'W_mlp{k}'], f32)) for k in (1, 2, 3)], 1)
    bdones16 = bd8(np.full((16, 16), 1.0 / 16, f32))
    Wb_eff = np.asarray(i['lnz_g'], f32)[:, :, None] * np.asarray(i['Wb'], f32)
    bdWb = np.concatenate([bd8(Wb_eff[l]) for l in range(L)], 1)    # [128, 96]

    def stackL(w):
        return np.concatenate([np.asarray(w[l], f32) for l in range(L)], 1)

    d = {
        'Wf2': Wf2, 'bdW5': bdW5, 'bdones3': bdones3, 'bdones16': bdones16,
        'bdmlp': bdmlp, 'bdWb': bdWb,
        'Wlm': np.concatenate([np.asarray(i['W_l'], f32),
                               np.asarray(i['W_m'], f32)], 1),
        'Wq': stackL(np.asarray(i['Wq'], f32) * inv_sqrt),
        'Wk': stackL(i['Wk']), 'Wv': stackL(i['Wv']),
        'Wgate': stackL(i['Wgate']), 'Wo': stackL(i['Wo']),
        'aWg': stackL(np.asarray(i['attn_ada_Wg'], f32) *
                      np.asarray(i['attn_ada_gamma_s'], f32)[:, :, None]),
        'aWs': stackL(np.asarray(i['attn_ada_Ws'], f32) *
                      np.asarray(i['attn_ada_gamma_s'], f32)[:, :, None]),
        'tWg': stackL(np.asarray(i['tr_ada_Wg'], f32) *
                      np.asarray(i['tr_ada_gamma_s'], f32)[:, :, None]),
        'tWs': stackL(np.asarray(i['tr_ada_Ws'], f32) *
                      np.asarray(i['tr_ada_gamma_s'], f32)[:, :, None]),
        'Wsg': stackL(i['Wsg']), 'tWog': stackL(i['tr_Wog']),
        'trW1': np.concatenate([np.asarray(i['tr_W1'], f32)[l] for l in range(L)], 1),
        'trW2': np.concatenate([np.asarray(i['tr_W2'], f32)[l] for l in range(L)], 1),
        'trWout': np.concatenate(
            [np.asarray(i['tr_Wout'], f32)[l, h * 128:(h + 1) * 128]
             for l in range(L) for h in range(2)], 1),
        'Wot': i['W_out_tok'],
    }
    d = {k: np.ascontiguousarray(np.asarray(v, f32).astype(NPBF)) for k, v in d.items()}
    d['ident'] = np.eye(128, dtype=f32).astype(NPBF)
    pmat = np.zeros((128, 128), f32)       # zt row p'=16j+4h+qq -> m=32h+4j+qq
    for j in range(8):
        for h in range(4):
            for qq in range(4):
                pmat[16 * j + 4 * h + qq, 32 * h + 4 * j + qq] = 1.0
    d['pmat'] = pmat.astype(NPBF)
    d['bq'] = np.ascontiguousarray((np.asarray(i['bq'], f32) * inv_sqrt).T)
    d['bg_a'] = np.ascontiguousarray(np.asarray(i['attn_ada_bg'], f32).T)
    d['bg_t'] = np.ascontiguousarray(np.asarray(i['tr_ada_bg'], f32).T)
    d['bsg'] = np.ascontiguousarray(np.asarray(i['bsg'], f32).T)
    d['tbog'] = np.ascontiguousarray(np.asarray(i['tr_bog'], f32).T)
    return d


# ------------------------------------------------------------ program builder
def build(debug=()):
    nc = bass.Bass()
    TT = nc.vector.tensor_tensor
    TS = nc.vector.tensor_scalar
    ACT = nc.scalar.activation
    MM = nc.tensor.matmul

    def din(name, shape, dt=BF):
        return nc.dram_tensor(name, shape, dt, kind="ExternalInput")[:]

    ins = {}
    for nm, shp in [('posq', [24, NB0 * 4]),
                    ('uidq', [40, NB0 * 4]), ('posw', [24, LOC]),
                    ('uidw', [40, LOC]), ('mb', [128, NB0 * 128]),
                    ('a2t', [128, 2 * N_TOK]),
                    ('bdW5', [40, 128]), ('bdones3', [24, 8]),
                    ('bdones16', [128, 128]), ('bdmlp', [128, 384]),
                    ('bdWb', [128, 32 * L]), ('Wlm', [128, 32]),
                    ('Wq', [128, 128 * L]), ('Wk', [128, 128 * L]),
                    ('Wv', [128, 128 * L]), ('Wgate', [128, 128 * L]),
                    ('Wo', [128, 128 * L]), ('aWg', [128, 128 * L]),
                    ('aWs', [128, 128 * L]), ('tWg', [128, 128 * L]),
                    ('tWs', [128, 128 * L]), ('Wsg', [128, 128 * L]),
                    ('tWog', [128, 128 * L]), ('trW1', [128, 256 * L]),
                    ('trW2', [128, 256 * L]), ('trWout', [128, 128 * 2 * L]),
                    ('Wot', [128, N_TOK])]:
        ins[nm] = din(nm, shp)
    for nm in ('bq', 'bg_a', 'bg_t', 'bsg', 'tbog'):
        ins[nm] = din(nm, [128, L], F32)
    feats_d = din('feats', [512, LOC])
    wf2_d = din('Wf2', [512, 128])
    ins['ident'] = din('ident', [128, 128])
    ins['pmat'] = din('pmat', [128, 128])

    out_part = nc.dram_tensor('part', [N_TOK, N_TOK], F32, kind="ExternalOutput")[:]
    out_cnt = nc.dram_tensor('cnt', [128, 3], F32, kind="ExternalOutput")[:]
    dumps = []

    def dump(name, ap):
        if name not in debug:
            return
        t = nc.dram_tensor('dbg_' + name, list(ap.shape), ap.dtype,
                           kind="ExternalOutput")[:]
        nc.sync.dma_start(out=t, in_=ap)

    with tile.TileContext(nc) as tc, \
         tc.tile_pool(name="const", bufs=1) as constp, \
         tc.tile_pool(name="state", bufs=1) as statep, \
         tc.tile_pool(name="work", bufs=3) as workp, \
         tc.tile_pool(name="abuf", bufs=2) as abufp, \
         tc.tile_pool(name="blk", bufs=16) as blkp, \
         tc.tile_pool(name="psum", bufs=1, space="PSUM") as psp:

        ones_row = constp.tile([1, 128], BF)
        nc.vector.memset(ones_row, 1.0)
        ones_col = constp.tile([128, 1], BF)
        nc.vector.memset(ones_col, 1.0)
        inv128 = constp.tile([128, 128], BF)
        nc.vector.memset(inv128, 1.0 / 128.0)
        epsc = constp.tile([128, 1], F32)
        nc.vector.memset(epsc, EPS)
        onef = constp.tile([128, 1], F32)
        nc.vector.memset(onef, 1.0)
        zeros128 = constp.tile([128, 128], BF)
        nc.vector.memset(zeros128, 0.0)

        _ldn = [0]

        def load(ap, name, pool=constp):
            t = pool.tile(list(ap.shape), ap.dtype, name=name)
            eng = nc.sync if _ldn[0] % 2 == 0 else nc.gpsimd
            _ldn[0] += 1
            eng.dma_start(out=t, in_=ap)
            return t

        s_ = {nm: load(ap, 'w_' + nm) for nm, ap in ins.items()}
        s_feats = [load(feats_d[kk * 128:(kk + 1) * 128, :], f'feats{kk}')
                   for kk in range(4)]
        s_wf2 = [load(wf2_d[kk * 128:(kk + 1) * 128, :], f'wf2_{kk}')
                 for kk in range(4)]

        def ps512():
            return psp.tile([128, 512], F32, tag='ps512', bufs=4, name='ps512')

        def psb(tag='psb'):
            return psp.tile([128, 128], F32, tag='psb', bufs=4, name='psb')

        # ---------------- embed: cl [128, LOC] ----------------
        cl = statep.tile([128, LOC], BF)
        for n0 in (0, 512):
            n1 = min(n0 + 512, LOC)
            ps = ps512()
            for kk in range(4):
                MM(ps[:, :n1 - n0], s_wf2[kk], s_feats[kk][:, n0:n1],
                   start=(kk == 0), stop=(kk == 3))
            TS(cl[:, n0:n1], ps[:, :n1 - n0], 1.0, None, AMUL)
        dump('cl', cl)

        # crelu -> [crl | crm] = crlm [32, LOC]; then crl_bd + crm_rep
        crelu = workp.tile([128, LOC], BF, tag='crelu', bufs=1)
        TS(crelu, cl, 0.0, None, AMAX)
        crlm = workp.tile([32, LOC], BF, tag='crlm', bufs=1)
        for n0 in (0, 512):
            n1 = min(n0 + 512, LOC)
            ps = ps512()
            MM(ps[0:32, :n1 - n0], s_['Wlm'], crelu[:, n0:n1], start=True, stop=True)
            TS(crlm[:, n0:n1], ps[0:32, :n1 - n0], 1.0, None, AMUL)
        crl_bd = statep.tile([128, NB0 * 4], BF)
        crm_rep = statep.tile([128, LOC], BF)
        pz = crlm.ap[0][0]
        for j in range(8):
            nc.gpsimd.dma_start(
                out=crl_bd[j * 16:(j + 1) * 16, :],
                in_=bass.AP(tensor=crlm.tensor, offset=crlm.offset + 64 + 4 * j,
                            ap=[[pz, 16], [32, NB0], [1, 4]]))
            nc.gpsimd.dma_start(out=crm_rep[j * 16:(j + 1) * 16, :],
                              in_=bass.AP(tensor=crlm.tensor,
                                          offset=crlm.offset + 16 * pz,
                                          ap=[[pz, 16], [1, LOC]]))
        dump('crl_bd', crl_bd)

        # ---------------- LN(s) via Newton rsqrt (table-free) ----------------
        sh = statep.tile([128, LOC], BF)
        xcs = workp.tile([128, LOC], BF, tag='xcs', bufs=1)
        vps = workp.tile([128, LOC], F32, tag='vps', bufs=1)   # var+eps
        y0 = workp.tile([128, LOC], F32, tag='nr1', bufs=1)
        t1 = workp.tile([128, LOC], F32, tag='nr2', bufs=1)
        for n0 in (0, 512):
            n1 = min(n0 + 512, LOC)
            w = n1 - n0
            ps = ps512()
            MM(ps[:, :w], inv128, cl[:, n0:n1], start=True, stop=True)
            TT(xcs[:, n0:n1], cl[:, n0:n1], ps[:, :w], ASUB)
            sq = workp.tile([128, 512], BF, tag='sqs')
            TT(sq[:, :w], xcs[:, n0:n1], xcs[:, n0:n1], AMUL)
            ps2 = ps512()
            MM(ps2[:, :w], inv128, sq[:, :w], start=True, stop=True)
            TS(vps[:, n0:n1], ps2[:, :w], EPS, None, AADD)
        v32 = vps.bitcast(I32)
        y32 = y0.bitcast(I32)
        t32 = t1.bitcast(I32)
        nc.vector.memset(t32, RSQRT_MAGIC)
        TS(y32, v32, 1, None, AluOpType.logical_shift_right)
        TT(y32, t32, y32, ASUB)                      # magic - (v>>1)
        for _ in range(2):                           # y *= 1.5 - 0.5*v*y*y
            TT(t1, y0, y0, AMUL)
            TT(t1, vps, t1, AMUL)
            TS(t1, t1, -0.5, 1.5, AMUL, AADD)
            TT(y0, y0, t1, AMUL)
        TT(sh, xcs, y0, AMUL)
        dump('sh', sh)

        # ---------------- s-gate precompute (sigmoid table set) ----------------
        sgate, sa = {}, {}
        for nm, src, bias in (('ga', 'aWg', 'bg_a'), ('gt', 'tWg', 'bg_t'),
                              ('sg', 'Wsg', 'bsg'), ('og', 'tWog', 'tbog')):
            x = sh if nm in ('ga', 'gt') else cl
            for l in range(L):
                g = statep.tile([128, LOC], BF, name=f'{nm}{l}')
                for n0 in (0, 512):
                    n1 = min(n0 + 512, LOC)
                    ps = ps512()
                    MM(ps[:, :n1 - n0], s_[src][:, l * 128:(l + 1) * 128],
                       x[:, n0:n1], start=True, stop=True)
                    ACT(g[:, n0:n1], ps[:, :n1 - n0], AF.Sigmoid,
                        bias=s_[bias][:, l:l + 1])
                sgate[(nm, l)] = g
        for nm, src in (('sa', 'aWs'), ('st', 'tWs')):
            for l in range(L):
                g = statep.tile([128, LOC], BF, name=f'{nm}{l}')
                for n0 in (0, 512):
                    n1 = min(n0 + 512, LOC)
                    ps = ps512()
                    MM(ps[:, :n1 - n0], s_[src][:, l * 128:(l + 1) * 128],
                       sh[:, n0:n1], start=True, stop=True)
                    TS(g[:, n0:n1], ps[:, :n1 - n0], 1.0, None, AMUL)
                sa[(nm, l)] = g
        dump('ga0', sgate[('ga', 0)])

        # ---------------- pair pipeline (fused per 512-chunk) ----------------
        plm = statep.tile([128, ROWS], BF)

        def winap(t, base, rows):
            return bass.AP(tensor=t.tensor, offset=t.offset + base,
                           ap=[[t.ap[0][0], rows], [0, 4], [1, 128]])

        def qapx(t, rows, goff):
            return bass.AP(tensor=t.tensor, offset=t.offset + goff * 4,
                           ap=[[t.ap[0][0], rows], [1, 4], [0, 128]])

        for n in range(NB0):
            base = 32 * (n + 2) - 48
            G = workp.tile([40, 512], BF, tag='G')
            d_raw = workp.tile([24, 512], BF, tag='draw')
            TT(d_raw, winap(s_['posw'], base, 24), qapx(s_['posq'], 24, n), ASUB)
            TT(G, winap(s_['uidw'], base, 40), qapx(s_['uidq'], 40, n),
               AluOpType.is_equal)
            TT(G[0:24, :], d_raw, G[0:24, :], AMUL)
            d2 = workp.tile([24, 512], BF, tag='d2')
            TT(d2, d_raw, d_raw, AMUL)
            psd = ps512()
            MM(psd[32:40, :], s_['bdones3'], d2, start=True, stop=True,
               tile_position=(0, 32))
            lni = workp.tile([40, 512], F32, tag='lni')
            ACT(lni[32:40, :], psd[32:40, :], AF.Ln, bias=onef[32:40, :])
            inv = workp.tile([40, 512], BF, tag='inv')
            ACT(inv[32:40, :], lni[32:40, :], AF.Exp, scale=-1.0)
            TT(G[32:40, :], inv[32:40, :], G[32:40, :], AMUL)

            ps = ps512()
            MM(ps, s_['bdW5'], G, start=True, stop=False)
            MM(ps, s_['ident'], qapx(crl_bd, 128, n), start=False, stop=False,
               skip_group_check=True)
            MM(ps, s_['ident'], winap(crm_rep, base, 128), start=False, stop=True,
               skip_group_check=True)
            ppre = workp.tile([128, 512], BF, tag='ppre')
            r = workp.tile([128, 512], BF, tag='rmlp')
            TS(ppre, ps, 1.0, None, AMUL)
            nc.gpsimd.tensor_scalar(r, ppre, 0.0, None, AMAX)
            pf = None
            for k in range(3):
                psm = ps512()
                MM(psm, s_['bdmlp'][:, k * 128:(k + 1) * 128], r,
                   start=True, stop=(k == 2))
                if k < 2:
                    r = workp.tile([128, 512], BF, tag='rmlp')
                    TS(r, psm, 0.0, None, AMAX)
                else:
                    MM(psm, s_['ident'], ppre, start=False, stop=True,
                       skip_group_check=True)
                    pf = workp.tile([128, 512], BF, tag='pfull')
                    TS(pf, psm, 1.0, None, AMUL)
            psmu = ps512()
            MM(psmu, s_['bdones16'], pf, start=True, stop=True)
            xc = workp.tile([128, 512], BF, tag='xc')
            TT(xc, pf, psmu, ASUB)
            sq2 = workp.tile([128, 512], BF, tag='sq2')
            TT(sq2, xc, xc, AMUL)
            psv = ps512()
            MM(psv, s_['bdones16'], sq2, start=True, stop=True)
            lnv = workp.tile([128, 512], F32, tag='lnv')
            ACT(lnv, psv, AF.Ln, bias=epsc)
            rstd = workp.tile([128, 512], BF, tag='rstdp')
            ACT(rstd, lnv, AF.Exp, scale=-0.5)
            TT(plm[:, n * 512:(n + 1) * 512], xc, rstd, AMUL)
        dump('plm', plm)

        # ---------------- layer loop ----------------
        a_cur = cl
        for l in range(L):
            r0, r1 = RANGES[l]
            blks = BLOCKS[l]
            nb = len(blks)

            ahat = abufp.tile([128, LOC], BF, tag='ahat')
            for c0 in range(r0, r1, 512):
                c1 = min(c0 + 512, r1)
                w = c1 - c0
                ps = ps512()
                MM(ps[:, :w], inv128, a_cur[:, c0:c1], start=True, stop=True)
                xca = abufp.tile([128, 512], BF, tag='xca')
                TT(xca[:, :w], a_cur[:, c0:c1], ps[:, :w], ASUB)
                sqa = abufp.tile([128, 512], BF, tag='sqa')
                TT(sqa[:, :w], xca[:, :w], xca[:, :w], AMUL)
                ps2 = ps512()
                MM(ps2[:, :w], inv128, sqa[:, :w], start=True, stop=True)
                lnva = abufp.tile([128, 512], F32, tag='lnva')
                ACT(lnva[:, :w], ps2[:, :w], AF.Ln, bias=epsc)
                rstda = abufp.tile([128, 512], BF, tag='rstda')
                ACT(rstda[:, :w], lnva[:, :w], AF.Exp, scale=-0.5)
                TT(ahat[:, c0:c1], xca[:, :w], rstda[:, :w], AMUL)
            an = abufp.tile([128, LOC], BF, tag='an')
            tn = abufp.tile([128, LOC], BF, tag='tn')
            TT(an[:, r0:r1], sgate[('ga', l)][:, r0:r1], ahat[:, r0:r1], AMUL)
            TT(an[:, r0:r1], an[:, r0:r1], sa[('sa', l)][:, r0:r1], AADD)
            TT(tn[:, r0:r1], sgate[('gt', l)][:, r0:r1], ahat[:, r0:r1], AMUL)
            TT(tn[:, r0:r1], tn[:, r0:r1], sa[('st', l)][:, r0:r1], AADD)
            if l == 0:
                dump('an0', an)

            q_sb = abufp.tile([128, LOC], BF, tag='q')
            k_sb = abufp.tile([128, LOC], BF, tag='k')
            g_sb = abufp.tile([128, LOC], BF, tag='g')
            for c0 in range(r0, r1, 512):
                c1 = min(c0 + 512, r1)
                w = c1 - c0
                psq = ps512()
                MM(psq[:, :w], s_['Wq'][:, l * 128:(l + 1) * 128], an[:, c0:c1],
                   start=True, stop=True)
                ACT(q_sb[:, c0:c1], psq[:, :w], AF.Identity,
                    bias=s_['bq'][:, l:l + 1])
                psk = ps512()
                MM(psk[:, :w], s_['Wk'][:, l * 128:(l + 1) * 128], an[:, c0:c1],
                   start=True, stop=True)
                TS(k_sb[:, c0:c1], psk[:, :w], 1.0, None, AMUL)
                psg = ps512()
                MM(psg[:, :w], s_['Wgate'][:, l * 128:(l + 1) * 128], an[:, c0:c1],
                   start=True, stop=True)
                ex0 = abufp.tile([128, 512], F32, tag='ex0')
                ACT(ex0[:, :w], psg[:, :w], AF.Exp)
                ACT(ex0[:, :w], ex0[:, :w], AF.Ln, bias=onef)
                TT(ex0[:, :w], psg[:, :w], ex0[:, :w], ASUB)
                ACT(g_sb[:, c0:c1], ex0[:, :w], AF.Exp)

            zbt = {}
            for b in blks:
                ch = b - 2
                psz = ps512()
                MM(psz[0:32, :], s_['bdWb'][:, l * 32:(l + 1) * 32],
                   plm[:, ch * 512:(ch + 1) * 512], start=True, stop=True)
                zs = blkp.tile([32, 512], BF, tag='zs', bufs=4)
                nc.scalar.copy(zs, psz[0:32, :])
                zt = blkp.tile([128, 128], BF, tag='zt', bufs=16)
                nc.gpsimd.tensor_copy(zt, s_['mb'][:, (b - 2) * 128:(b - 1) * 128])
                pzt = zt.ap[0][0]
                for qq in range(4):
                    nc.gpsimd.dma_start(
                        out=bass.AP(tensor=zt.tensor, offset=zt.offset + qq * pzt,
                                    ap=[[4 * pzt, 32], [1, 128]]),
                        in_=zs[:, 128 * qq:128 * qq + 128],
                        accum_op=AluOpType.add)
                zbt[b] = zt
            if l == 0:
                dump('zbt2', zbt[2])

            v_sb = {}
            for b in blks:
                base = 32 * b - 48
                psv2 = psb()
                MM(psv2, an[:, base:base + 128],
                   s_['Wv'][:, l * 128:(l + 1) * 128], start=True, stop=True)
                vt = blkp.tile([128, 128], BF, tag='vt', bufs=16)
                nc.scalar.copy(vt, psv2)
                v_sb[b] = vt

            # pass 1: logits -> exp -> A (accumulate row sums)
            dsum = abufp.tile([128, 16], F32, tag='dsum')
            A_sb = {}
            for n, b in enumerate(blks):
                base = 32 * b - 48
                psl = psb()
                MM(psl, s_['pmat'], zbt[b], start=True, stop=False,
                   skip_group_check=True)
                for h in range(4):
                    MM(psl[32 * h:32 * h + 32, :],
                       q_sb[32 * h:32 * h + 32, 32 * b:32 * b + 32],
                       k_sb[32 * h:32 * h + 32, base:base + 128],
                       start=False, stop=(h == 3),
                       tile_position=(32 * h, 32 * h), skip_group_check=True)
                At = blkp.tile([128, 128], BF, tag='At', bufs=16)
                ACT(At, psl, AF.Exp, accum_out=dsum[:, n:n + 1])
                A_sb[b] = At
            if l == 0:
                dump('A2', A_sb[2])
            rd = abufp.tile([128, 16], F32, tag='rd')
            TS(rd[:, :nb], dsum[:, :nb], 1e-9, None, AADD)
            nc.vector.reciprocal(rd[:, :nb], rd[:, :nb])

            # pass 2: normalize A -> transpose -> AV -> O (ch-major)
            Ocm = abufp.tile([128, LOC], F32, tag='Ocm')
            for n, b in enumerate(blks):
                At = A_sb[b]
                TS(At, At, rd[:, n:n + 1], None, AMUL)
                pst = psp.tile([128, 128], BF, tag='psb', bufs=4, name='pstT')
                nc.tensor.transpose(pst, At, s_['ident'])
                ATs = blkp.tile([128, 128], BF, tag='ATs', bufs=8)
                TS(ATs, pst, 1.0, None, AMUL)
                pso = psb()
                MM(pso[:, 0:32], s_['ident'], zeros128[:, 0:32],
                   start=True, stop=False, skip_group_check=True)
                for h in range(4):
                    MM(pso[32 * h:32 * h + 32, 0:32],
                       ATs[:, 32 * h:32 * h + 32],
                       v_sb[b][:, 32 * h:32 * h + 32],
                       start=False, stop=(h == 3),
                       tile_position=(0, 32 * h), skip_group_check=True)
                nc.vector.transpose(Ocm[:, 32 * b:32 * b + 32], pso[:, 0:32])
            if l == 0:
                dump('Ocm0', Ocm)

            # epilogue: gated attn out + transition, next range only
            nr0, nr1 = 32 * blks[0], 32 * blks[-1] + 32
            go = abufp.tile([128, LOC], BF, tag='go')
            TT(go[:, nr0:nr1], g_sb[:, nr0:nr1], Ocm[:, nr0:nr1], AMUL)
            a_nxt = abufp.tile([128, LOC], BF, tag='anxt')
            nc.vector.memset(a_nxt, 0.0)
            for c0 in range(nr0, nr1, 512):
                c1 = min(c0 + 512, nr1)
                w = c1 - c0
                psa = ps512()
                MM(psa[:, :w], s_['Wo'][:, l * 128:(l + 1) * 128], go[:, c0:c1],
                   start=True, stop=True)
                ao = abufp.tile([128, 512], BF, tag='ao')
                TT(ao[:, :w], sgate[('sg', l)][:, c0:c1], psa[:, :w], AMUL)
                hh = []
                for t2 in range(2):
                    psh = ps512()
                    MM(psh[:, :w],
                       s_['trW1'][:, l * 256 + t2 * 128:l * 256 + (t2 + 1) * 128],
                       tn[:, c0:c1], start=True, stop=True)
                    ex = abufp.tile([128, 512], F32, tag='ex')
                    ACT(ex[:, :w], psh[:, :w], AF.Exp)
                    ACT(ex[:, :w], ex[:, :w], AF.Ln, bias=onef)
                    TT(ex[:, :w], psh[:, :w], ex[:, :w], ASUB)
                    ACT(ex[:, :w], ex[:, :w], AF.Exp)
                    s1 = abufp.tile([128, 512], BF, tag='s1')
                    TT(s1[:, :w], psh[:, :w], ex[:, :w], AMUL)
                    psh2 = ps512()
                    MM(psh2[:, :w],
                       s_['trW2'][:, l * 256 + t2 * 128:l * 256 + (t2 + 1) * 128],
                       tn[:, c0:c1], start=True, stop=True)
                    ht = abufp.tile([128, 512], BF, tag=f'hh{t2}')
                    TT(ht[:, :w], s1[:, :w], psh2[:, :w], AMUL)
                    hh.append(ht)
                pst2 = ps512()
                MM(pst2[:, :w], s_['trWout'][:, (l * 2) * 128:(l * 2 + 1) * 128],
                   hh[0][:, :w], start=True, stop=False)
                MM(pst2[:, :w], s_['trWout'][:, (l * 2 + 1) * 128:(l * 2 + 2) * 128],
                   hh[1][:, :w], start=False, stop=True)
                to = abufp.tile([128, 512], BF, tag='to')
                TT(to[:, :w], sgate[('og', l)][:, c0:c1], pst2[:, :w], AMUL)
                TT(a_nxt[:, c0:c1], ao[:, :w], to[:, :w], AADD)
            a_cur = a_nxt
        dump('a3', a_cur)

        # ---------------- final: al -> token partials ----------------
        al_rm = []
        for c in range(2):
            c0 = 192 + 128 * c
            psf = ps512()
            MM(psf[:, :N_TOK], a_cur[:, c0:c0 + 128], s_['Wot'],
               start=True, stop=True)
            alr = statep.tile([128, N_TOK], BF, name=f'alr{c}')
            TS(alr, psf[:, :N_TOK], 0.0, None, AMAX)
            al_rm.append(alr)
        partf = statep.tile([128, N_TOK], F32, name='partf')
        cntf = statep.tile([128, 3], F32, name='cntf')
        for tchunk in range(3):
            psp2 = ps512()
            for c in range(2):
                MM(psp2[:, :N_TOK],
                   s_['a2t'][:, c * N_TOK + tchunk * 128:
                             c * N_TOK + tchunk * 128 + 128],
                   al_rm[c], start=(c == 0), stop=(c == 1))
            TS(partf, psp2[:, :N_TOK], 1.0, None, AMUL)
            nc.sync.dma_start(out=out_part[tchunk * 128:(tchunk + 1) * 128, :],
                              in_=partf)
            psc = psb()
            for c in range(2):
                MM(psc[:, 0:1],
                   s_['a2t'][:, c * N_TOK + tchunk * 128:
                             c * N_TOK + tchunk * 128 + 128],
                   ones_col, start=(c == 0), stop=(c == 1))
            TS(cntf[:, tchunk:tchunk + 1], psc[:, 0:1], 1.0, None, AMUL)
        nc.sync.dma_start(out=out_cnt, in_=cntf)

    return nc


def _legalize_waits(nc, maxw=1):
    """The container's walrus accepts at most one sync-wait per instruction;
    Tile emits several. Split excess waits onto preceding same-engine NoOps
    (each wait is a >= threshold, so sequential waits are equivalent)."""
    for fn in nc.m.functions:
        for b in fn.blocks:
            out = []
            for i in b.instructions:
                si = i.sync_info
                if si is not None and len(si.on_wait) > maxw:
                    waits = list(si.on_wait)
                    k = 0
                    while len(waits) > maxw:
                        chunk, waits = waits[:maxw], waits[maxw:]
                        out.append(mybir.InstNoOp(
                            name=f"{i.name}-wsplit{k}", ins=[], outs=[],
                            engine=i.engine,
                            sync_info=mybir.SyncInfo(on_wait=chunk, on_update=[])))
                        k += 1
                    i.sync_info = mybir.SyncInfo(on_wait=waits,
                                                 on_update=list(si.on_update))
                out.append(i)
            b.instructions = out
    return nc


_CACHE = {}


def _get_nc(debug=()):
    key = tuple(sorted(debug))
    if key not in _CACHE:
        _CACHE[key] = _legalize_waits(build(key))
    return _CACHE[key]


def _maps(inputs):
    i = {k: np.asarray(v) for k, v in inputs.items()}
    shared = _prep_shared(i)
    maps = []
    for c in range(NCORES):
        m = dict(shared)
        m.update(_prep_core(c, i))
        maps.append(m)
    return maps


def kernel(**inputs):
    nc = _get_nc()
    res = run_bass_kernel_spmd(nc, _maps(inputs), list(range(NCORES))).results
    tot = np.zeros((N_TOK, N_TOK), np.float64)
    cnt = np.zeros(N_TOK, np.float64)
    for c in range(NCORES):
        tot += np.asarray(res[c]['part'], np.float64)
        cnt += np.asarray(res[c]['cnt'], np.float64).T.reshape(-1)
    out = tot / np.maximum(cnt, 1.0)[:, None]
    return out[None].astype(np.float32)


# revision 26
# speedup vs baseline: 1.2593x; 1.2593x over previous
"""AtomAttentionEncoder — hand-written Bass/Tile kernel for 8 trn2 NeuronCores.

Sequence-parallel over atoms: each core owns 192 atoms with a 192-atom halo on
each side (576 local atoms, zero inter-core collectives — the host sums the 8
per-core [384,384] token partials). Per core, bf16 channel-major throughout:

- atom activations [128 ch, 576 atoms]; LN stats via ones-matmuls (the PE
  broadcast comes for free), rstd = exp(-0.5*ln(var+eps)) so the whole main
  phase uses only the natural_log_exp activation-table set (one sigmoid-set
  phase precomputes all s-dependent gates, one Newton-rsqrt handles LN(s)).
- pair tensor [(gc,j)->(j,c) 128, rows=(block, q//4, q%4, k)] block-diagonal
  grouping so the 16-channel pair MLP and LN run as 128-wide matmuls; the
  whole pair pipeline is fused per 512-column chunk (one chunk == one block).
- 32x128 block-local attention on validity-shrinking blocks (14/10/6 per
  layer); no-max softmax (logits are tiny; masked keys get -1e4 before exp,
  exactly matching the reference's -1e9 since exp underflows to 0).
"""
import numpy as np
import ml_dtypes

import concourse.bass as bass
import concourse.mybir as mybir
import concourse.tile as tile
from concourse.alu_op_type import AluOpType
from concourse.bass_utils import run_bass_kernel_spmd

BF = mybir.dt.bfloat16
F32 = mybir.dt.float32
I32 = mybir.dt.int32
NPBF = ml_dtypes.bfloat16
AF = mybir.ActivationFunctionType

NCORES, OWN, MARGIN, LOC, PAD = 8, 192, 192, 576, 48
N_ATOM, N_TOK = 1536, 384
H, CH, CP = 4, 32, 16
NB0 = 14                      # layer-0 valid blocks (2..15)
ROWS = NB0 * 512              # pair rows (g, j, qq, k), q = 4j+qq
BIG = 1e4
EPS = 1e-5
L = 3
BLOCKS = [list(range(2, 16)), list(range(4, 14)), list(range(6, 12))]
RANGES = [(0, 576), (64, 512), (128, 448)]
AMUL, AADD, ASUB = AluOpType.mult, AluOpType.add, AluOpType.subtract
AMAX = AluOpType.max
RSQRT_MAGIC = 0x5F3759DF


# ------------------------------------------------------------------ host prep
def _prep_core(core, i):
    f32 = np.float32
    start = core * OWN - MARGIN
    idx = np.clip(start + np.arange(LOC), 0, N_ATOM - 1)
    pos = i['ref_pos'][0][idx].astype(f32)
    uid = i['ref_space_uid'][0][idx].astype(f32)
    feats = np.concatenate([
        i['ref_element'][0][idx],
        i['ref_atom_name_chars'][0].reshape(N_ATOM, -1)[idx],
        pos, i['ref_mask'][0][idx][:, None], i['ref_charge'][0][idx][:, None],
        uid[:, None], np.zeros((LOC, 122), f32)], axis=1)   # [LOC, 512]

    atom_mask = (i['atom_to_token_index'][0][idx] @ i['token_mask'][0]).astype(f32)
    mb = np.zeros((NB0, 128), f32)
    for n, b in enumerate(BLOCKS[0]):
        kl = 32 * b - PAD + np.arange(128)
        kg = start + kl
        ok = (kg >= 0) & (kg < N_ATOM) & (kl >= 0) & (kl < LOC)
        am = atom_mask[np.clip(kl, 0, LOC - 1)] > 0
        mb[n] = (np.where(ok & am, 1.0, 0.0) - 1.0) * BIG
    mb = np.broadcast_to(mb.reshape(1, -1), (128, NB0 * 128)).copy()

    qat = np.zeros((NB0, 8, 4), np.int64)          # local q-atom of (g, j, qq)
    for n, b in enumerate(BLOCKS[0]):
        qat[n] = 32 * b + (4 * np.arange(8)[:, None] + np.arange(4)[None, :])
    posq = np.zeros((24, NB0 * 4), f32)
    uidq = np.zeros((40, NB0 * 4), f32)
    for j in range(8):
        for gc in range(5):
            uidq[gc * 8 + j] = uid[qat[:, j, :]].reshape(-1)
            if gc < 3:
                posq[gc * 8 + j] = pos[qat[:, j, :], gc].reshape(-1)
    posw = np.zeros((24, LOC), f32)
    uidw = np.zeros((40, LOC), f32)
    for j in range(8):
        for gc in range(5):
            uidw[gc * 8 + j] = uid
            if gc < 3:
                posw[gc * 8 + j] = pos[:, gc]

    # a2t row-major, atoms padded 192 -> 2 chunks of 128 stacked on free axis
    a2t_own = i['atom_to_token_index'][0][core * OWN:(core + 1) * OWN].astype(f32)
    a2t_st = np.zeros((128, 2 * N_TOK), f32)
    a2t_st[:, :N_TOK] = a2t_own[0:128]
    a2t_st[0:64, N_TOK:] = a2t_own[128:192]
    d = {'feats': feats.T, 'posq': posq, 'uidq': uidq, 'posw': posw,
         'uidw': uidw, 'mb': mb, 'a2t': a2t_st}
    return {k: np.ascontiguousarray(v.astype(NPBF)) for k, v in d.items()}


def _prep_shared(i):
    f32 = np.float32
    inv_sqrt = 1.0 / np.sqrt(CH)
    Wf = np.asarray(i['W_feats'], f32)
    Wf2 = np.concatenate([Wf[4:132], Wf[133:389], Wf[0:3], Wf[3:4],
                          Wf[132:133], Wf[389:390],
                          np.zeros((122, 128), f32)], axis=0)       # [512,128]

    W5 = np.concatenate([np.asarray(i['W_ref_offset'], f32),
                         np.asarray(i['W_valid'], f32),
                         np.asarray(i['W_inv_sq'], f32)], 0)        # [5,16]
    bdW5 = np.zeros((40, 128), f32)
    for j in range(8):
        for gc in range(5):
            bdW5[gc * 8 + j, j * 16:(j + 1) * 16] = W5[gc]
    bdones3 = np.zeros((24, 8), f32)
    for j in range(8):
        for gc in range(3):
            bdones3[gc * 8 + j, j] = 1.0

    def bd8(w):
        n = w.shape[1]
        o = np.zeros((128, 8 * n), f32)
        for j in range(8):
            o[j * 16:(j + 1) * 16, j * n:(j + 1) * n] = w
        return o

    bdmlp = np.concatenate([bd8(np.asarray(i[f'W_mlp{k}'], f32)) for k in (1, 2, 3)], 1)
    bdones16 = bd8(np.full((16, 16), 1.0 / 16, f32))
    Wb_eff = np.asarray(i['lnz_g'], f32)[:, :, None] * np.asarray(i['Wb'], f32)
    bdWb = np.concatenate([bd8(Wb_eff[l]) for l in range(L)], 1)    # [128, 96]

    def stackL(w):
        return np.concatenate([np.asarray(w[l], f32) for l in range(L)], 1)

    d = {
        'Wf2': Wf2, 'bdW5': bdW5, 'bdones3': bdones3, 'bdones16': bdones16,
        'bdmlp': bdmlp, 'bdWb': bdWb,
        'Wlm': np.concatenate([np.asarray(i['W_l'], f32),
                               np.asarray(i['W_m'], f32)], 1),
        'Wq': stackL(np.asarray(i['Wq'], f32) * inv_sqrt),
        'Wk': stackL(i['Wk']), 'Wv': stackL(i['Wv']),
        'Wgate': stackL(i['Wgate']), 'Wo': stackL(i['Wo']),
        'aWg': stackL(np.asarray(i['attn_ada_Wg'], f32) *
                      np.asarray(i['attn_ada_gamma_s'], f32)[:, :, None]),
        'aWs': stackL(np.asarray(i['attn_ada_Ws'], f32) *
                      np.asarray(i['attn_ada_gamma_s'], f32)[:, :, None]),
        'tWg': stackL(np.asarray(i['tr_ada_Wg'], f32) *
                      np.asarray(i['tr_ada_gamma_s'], f32)[:, :, None]),
        'tWs': stackL(np.asarray(i['tr_ada_Ws'], f32) *
                      np.asarray(i['tr_ada_gamma_s'], f32)[:, :, None]),
        'Wsg': stackL(i['Wsg']), 'tWog': stackL(i['tr_Wog']),
        'trW1': np.concatenate([np.asarray(i['tr_W1'], f32)[l] for l in range(L)], 1),
        'trW2': np.concatenate([np.asarray(i['tr_W2'], f32)[l] for l in range(L)], 1),
        'trWout': np.concatenate(
            [np.asarray(i['tr_Wout'], f32)[l, h * 128:(h + 1) * 128]
             for l in range(L) for h in range(2)], 1),
        'Wot': i['W_out_tok'],
    }
    d = {k: np.ascontiguousarray(np.asarray(v, f32).astype(NPBF)) for k, v in d.items()}
    d['ident'] = np.eye(128, dtype=f32).astype(NPBF)
    pmat = np.zeros((128, 128), f32)       # zt row p'=16j+4h+qq -> m=32h+4j+qq
    for j in range(8):
        for h in range(4):
            for qq in range(4):
                pmat[16 * j + 4 * h + qq, 32 * h + 4 * j + qq] = 1.0
    d['pmat'] = pmat.astype(NPBF)
    d['bq'] = np.ascontiguousarray((np.asarray(i['bq'], f32) * inv_sqrt).T)
    d['bg_a'] = np.ascontiguousarray(np.asarray(i['attn_ada_bg'], f32).T)
    d['bg_t'] = np.ascontiguousarray(np.asarray(i['tr_ada_bg'], f32).T)
    d['bsg'] = np.ascontiguousarray(np.asarray(i['bsg'], f32).T)
    d['tbog'] = np.ascontiguousarray(np.asarray(i['tr_bog'], f32).T)
    return d


# ------------------------------------------------------------ program builder
def build(debug=()):
    nc = bass.Bass()
    TT = nc.vector.tensor_tensor
    TS = nc.vector.tensor_scalar
    ACT = nc.scalar.activation
    MM = nc.tensor.matmul

    def din(name, shape, dt=BF):
        return nc.dram_tensor(name, shape, dt, kind="ExternalInput")[:]

    ins = {}
    for nm, shp in [('posq', [24, NB0 * 4]),
                    ('uidq', [40, NB0 * 4]), ('posw', [24, LOC]),
                    ('uidw', [40, LOC]), ('mb', [128, NB0 * 128]),
                    ('a2t', [128, 2 * N_TOK]),
                    ('bdW5', [40, 128]), ('bdones3', [24, 8]),
                    ('bdones16', [128, 128]), ('bdmlp', [128, 384]),
                    ('bdWb', [128, 32 * L]), ('Wlm', [128, 32]),
                    ('Wq', [128, 128 * L]), ('Wk', [128, 128 * L]),
                    ('Wv', [128, 128 * L]), ('Wgate', [128, 128 * L]),
                    ('Wo', [128, 128 * L]), ('aWg', [128, 128 * L]),
                    ('aWs', [128, 128 * L]), ('tWg', [128, 128 * L]),
                    ('tWs', [128, 128 * L]), ('Wsg', [128, 128 * L]),
                    ('tWog', [128, 128 * L]), ('trW1', [128, 256 * L]),
                    ('trW2', [128, 256 * L]), ('trWout', [128, 128 * 2 * L]),
                    ('Wot', [128, N_TOK])]:
        ins[nm] = din(nm, shp)
    for nm in ('bq', 'bg_a', 'bg_t', 'bsg', 'tbog'):
        ins[nm] = din(nm, [128, L], F32)
    feats_d = din('feats', [512, LOC])
    wf2_d = din('Wf2', [512, 128])
    ins['ident'] = din('ident', [128, 128])
    ins['pmat'] = din('pmat', [128, 128])

    out_part = nc.dram_tensor('part', [N_TOK, N_TOK], F32, kind="ExternalOutput")[:]
    out_cnt = nc.dram_tensor('cnt', [128, 3], F32, kind="ExternalOutput")[:]
    dumps = []

    def dump(name, ap):
        if name not in debug:
            return
        t = nc.dram_tensor('dbg_' + name, list(ap.shape), ap.dtype,
                           kind="ExternalOutput")[:]
        nc.sync.dma_start(out=t, in_=ap)

    with tile.TileContext(nc) as tc, \
         tc.tile_pool(name="const", bufs=1) as constp, \
         tc.tile_pool(name="state", bufs=1) as statep, \
         tc.tile_pool(name="work", bufs=3) as workp, \
         tc.tile_pool(name="abuf", bufs=2) as abufp, \
         tc.tile_pool(name="blk", bufs=16) as blkp, \
         tc.tile_pool(name="psum", bufs=1, space="PSUM") as psp:

        ones_row = constp.tile([1, 128], BF)
        nc.vector.memset(ones_row, 1.0)
        ones_col = constp.tile([128, 1], BF)
        nc.vector.memset(ones_col, 1.0)
        inv128 = constp.tile([128, 128], BF)
        nc.vector.memset(inv128, 1.0 / 128.0)
        epsc = constp.tile([128, 1], F32)
        nc.vector.memset(epsc, EPS)
        onef = constp.tile([128, 1], F32)
        nc.vector.memset(onef, 1.0)
        zeros128 = constp.tile([128, 128], BF)
        nc.vector.memset(zeros128, 0.0)

        _ldn = [0]

        def load(ap, name, pool=constp):
            t = pool.tile(list(ap.shape), ap.dtype, name=name)
            eng = nc.sync if _ldn[0] % 2 == 0 else nc.gpsimd
            _ldn[0] += 1
            eng.dma_start(out=t, in_=ap)
            return t

        s_ = {nm: load(ap, 'w_' + nm) for nm, ap in ins.items()}
        s_feats = [load(feats_d[kk * 128:(kk + 1) * 128, :], f'feats{kk}')
                   for kk in range(4)]
        s_wf2 = [load(wf2_d[kk * 128:(kk + 1) * 128, :], f'wf2_{kk}')
                 for kk in range(4)]

        def ps512():
            return psp.tile([128, 512], F32, tag='ps512', bufs=4, name='ps512')

        def psb(tag='psb'):
            return psp.tile([128, 128], F32, tag='psb', bufs=4, name='psb')

        # ---------------- embed: cl [128, LOC] ----------------
        cl = statep.tile([128, LOC], BF)
        for n0 in (0, 512):
            n1 = min(n0 + 512, LOC)
            ps = ps512()
            for kk in range(4):
                MM(ps[:, :n1 - n0], s_wf2[kk], s_feats[kk][:, n0:n1],
                   start=(kk == 0), stop=(kk == 3))
            TS(cl[:, n0:n1], ps[:, :n1 - n0], 1.0, None, AMUL)
        dump('cl', cl)

        # crelu -> [crl | crm] = crlm [32, LOC]; then crl_bd + crm_rep
        crelu = workp.tile([128, LOC], BF, tag='crelu', bufs=1)
        TS(crelu, cl, 0.0, None, AMAX)
        crlm = workp.tile([32, LOC], BF, tag='crlm', bufs=1)
        for n0 in (0, 512):
            n1 = min(n0 + 512, LOC)
            ps = ps512()
            MM(ps[0:32, :n1 - n0], s_['Wlm'], crelu[:, n0:n1], start=True, stop=True)
            TS(crlm[:, n0:n1], ps[0:32, :n1 - n0], 1.0, None, AMUL)
        crl_bd = statep.tile([128, NB0 * 4], BF)
        crm_rep = statep.tile([128, LOC], BF)
        pz = crlm.ap[0][0]
        for j in range(8):
            nc.gpsimd.dma_start(
                out=crl_bd[j * 16:(j + 1) * 16, :],
                in_=bass.AP(tensor=crlm.tensor, offset=crlm.offset + 64 + 4 * j,
                            ap=[[pz, 16], [32, NB0], [1, 4]]))
            nc.gpsimd.dma_start(out=crm_rep[j * 16:(j + 1) * 16, :],
                              in_=bass.AP(tensor=crlm.tensor,
                                          offset=crlm.offset + 16 * pz,
                                          ap=[[pz, 16], [1, LOC]]))
        dump('crl_bd', crl_bd)

        # ---------------- LN(s) via Newton rsqrt (table-free) ----------------
        sh = statep.tile([128, LOC], BF)
        xcs = workp.tile([128, LOC], BF, tag='xcs', bufs=1)
        vps = workp.tile([128, LOC], F32, tag='vps', bufs=1)   # var+eps
        y0 = workp.tile([128, LOC], F32, tag='nr1', bufs=1)
        t1 = workp.tile([128, LOC], F32, tag='nr2', bufs=1)
        for n0 in (0, 512):
            n1 = min(n0 + 512, LOC)
            w = n1 - n0
            ps = ps512()
            MM(ps[:, :w], inv128, cl[:, n0:n1], start=True, stop=True)
            TT(xcs[:, n0:n1], cl[:, n0:n1], ps[:, :w], ASUB)
            sq = workp.tile([128, 512], BF, tag='sqs')
            TT(sq[:, :w], xcs[:, n0:n1], xcs[:, n0:n1], AMUL)
            ps2 = ps512()
            MM(ps2[:, :w], inv128, sq[:, :w], start=True, stop=True)
            TS(vps[:, n0:n1], ps2[:, :w], EPS, None, AADD)
        v32 = vps.bitcast(I32)
        y32 = y0.bitcast(I32)
        t32 = t1.bitcast(I32)
        nc.vector.memset(t32, RSQRT_MAGIC)
        TS(y32, v32, 1, None, AluOpType.logical_shift_right)
        TT(y32, t32, y32, ASUB)                      # magic - (v>>1)
        for _ in range(2):                           # y *= 1.5 - 0.5*v*y*y
            TT(t1, y0, y0, AMUL)
            TT(t1, vps, t1, AMUL)
            TS(t1, t1, -0.5, 1.5, AMUL, AADD)
            TT(y0, y0, t1, AMUL)
        TT(sh, xcs, y0, AMUL)
        dump('sh', sh)

        # ---------------- s-gate precompute (sigmoid table set) ----------------
        sgate, sa = {}, {}
        for nm, src, bias in (('ga', 'aWg', 'bg_a'), ('gt', 'tWg', 'bg_t'),
                              ('sg', 'Wsg', 'bsg'), ('og', 'tWog', 'tbog')):
            x = sh if nm in ('ga', 'gt') else cl
            for l in range(L):
                g = statep.tile([128, LOC], BF, name=f'{nm}{l}')
                for n0 in (0, 512):
                    n1 = min(n0 + 512, LOC)
                    ps = ps512()
                    MM(ps[:, :n1 - n0], s_[src][:, l * 128:(l + 1) * 128],
                       x[:, n0:n1], start=True, stop=True)
                    ACT(g[:, n0:n1], ps[:, :n1 - n0], AF.Sigmoid,
                        bias=s_[bias][:, l:l + 1])
                sgate[(nm, l)] = g
        for nm, src in (('sa', 'aWs'), ('st', 'tWs')):
            for l in range(L):
                g = statep.tile([128, LOC], BF, name=f'{nm}{l}')
                for n0 in (0, 512):
                    n1 = min(n0 + 512, LOC)
                    ps = ps512()
                    MM(ps[:, :n1 - n0], s_[src][:, l * 128:(l + 1) * 128],
                       sh[:, n0:n1], start=True, stop=True)
                    TS(g[:, n0:n1], ps[:, :n1 - n0], 1.0, None, AMUL)
                sa[(nm, l)] = g
        dump('ga0', sgate[('ga', 0)])

        # ---------------- pair pipeline (fused per 512-chunk) ----------------
        plm = statep.tile([128, ROWS], BF)

        def winap(t, base, rows):
            return bass.AP(tensor=t.tensor, offset=t.offset + base,
                           ap=[[t.ap[0][0], rows], [0, 4], [1, 128]])

        def qapx(t, rows, goff):
            return bass.AP(tensor=t.tensor, offset=t.offset + goff * 4,
                           ap=[[t.ap[0][0], rows], [1, 4], [0, 128]])

        for n in range(NB0):
            base = 32 * (n + 2) - 48
            G = workp.tile([40, 512], BF, tag='G')
            d_raw = workp.tile([24, 512], BF, tag='draw')
            TT(d_raw, winap(s_['posw'], base, 24), qapx(s_['posq'], 24, n), ASUB)
            TT(G, winap(s_['uidw'], base, 40), qapx(s_['uidq'], 40, n),
               AluOpType.is_equal)
            TT(G[0:24, :], d_raw, G[0:24, :], AMUL)
            d2 = workp.tile([24, 512], BF, tag='d2')
            TT(d2, d_raw, d_raw, AMUL)
            psd = ps512()
            MM(psd[32:40, :], s_['bdones3'], d2, start=True, stop=True,
               tile_position=(0, 32))
            lni = workp.tile([40, 512], F32, tag='lni')
            ACT(lni[32:40, :], psd[32:40, :], AF.Ln, bias=onef[32:40, :])
            inv = workp.tile([40, 512], BF, tag='inv')
            ACT(inv[32:40, :], lni[32:40, :], AF.Exp, scale=-1.0)
            TT(G[32:40, :], inv[32:40, :], G[32:40, :], AMUL)

            ps = ps512()
            MM(ps, s_['bdW5'], G, start=True, stop=False)
            MM(ps, s_['ident'], qapx(crl_bd, 128, n), start=False, stop=False,
               skip_group_check=True)
            MM(ps, s_['ident'], winap(crm_rep, base, 128), start=False, stop=True,
               skip_group_check=True)
            ppre = workp.tile([128, 512], BF, tag='ppre')
            r = workp.tile([128, 512], BF, tag='rmlp')
            TS(ppre, ps, 1.0, None, AMUL)
            nc.gpsimd.tensor_scalar(r, ppre, 0.0, None, AMAX)
            pf = None
            for k in range(3):
                psm = ps512()
                MM(psm, s_['bdmlp'][:, k * 128:(k + 1) * 128], r,
                   start=True, stop=(k == 2))
                if k < 2:
                    r = workp.tile([128, 512], BF, tag='rmlp')
                    TS(r, psm, 0.0, None, AMAX)
                else:
                    MM(psm, s_['ident'], ppre, start=False, stop=True,
                       skip_group_check=True)
                    pf = workp.tile([128, 512], BF, tag='pfull')
                    TS(pf, psm, 1.0, None, AMUL)
            psmu = ps512()
            MM(psmu, s_['bdones16'], pf, start=True, stop=True)
            xc = workp.tile([128, 512], BF, tag='xc')
            TT(xc, pf, psmu, ASUB)
            sq2 = workp.tile([128, 512], BF, tag='sq2')
            TT(sq2, xc, xc, AMUL)
            psv = ps512()
            MM(psv, s_['bdones16'], sq2, start=True, stop=True)
            lnv = workp.tile([128, 512], F32, tag='lnv')
            ACT(lnv, psv, AF.Ln, bias=epsc)
            rstd = workp.tile([128, 512], BF, tag='rstdp')
            ACT(rstd, lnv, AF.Exp, scale=-0.5)
            TT(plm[:, n * 512:(n + 1) * 512], xc, rstd, AMUL)
        dump('plm', plm)

        # ---------------- layer loop ----------------
        a_cur = cl
        for l in range(L):
            r0, r1 = RANGES[l]
            blks = BLOCKS[l]
            nb = len(blks)

            ahat = abufp.tile([128, LOC], BF, tag='ahat')
            for c0 in range(r0, r1, 512):
                c1 = min(c0 + 512, r1)
                w = c1 - c0
                ps = ps512()
                MM(ps[:, :w], inv128, a_cur[:, c0:c1], start=True, stop=True)
                xca = abufp.tile([128, 512], BF, tag='xca')
                TT(xca[:, :w], a_cur[:, c0:c1], ps[:, :w], ASUB)
                sqa = abufp.tile([128, 512], BF, tag='sqa')
                TT(sqa[:, :w], xca[:, :w], xca[:, :w], AMUL)
                ps2 = ps512()
                MM(ps2[:, :w], inv128, sqa[:, :w], start=True, stop=True)
                lnva = abufp.tile([128, 512], F32, tag='lnva')
                ACT(lnva[:, :w], ps2[:, :w], AF.Ln, bias=epsc)
                rstda = abufp.tile([128, 512], BF, tag='rstda')
                ACT(rstda[:, :w], lnva[:, :w], AF.Exp, scale=-0.5)
                TT(ahat[:, c0:c1], xca[:, :w], rstda[:, :w], AMUL)
            an = abufp.tile([128, LOC], BF, tag='an')
            tn = abufp.tile([128, LOC], BF, tag='tn')
            TT(an[:, r0:r1], sgate[('ga', l)][:, r0:r1], ahat[:, r0:r1], AMUL)
            TT(an[:, r0:r1], an[:, r0:r1], sa[('sa', l)][:, r0:r1], AADD)
            TT(tn[:, r0:r1], sgate[('gt', l)][:, r0:r1], ahat[:, r0:r1], AMUL)
            TT(tn[:, r0:r1], tn[:, r0:r1], sa[('st', l)][:, r0:r1], AADD)
            if l == 0:
                dump('an0', an)

            q_sb = abufp.tile([128, LOC], BF, tag='q')
            k_sb = abufp.tile([128, LOC], BF, tag='k')
            g_sb = abufp.tile([128, LOC], BF, tag='g')
            for c0 in range(r0, r1, 512):
                c1 = min(c0 + 512, r1)
                w = c1 - c0
                psq = ps512()
                MM(psq[:, :w], s_['Wq'][:, l * 128:(l + 1) * 128], an[:, c0:c1],
                   start=True, stop=True)
                ACT(q_sb[:, c0:c1], psq[:, :w], AF.Identity,
                    bias=s_['bq'][:, l:l + 1])
                psk = ps512()
                MM(psk[:, :w], s_['Wk'][:, l * 128:(l + 1) * 128], an[:, c0:c1],
                   start=True, stop=True)
                TS(k_sb[:, c0:c1], psk[:, :w], 1.0, None, AMUL)
                psg = ps512()
                MM(psg[:, :w], s_['Wgate'][:, l * 128:(l + 1) * 128], an[:, c0:c1],
                   start=True, stop=True)
                ex0 = abufp.tile([128, 512], F32, tag='ex0')
                ACT(ex0[:, :w], psg[:, :w], AF.Exp)
                ACT(ex0[:, :w], ex0[:, :w], AF.Ln, bias=onef)
                TT(ex0[:, :w], psg[:, :w], ex0[:, :w], ASUB)
                ACT(g_sb[:, c0:c1], ex0[:, :w], AF.Exp)

            zbt = {}
            for b in blks:
                ch = b - 2
                psz = ps512()
                MM(psz[0:32, :], s_['bdWb'][:, l * 32:(l + 1) * 32],
                   plm[:, ch * 512:(ch + 1) * 512], start=True, stop=True)
                zs = blkp.tile([32, 512], BF, tag='zs', bufs=4)
                nc.scalar.copy(zs, psz[0:32, :])
                zt = blkp.tile([128, 128], BF, tag='zt', bufs=16)
                pzt = zt.ap[0][0]
                for qq in range(4):
                    nc.gpsimd.dma_start(
                        out=bass.AP(tensor=zt.tensor, offset=zt.offset + qq * pzt,
                                    ap=[[4 * pzt, 32], [1, 128]]),
                        in_=zs[:, 128 * qq:128 * qq + 128])
                zbt[b] = zt
            if l == 0:
                dump('zbt2', zbt[2])

            v_sb = {}
            for b in blks:
                base = 32 * b - 48
                psv2 = psb()
                MM(psv2, an[:, base:base + 128],
                   s_['Wv'][:, l * 128:(l + 1) * 128], start=True, stop=True)
                vt = blkp.tile([128, 128], BF, tag='vt', bufs=16)
                nc.scalar.copy(vt, psv2)
                v_sb[b] = vt

            # pass 1: logits -> exp -> A (accumulate row sums)
            dsum = abufp.tile([128, 16], F32, tag='dsum')
            A_sb = {}
            for n, b in enumerate(blks):
                base = 32 * b - 48
                psl = psb()
                MM(psl, s_['pmat'], zbt[b], start=True, stop=False,
                   skip_group_check=True)
                MM(psl, ones_row, s_['mb'][0:1, (b - 2) * 128:(b - 1) * 128],
                   start=False, stop=False, skip_group_check=True)
                for h in range(4):
                    MM(psl[32 * h:32 * h + 32, :],
                       q_sb[32 * h:32 * h + 32, 32 * b:32 * b + 32],
                       k_sb[32 * h:32 * h + 32, base:base + 128],
                       start=False, stop=(h == 3),
                       tile_position=(32 * h, 32 * h), skip_group_check=True)
                At = blkp.tile([128, 128], BF, tag='At', bufs=16)
                ACT(At, psl, AF.Exp, accum_out=dsum[:, n:n + 1])
                A_sb[b] = At
            if l == 0:
                dump('A2', A_sb[2])
            rd = abufp.tile([128, 16], F32, tag='rd')
            TS(rd[:, :nb], dsum[:, :nb], 1e-9, None, AADD)
            nc.vector.reciprocal(rd[:, :nb], rd[:, :nb])

            # pass 2: normalize A -> transpose -> AV -> O (ch-major)
            Ocm = abufp.tile([128, LOC], F32, tag='Ocm')
            for n, b in enumerate(blks):
                At = A_sb[b]
                TS(At, At, rd[:, n:n + 1], None, AMUL)
                pst = psp.tile([128, 128], BF, tag='psb', bufs=4, name='pstT')
                nc.tensor.transpose(pst, At, s_['ident'])
                ATs = blkp.tile([128, 128], BF, tag='ATs', bufs=8)
                TS(ATs, pst, 1.0, None, AMUL)
                pso = psb()
                MM(pso[:, 0:32], s_['ident'], zeros128[:, 0:32],
                   start=True, stop=False, skip_group_check=True)
                for h in range(4):
                    MM(pso[32 * h:32 * h + 32, 0:32],
                       ATs[:, 32 * h:32 * h + 32],
                       v_sb[b][:, 32 * h:32 * h + 32],
                       start=False, stop=(h == 3),
                       tile_position=(0, 32 * h), skip_group_check=True)
                nc.vector.transpose(Ocm[:, 32 * b:32 * b + 32], pso[:, 0:32])
            if l == 0:
                dump('Ocm0', Ocm)

            # epilogue: gated attn out + transition, next range only
            nr0, nr1 = 32 * blks[0], 32 * blks[-1] + 32
            go = abufp.tile([128, LOC], BF, tag='go')
            TT(go[:, nr0:nr1], g_sb[:, nr0:nr1], Ocm[:, nr0:nr1], AMUL)
            a_nxt = abufp.tile([128, LOC], BF, tag='anxt')
            nc.vector.memset(a_nxt, 0.0)
            for c0 in range(nr0, nr1, 512):
                c1 = min(c0 + 512, nr1)
                w = c1 - c0
                psa = ps512()
                MM(psa[:, :w], s_['Wo'][:, l * 128:(l + 1) * 128], go[:, c0:c1],
                   start=True, stop=True)
                ao = abufp.tile([128, 512], BF, tag='ao')
                TT(ao[:, :w], sgate[('sg', l)][:, c0:c1], psa[:, :w], AMUL)
                hh = []
                for t2 in range(2):
                    psh = ps512()
                    MM(psh[:, :w],
                       s_['trW1'][:, l * 256 + t2 * 128:l * 256 + (t2 + 1) * 128],
                       tn[:, c0:c1], start=True, stop=True)
                    ex = abufp.tile([128, 512], F32, tag='ex')
                    ACT(ex[:, :w], psh[:, :w], AF.Exp)
                    ACT(ex[:, :w], ex[:, :w], AF.Ln, bias=onef)
                    TT(ex[:, :w], psh[:, :w], ex[:, :w], ASUB)
                    ACT(ex[:, :w], ex[:, :w], AF.Exp)
                    s1 = abufp.tile([128, 512], BF, tag='s1')
                    TT(s1[:, :w], psh[:, :w], ex[:, :w], AMUL)
                    psh2 = ps512()
                    MM(psh2[:, :w],
                       s_['trW2'][:, l * 256 + t2 * 128:l * 256 + (t2 + 1) * 128],
                       tn[:, c0:c1], start=True, stop=True)
                    ht = abufp.tile([128, 512], BF, tag=f'hh{t2}')
                    TT(ht[:, :w], s1[:, :w], psh2[:, :w], AMUL)
                    hh.append(ht)
                pst2 = ps512()
                MM(pst2[:, :w], s_['trWout'][:, (l * 2) * 128:(l * 2 + 1) * 128],
                   hh[0][:, :w], start=True, stop=False)
                MM(pst2[:, :w], s_['trWout'][:, (l * 2 + 1) * 128:(l * 2 + 2) * 128],
                   hh[1][:, :w], start=False, stop=True)
                to = abufp.tile([128, 512], BF, tag='to')
                TT(to[:, :w], sgate[('og', l)][:, c0:c1], pst2[:, :w], AMUL)
                TT(a_nxt[:, c0:c1], ao[:, :w], to[:, :w], AADD)
            a_cur = a_nxt
        dump('a3', a_cur)

        # ---------------- final: al -> token partials ----------------
        al_rm = []
        for c in range(2):
            c0 = 192 + 128 * c
            psf = ps512()
            MM(psf[:, :N_TOK], a_cur[:, c0:c0 + 128], s_['Wot'],
               start=True, stop=True)
            alr = statep.tile([128, N_TOK], BF, name=f'alr{c}')
            TS(alr, psf[:, :N_TOK], 0.0, None, AMAX)
            al_rm.append(alr)
        partf = statep.tile([128, N_TOK], F32, name='partf')
        cntf = statep.tile([128, 3], F32, name='cntf')
        for tchunk in range(3):
            psp2 = ps512()
            for c in range(2):
                MM(psp2[:, :N_TOK],
                   s_['a2t'][:, c * N_TOK + tchunk * 128:
                             c * N_TOK + tchunk * 128 + 128],
                   al_rm[c], start=(c == 0), stop=(c == 1))
            TS(partf, psp2[:, :N_TOK], 1.0, None, AMUL)
            nc.sync.dma_start(out=out_part[tchunk * 128:(tchunk + 1) * 128, :],
                              in_=partf)
            psc = psb()
            for c in range(2):
                MM(psc[:, 0:1],
                   s_['a2t'][:, c * N_TOK + tchunk * 128:
                             c * N_TOK + tchunk * 128 + 128],
                   ones_col, start=(c == 0), stop=(c == 1))
            TS(cntf[:, tchunk:tchunk + 1], psc[:, 0:1], 1.0, None, AMUL)
        nc.sync.dma_start(out=out_cnt, in_=cntf)

    return nc


def _legalize_waits(nc, maxw=1):
    """The container's walrus accepts at most one sync-wait per instruction;
    Tile emits several. Split excess waits onto preceding same-engine NoOps
    (each wait is a >= threshold, so sequential waits are equivalent)."""
    for fn in nc.m.functions:
        for b in fn.blocks:
            out = []
            for i in b.instructions:
                si = i.sync_info
                if si is not None and len(si.on_wait) > maxw:
                    waits = list(si.on_wait)
                    k = 0
                    while len(waits) > maxw:
                        chunk, waits = waits[:maxw], waits[maxw:]
                        out.append(mybir.InstNoOp(
                            name=f"{i.name}-wsplit{k}", ins=[], outs=[],
                            engine=i.engine,
                            sync_info=mybir.SyncInfo(on_wait=chunk, on_update=[])))
                        k += 1
                    i.sync_info = mybir.SyncInfo(on_wait=waits,
                                                 on_update=list(si.on_update))
                out.append(i)
            b.instructions = out
    return nc


_CACHE = {}


def _get_nc(debug=()):
    key = tuple(sorted(debug))
    if key not in _CACHE:
        _CACHE[key] = _legalize_waits(build(key))
    return _CACHE[key]


def _maps(inputs):
    i = {k: np.asarray(v) for k, v in inputs.items()}
    shared = _prep_shared(i)
    maps = []
    for c in range(NCORES):
        m = dict(shared)
        m.update(_prep_core(c, i))
        maps.append(m)
    return maps


def kernel(**inputs):
    nc = _get_nc()
    res = run_bass_kernel_spmd(nc, _maps(inputs), list(range(NCORES))).results
    tot = np.zeros((N_TOK, N_TOK), np.float64)
    cnt = np.zeros(N_TOK, np.float64)
    for c in range(NCORES):
        tot += np.asarray(res[c]['part'], np.float64)
        cnt += np.asarray(res[c]['cnt'], np.float64).T.reshape(-1)
    out = tot / np.maximum(cnt, 1.0)[:, None]
    return out[None].astype(np.float32)


# revision 27
# speedup vs baseline: 1.5041x; 1.1944x over previous
"""AtomAttentionEncoder — hand-written Bass/Tile kernel for 8 trn2 NeuronCores.

Sequence-parallel over atoms: each core owns 192 atoms with a 192-atom halo on
each side (576 local atoms, zero inter-core collectives — the host sums the 8
per-core [384,384] token partials). Per core, bf16 channel-major throughout:

- atom activations [128 ch, 576 atoms]; LN stats via ones-matmuls (the PE
  broadcast comes for free), rstd = exp(-0.5*ln(var+eps)) so the whole main
  phase uses only the natural_log_exp activation-table set (one sigmoid-set
  phase precomputes all s-dependent gates, one Newton-rsqrt handles LN(s)).
- pair tensor [(gc,j)->(j,c) 128, rows=(block, q//4, q%4, k)] block-diagonal
  grouping so the 16-channel pair MLP and LN run as 128-wide matmuls; the
  whole pair pipeline is fused per 512-column chunk (one chunk == one block).
- 32x128 block-local attention on validity-shrinking blocks (14/10/6 per
  layer); no-max softmax (logits are tiny; masked keys get -1e4 before exp,
  exactly matching the reference's -1e9 since exp underflows to 0).
"""
import numpy as np
import ml_dtypes

import concourse.bass as bass
import concourse.mybir as mybir
import concourse.tile as tile
from concourse.alu_op_type import AluOpType
from concourse.bass_utils import run_bass_kernel_spmd

BF = mybir.dt.bfloat16
F32 = mybir.dt.float32
I32 = mybir.dt.int32
NPBF = ml_dtypes.bfloat16
AF = mybir.ActivationFunctionType

NCORES, OWN, MARGIN, LOC, PAD = 8, 192, 192, 576, 48
N_ATOM, N_TOK = 1536, 384
H, CH, CP = 4, 32, 16
NB0 = 14                      # layer-0 valid blocks (2..15)
ROWS = NB0 * 512              # pair rows (g, j, qq, k), q = 4j+qq
BIG = 1e4
EPS = 1e-5
L = 3
BLOCKS = [list(range(2, 16)), list(range(4, 14)), list(range(6, 12))]
RANGES = [(0, 576), (64, 512), (128, 448)]
AMUL, AADD, ASUB = AluOpType.mult, AluOpType.add, AluOpType.subtract
AMAX = AluOpType.max
RSQRT_MAGIC = 0x5F3759DF


# ------------------------------------------------------------------ host prep
def _prep_core(core, i):
    f32 = np.float32
    start = core * OWN - MARGIN
    idx = np.clip(start + np.arange(LOC), 0, N_ATOM - 1)
    pos = i['ref_pos'][0][idx].astype(f32)
    uid = i['ref_space_uid'][0][idx].astype(f32)
    feats = np.concatenate([
        i['ref_element'][0][idx],
        i['ref_atom_name_chars'][0].reshape(N_ATOM, -1)[idx],
        pos, i['ref_mask'][0][idx][:, None], i['ref_charge'][0][idx][:, None],
        uid[:, None], np.zeros((LOC, 122), f32)], axis=1)   # [LOC, 512]

    atom_mask = (i['atom_to_token_index'][0][idx] @ i['token_mask'][0]).astype(f32)
    mb = np.zeros((NB0, 128), f32)
    for n, b in enumerate(BLOCKS[0]):
        kl = 32 * b - PAD + np.arange(128)
        kg = start + kl
        ok = (kg >= 0) & (kg < N_ATOM) & (kl >= 0) & (kl < LOC)
        am = atom_mask[np.clip(kl, 0, LOC - 1)] > 0
        mb[n] = (np.where(ok & am, 1.0, 0.0) - 1.0) * BIG
    mb = np.broadcast_to(mb.reshape(1, -1), (128, NB0 * 128)).copy()

    qat = np.zeros((NB0, 8, 4), np.int64)          # local q-atom of (g, j, qq)
    for n, b in enumerate(BLOCKS[0]):
        qat[n] = 32 * b + (4 * np.arange(8)[:, None] + np.arange(4)[None, :])
    posq = np.zeros((24, NB0 * 4), f32)
    uidq = np.zeros((40, NB0 * 4), f32)
    for j in range(8):
        for gc in range(5):
            uidq[gc * 8 + j] = uid[qat[:, j, :]].reshape(-1)
            if gc < 3:
                posq[gc * 8 + j] = pos[qat[:, j, :], gc].reshape(-1)
    posw = np.zeros((24, LOC), f32)
    uidw = np.zeros((40, LOC), f32)
    for j in range(8):
        for gc in range(5):
            uidw[gc * 8 + j] = uid
            if gc < 3:
                posw[gc * 8 + j] = pos[:, gc]

    # a2t row-major, atoms padded 192 -> 2 chunks of 128 stacked on free axis
    a2t_own = i['atom_to_token_index'][0][core * OWN:(core + 1) * OWN].astype(f32)
    a2t_st = np.zeros((128, 2 * N_TOK), f32)
    a2t_st[:, :N_TOK] = a2t_own[0:128]
    a2t_st[0:64, N_TOK:] = a2t_own[128:192]
    d = {'feats': feats.T, 'posq': posq, 'uidq': uidq, 'posw': posw,
         'uidw': uidw, 'mb': mb, 'a2t': a2t_st}
    return {k: np.ascontiguousarray(v.astype(NPBF)) for k, v in d.items()}


def _prep_shared(i):
    f32 = np.float32
    inv_sqrt = 1.0 / np.sqrt(CH)
    Wf = np.asarray(i['W_feats'], f32)
    Wf2 = np.concatenate([Wf[4:132], Wf[133:389], Wf[0:3], Wf[3:4],
                          Wf[132:133], Wf[389:390],
                          np.zeros((122, 128), f32)], axis=0)       # [512,128]

    W5 = np.concatenate([np.asarray(i['W_ref_offset'], f32),
                         np.asarray(i['W_valid'], f32),
                         np.asarray(i['W_inv_sq'], f32)], 0)        # [5,16]
    bdW5 = np.zeros((40, 128), f32)
    for j in range(8):
        for gc in range(5):
            bdW5[gc * 8 + j, j * 16:(j + 1) * 16] = W5[gc]
    bdones3 = np.zeros((24, 8), f32)
    for j in range(8):
        for gc in range(3):
            bdones3[gc * 8 + j, j] = 1.0

    def bd8(w):
        n = w.shape[1]
        o = np.zeros((128, 8 * n), f32)
        for j in range(8):
            o[j * 16:(j + 1) * 16, j * n:(j + 1) * n] = w
        return o

    bdmlp = np.concatenate([bd8(np.asarray(i[f'W_mlp{k}'], f32)) for k in (1, 2, 3)], 1)
    bdones16 = bd8(np.full((16, 16), 1.0 / 16, f32))
    Wb_eff = np.asarray(i['lnz_g'], f32)[:, :, None] * np.asarray(i['Wb'], f32)
    bdWb = np.concatenate([bd8(Wb_eff[l]) for l in range(L)], 1)    # [128, 96]

    def stackL(w):
        return np.concatenate([np.asarray(w[l], f32) for l in range(L)], 1)

    d = {
        'Wf2': Wf2, 'bdW5': bdW5, 'bdones3': bdones3, 'bdones16': bdones16,
        'bdmlp': bdmlp, 'bdWb': bdWb,
        'Wlm': np.concatenate([np.asarray(i['W_l'], f32),
                               np.asarray(i['W_m'], f32)], 1),
        'Wq': stackL(np.asarray(i['Wq'], f32) * inv_sqrt),
        'Wk': stackL(i['Wk']), 'Wv': stackL(i['Wv']),
        'Wgate': stackL(i['Wgate']), 'Wo': stackL(i['Wo']),
        'aWg': stackL(np.asarray(i['attn_ada_Wg'], f32) *
                      np.asarray(i['attn_ada_gamma_s'], f32)[:, :, None]),
        'aWs': stackL(np.asarray(i['attn_ada_Ws'], f32) *
                      np.asarray(i['attn_ada_gamma_s'], f32)[:, :, None]),
        'tWg': stackL(np.asarray(i['tr_ada_Wg'], f32) *
                      np.asarray(i['tr_ada_gamma_s'], f32)[:, :, None]),
        'tWs': stackL(np.asarray(i['tr_ada_Ws'], f32) *
                      np.asarray(i['tr_ada_gamma_s'], f32)[:, :, None]),
        'Wsg': stackL(i['Wsg']), 'tWog': stackL(i['tr_Wog']),
        'trW1': np.concatenate([np.asarray(i['tr_W1'], f32)[l] for l in range(L)], 1),
        'trW2': np.concatenate([np.asarray(i['tr_W2'], f32)[l] for l in range(L)], 1),
        'trWout': np.concatenate(
            [np.asarray(i['tr_Wout'], f32)[l, h * 128:(h + 1) * 128]
             for l in range(L) for h in range(2)], 1),
        'Wot': i['W_out_tok'],
    }
    d = {k: np.ascontiguousarray(np.asarray(v, f32).astype(NPBF)) for k, v in d.items()}
    d['ident'] = np.eye(128, dtype=f32).astype(NPBF)
    pmat = np.zeros((128, 128), f32)       # zt row p'=16j+4h+qq -> m=32h+4j+qq
    for j in range(8):
        for h in range(4):
            for qq in range(4):
                pmat[16 * j + 4 * h + qq, 32 * h + 4 * j + qq] = 1.0
    d['pmat'] = pmat.astype(NPBF)
    d['bq'] = np.ascontiguousarray((np.asarray(i['bq'], f32) * inv_sqrt).T)
    d['bg_a'] = np.ascontiguousarray(np.asarray(i['attn_ada_bg'], f32).T)
    d['bg_t'] = np.ascontiguousarray(np.asarray(i['tr_ada_bg'], f32).T)
    d['bsg'] = np.ascontiguousarray(np.asarray(i['bsg'], f32).T)
    d['tbog'] = np.ascontiguousarray(np.asarray(i['tr_bog'], f32).T)
    return d


# ------------------------------------------------------------ program builder
def build(debug=()):
    nc = bass.Bass()
    TT = nc.vector.tensor_tensor
    TS = nc.vector.tensor_scalar
    ACT = nc.scalar.activation
    MM = nc.tensor.matmul

    def din(name, shape, dt=BF):
        return nc.dram_tensor(name, shape, dt, kind="ExternalInput")[:]

    ins = {}
    for nm, shp in [('posq', [24, NB0 * 4]),
                    ('uidq', [40, NB0 * 4]), ('posw', [24, LOC]),
                    ('uidw', [40, LOC]), ('mb', [128, NB0 * 128]),
                    ('a2t', [128, 2 * N_TOK]),
                    ('bdW5', [40, 128]), ('bdones3', [24, 8]),
                    ('bdones16', [128, 128]), ('bdmlp', [128, 384]),
                    ('bdWb', [128, 32 * L]), ('Wlm', [128, 32]),
                    ('Wq', [128, 128 * L]), ('Wk', [128, 128 * L]),
                    ('Wv', [128, 128 * L]), ('Wgate', [128, 128 * L]),
                    ('Wo', [128, 128 * L]), ('aWg', [128, 128 * L]),
                    ('aWs', [128, 128 * L]), ('tWg', [128, 128 * L]),
                    ('tWs', [128, 128 * L]), ('Wsg', [128, 128 * L]),
                    ('tWog', [128, 128 * L]), ('trW1', [128, 256 * L]),
                    ('trW2', [128, 256 * L]), ('trWout', [128, 128 * 2 * L]),
                    ('Wot', [128, N_TOK])]:
        ins[nm] = din(nm, shp)
    for nm in ('bq', 'bg_a', 'bg_t', 'bsg', 'tbog'):
        ins[nm] = din(nm, [128, L], F32)
    feats_d = din('feats', [512, LOC])
    wf2_d = din('Wf2', [512, 128])
    ins['ident'] = din('ident', [128, 128])
    ins['pmat'] = din('pmat', [128, 128])

    out_part = nc.dram_tensor('part', [N_TOK, N_TOK], F32, kind="ExternalOutput")[:]
    out_cnt = nc.dram_tensor('cnt', [128, 3], F32, kind="ExternalOutput")[:]
    dumps = []

    def dump(name, ap):
        if name not in debug:
            return
        t = nc.dram_tensor('dbg_' + name, list(ap.shape), ap.dtype,
                           kind="ExternalOutput")[:]
        nc.sync.dma_start(out=t, in_=ap)

    with tile.TileContext(nc) as tc, \
         tc.tile_pool(name="const", bufs=1) as constp, \
         tc.tile_pool(name="state", bufs=1) as statep, \
         tc.tile_pool(name="work", bufs=3) as workp, \
         tc.tile_pool(name="abuf", bufs=2) as abufp, \
         tc.tile_pool(name="blk", bufs=16) as blkp, \
         tc.tile_pool(name="psum", bufs=1, space="PSUM") as psp:

        ones_row = constp.tile([1, 128], BF)
        nc.vector.memset(ones_row, 1.0)
        ones_col = constp.tile([128, 1], BF)
        nc.vector.memset(ones_col, 1.0)
        inv128 = constp.tile([128, 128], BF)
        nc.vector.memset(inv128, 1.0 / 128.0)
        epsc = constp.tile([128, 1], F32)
        nc.vector.memset(epsc, EPS)
        onef = constp.tile([128, 1], F32)
        nc.vector.memset(onef, 1.0)
        zeros128 = constp.tile([128, 128], BF)
        nc.vector.memset(zeros128, 0.0)

        def load(ap, name, pool=constp):
            t = pool.tile(list(ap.shape), ap.dtype, name=name)
            nc.sync.dma_start(out=t, in_=ap)
            return t

        s_ = {nm: load(ap, 'w_' + nm) for nm, ap in ins.items()}
        s_feats = [load(feats_d[kk * 128:(kk + 1) * 128, :], f'feats{kk}')
                   for kk in range(4)]
        s_wf2 = [load(wf2_d[kk * 128:(kk + 1) * 128, :], f'wf2_{kk}')
                 for kk in range(4)]

        def ps512():
            return psp.tile([128, 512], F32, tag='ps512', bufs=4, name='ps512')

        def psb(tag='psb'):
            return psp.tile([128, 128], F32, tag='psb', bufs=4, name='psb')

        # ---------------- embed: cl [128, LOC] ----------------
        cl = statep.tile([128, LOC], BF)
        for n0 in (0, 512):
            n1 = min(n0 + 512, LOC)
            ps = ps512()
            for kk in range(4):
                MM(ps[:, :n1 - n0], s_wf2[kk], s_feats[kk][:, n0:n1],
                   start=(kk == 0), stop=(kk == 3))
            TS(cl[:, n0:n1], ps[:, :n1 - n0], 1.0, None, AMUL)
        dump('cl', cl)

        # crelu -> [crl | crm] = crlm [32, LOC]; then crl_bd + crm_rep
        crelu = workp.tile([128, LOC], BF, tag='crelu', bufs=1)
        TS(crelu, cl, 0.0, None, AMAX)
        crlm = workp.tile([32, LOC], BF, tag='crlm', bufs=1)
        for n0 in (0, 512):
            n1 = min(n0 + 512, LOC)
            ps = ps512()
            MM(ps[0:32, :n1 - n0], s_['Wlm'], crelu[:, n0:n1], start=True, stop=True)
            TS(crlm[:, n0:n1], ps[0:32, :n1 - n0], 1.0, None, AMUL)
        crl_bd = statep.tile([128, NB0 * 4], BF)
        crm_rep = statep.tile([128, LOC], BF)
        pz = crlm.ap[0][0]
        for j in range(8):
            nc.gpsimd.dma_start(
                out=crl_bd[j * 16:(j + 1) * 16, :],
                in_=bass.AP(tensor=crlm.tensor, offset=crlm.offset + 64 + 4 * j,
                            ap=[[pz, 16], [32, NB0], [1, 4]]))
            nc.gpsimd.dma_start(out=crm_rep[j * 16:(j + 1) * 16, :],
                              in_=bass.AP(tensor=crlm.tensor,
                                          offset=crlm.offset + 16 * pz,
                                          ap=[[pz, 16], [1, LOC]]))
        dump('crl_bd', crl_bd)

        # ---------------- LN(s) via Newton rsqrt (table-free) ----------------
        sh = statep.tile([128, LOC], BF)
        xcs = workp.tile([128, LOC], BF, tag='xcs', bufs=1)
        vps = workp.tile([128, LOC], F32, tag='vps', bufs=1)   # var+eps
        y0 = workp.tile([128, LOC], F32, tag='nr1', bufs=1)
        t1 = workp.tile([128, LOC], F32, tag='nr2', bufs=1)
        for n0 in (0, 512):
            n1 = min(n0 + 512, LOC)
            w = n1 - n0
            ps = ps512()
            MM(ps[:, :w], inv128, cl[:, n0:n1], start=True, stop=True)
            TT(xcs[:, n0:n1], cl[:, n0:n1], ps[:, :w], ASUB)
            sq = workp.tile([128, 512], BF, tag='sqs')
            TT(sq[:, :w], xcs[:, n0:n1], xcs[:, n0:n1], AMUL)
            ps2 = ps512()
            MM(ps2[:, :w], inv128, sq[:, :w], start=True, stop=True)
            TS(vps[:, n0:n1], ps2[:, :w], EPS, None, AADD)
        v32 = vps.bitcast(I32)
        y32 = y0.bitcast(I32)
        t32 = t1.bitcast(I32)
        nc.vector.memset(t32, RSQRT_MAGIC)
        TS(y32, v32, 1, None, AluOpType.logical_shift_right)
        TT(y32, t32, y32, ASUB)                      # magic - (v>>1)
        for _ in range(2):                           # y *= 1.5 - 0.5*v*y*y
            TT(t1, y0, y0, AMUL)
            TT(t1, vps, t1, AMUL)
            TS(t1, t1, -0.5, 1.5, AMUL, AADD)
            TT(y0, y0, t1, AMUL)
        TT(sh, xcs, y0, AMUL)
        dump('sh', sh)

        # ---------------- s-gate precompute (sigmoid table set) ----------------
        sgate, sa = {}, {}
        for nm, src, bias in (('ga', 'aWg', 'bg_a'), ('gt', 'tWg', 'bg_t'),
                              ('sg', 'Wsg', 'bsg'), ('og', 'tWog', 'tbog')):
            x = sh if nm in ('ga', 'gt') else cl
            for l in range(L):
                g = statep.tile([128, LOC], BF, name=f'{nm}{l}')
                for n0 in (0, 512):
                    n1 = min(n0 + 512, LOC)
                    ps = ps512()
                    MM(ps[:, :n1 - n0], s_[src][:, l * 128:(l + 1) * 128],
                       x[:, n0:n1], start=True, stop=True)
                    ACT(g[:, n0:n1], ps[:, :n1 - n0], AF.Sigmoid,
                        bias=s_[bias][:, l:l + 1])
                sgate[(nm, l)] = g
        for nm, src in (('sa', 'aWs'), ('st', 'tWs')):
            for l in range(L):
                g = statep.tile([128, LOC], BF, name=f'{nm}{l}')
                for n0 in (0, 512):
                    n1 = min(n0 + 512, LOC)
                    ps = ps512()
                    MM(ps[:, :n1 - n0], s_[src][:, l * 128:(l + 1) * 128],
                       sh[:, n0:n1], start=True, stop=True)
                    TS(g[:, n0:n1], ps[:, :n1 - n0], 1.0, None, AMUL)
                sa[(nm, l)] = g
        dump('ga0', sgate[('ga', 0)])

        # ---------------- pair pipeline (fused per 512-chunk) ----------------
        plm = statep.tile([128, ROWS], BF)

        def winap(t, base, rows):
            return bass.AP(tensor=t.tensor, offset=t.offset + base,
                           ap=[[t.ap[0][0], rows], [0, 4], [1, 128]])

        def qapx(t, rows, goff):
            return bass.AP(tensor=t.tensor, offset=t.offset + goff * 4,
                           ap=[[t.ap[0][0], rows], [1, 4], [0, 128]])

        for n in range(NB0):
            base = 32 * (n + 2) - 48
            G = workp.tile([40, 512], BF, tag='G')
            d_raw = workp.tile([24, 512], BF, tag='draw')
            TT(d_raw, winap(s_['posw'], base, 24), qapx(s_['posq'], 24, n), ASUB)
            TT(G, winap(s_['uidw'], base, 40), qapx(s_['uidq'], 40, n),
               AluOpType.is_equal)
            TT(G[0:24, :], d_raw, G[0:24, :], AMUL)
            d2 = workp.tile([24, 512], BF, tag='d2')
            TT(d2, d_raw, d_raw, AMUL)
            psd = ps512()
            MM(psd[32:40, :], s_['bdones3'], d2, start=True, stop=True,
               tile_position=(0, 32))
            lni = workp.tile([40, 512], F32, tag='lni')
            ACT(lni[32:40, :], psd[32:40, :], AF.Ln, bias=onef[32:40, :])
            inv = workp.tile([40, 512], BF, tag='inv')
            ACT(inv[32:40, :], lni[32:40, :], AF.Exp, scale=-1.0)
            TT(G[32:40, :], inv[32:40, :], G[32:40, :], AMUL)

            ps = ps512()
            MM(ps, s_['bdW5'], G, start=True, stop=False)
            MM(ps, s_['ident'], qapx(crl_bd, 128, n), start=False, stop=False,
               skip_group_check=True)
            MM(ps, s_['ident'], winap(crm_rep, base, 128), start=False, stop=True,
               skip_group_check=True)
            ppre = workp.tile([128, 512], BF, tag='ppre')
            r = workp.tile([128, 512], BF, tag='rmlp')
            TS(ppre, ps, 1.0, None, AMUL)
            TS(r, ps, 0.0, None, AMAX)
            pf = None
            for k in range(3):
                psm = ps512()
                MM(psm, s_['bdmlp'][:, k * 128:(k + 1) * 128], r,
                   start=True, stop=(k == 2))
                if k < 2:
                    r = workp.tile([128, 512], BF, tag='rmlp')
                    TS(r, psm, 0.0, None, AMAX)
                else:
                    MM(psm, s_['ident'], ppre, start=False, stop=True,
                       skip_group_check=True)
                    pf = workp.tile([128, 512], BF, tag='pfull')
                    TS(pf, psm, 1.0, None, AMUL)
            psmu = ps512()
            MM(psmu, s_['bdones16'], pf, start=True, stop=True)
            xc = workp.tile([128, 512], BF, tag='xc')
            TT(xc, pf, psmu, ASUB)
            sq2 = workp.tile([128, 512], BF, tag='sq2')
            TT(sq2, xc, xc, AMUL)
            psv = ps512()
            MM(psv, s_['bdones16'], sq2, start=True, stop=True)
            lnv = workp.tile([128, 512], F32, tag='lnv')
            ACT(lnv, psv, AF.Ln, bias=epsc)
            rstd = workp.tile([128, 512], BF, tag='rstdp')
            ACT(rstd, lnv, AF.Exp, scale=-0.5)
            TT(plm[:, n * 512:(n + 1) * 512], xc, rstd, AMUL)
        dump('plm', plm)

        # ---------------- layer loop ----------------
        a_cur = cl
        for l in range(L):
            r0, r1 = RANGES[l]
            blks = BLOCKS[l]
            nb = len(blks)

            ahat = abufp.tile([128, LOC], BF, tag='ahat')
            for c0 in range(r0, r1, 512):
                c1 = min(c0 + 512, r1)
                w = c1 - c0
                ps = ps512()
                MM(ps[:, :w], inv128, a_cur[:, c0:c1], start=True, stop=True)
                xca = abufp.tile([128, 512], BF, tag='xca')
                TT(xca[:, :w], a_cur[:, c0:c1], ps[:, :w], ASUB)
                sqa = abufp.tile([128, 512], BF, tag='sqa')
                TT(sqa[:, :w], xca[:, :w], xca[:, :w], AMUL)
                ps2 = ps512()
                MM(ps2[:, :w], inv128, sqa[:, :w], start=True, stop=True)
                lnva = abufp.tile([128, 512], F32, tag='lnva')
                ACT(lnva[:, :w], ps2[:, :w], AF.Ln, bias=epsc)
                rstda = abufp.tile([128, 512], BF, tag='rstda')
                ACT(rstda[:, :w], lnva[:, :w], AF.Exp, scale=-0.5)
                TT(ahat[:, c0:c1], xca[:, :w], rstda[:, :w], AMUL)
            an = abufp.tile([128, LOC], BF, tag='an')
            tn = abufp.tile([128, LOC], BF, tag='tn')
            TT(an[:, r0:r1], sgate[('ga', l)][:, r0:r1], ahat[:, r0:r1], AMUL)
            TT(an[:, r0:r1], an[:, r0:r1], sa[('sa', l)][:, r0:r1], AADD)
            TT(tn[:, r0:r1], sgate[('gt', l)][:, r0:r1], ahat[:, r0:r1], AMUL)
            TT(tn[:, r0:r1], tn[:, r0:r1], sa[('st', l)][:, r0:r1], AADD)
            if l == 0:
                dump('an0', an)

            q_sb = abufp.tile([128, LOC], BF, tag='q')
            k_sb = abufp.tile([128, LOC], BF, tag='k')
            g_sb = abufp.tile([128, LOC], BF, tag='g')
            for c0 in range(r0, r1, 512):
                c1 = min(c0 + 512, r1)
                w = c1 - c0
                psq = ps512()
                MM(psq[:, :w], s_['Wq'][:, l * 128:(l + 1) * 128], an[:, c0:c1],
                   start=True, stop=True)
                ACT(q_sb[:, c0:c1], psq[:, :w], AF.Identity,
                    bias=s_['bq'][:, l:l + 1])
                psk = ps512()
                MM(psk[:, :w], s_['Wk'][:, l * 128:(l + 1) * 128], an[:, c0:c1],
                   start=True, stop=True)
                TS(k_sb[:, c0:c1], psk[:, :w], 1.0, None, AMUL)
                psg = ps512()
                MM(psg[:, :w], s_['Wgate'][:, l * 128:(l + 1) * 128], an[:, c0:c1],
                   start=True, stop=True)
                ex0 = abufp.tile([128, 512], F32, tag='ex0')
                ACT(ex0[:, :w], psg[:, :w], AF.Exp)
                ACT(ex0[:, :w], ex0[:, :w], AF.Ln, bias=onef)
                TT(ex0[:, :w], psg[:, :w], ex0[:, :w], ASUB)
                ACT(g_sb[:, c0:c1], ex0[:, :w], AF.Exp)

            zbt = {}
            for b in blks:
                ch = b - 2
                psz = ps512()
                MM(psz[0:32, :], s_['bdWb'][:, l * 32:(l + 1) * 32],
                   plm[:, ch * 512:(ch + 1) * 512], start=True, stop=True)
                zs = blkp.tile([32, 512], BF, tag='zs', bufs=4)
                nc.scalar.copy(zs, psz[0:32, :])
                zt = blkp.tile([128, 128], BF, tag='zt', bufs=16)
                pzt = zt.ap[0][0]
                for qq in range(4):
                    nc.gpsimd.dma_start(
                        out=bass.AP(tensor=zt.tensor, offset=zt.offset + qq * pzt,
                                    ap=[[4 * pzt, 32], [1, 128]]),
                        in_=zs[:, 128 * qq:128 * qq + 128])
                zbt[b] = zt
            if l == 0:
                dump('zbt2', zbt[2])

            v_sb = {}
            for b in blks:
                base = 32 * b - 48
                psv2 = psb()
                MM(psv2, an[:, base:base + 128],
                   s_['Wv'][:, l * 128:(l + 1) * 128], start=True, stop=True)
                vt = blkp.tile([128, 128], BF, tag='vt', bufs=16)
                nc.scalar.copy(vt, psv2)
                v_sb[b] = vt

            # pass 1: logits -> exp -> A (accumulate row sums)
            dsum = abufp.tile([128, 16], F32, tag='dsum')
            A_sb = {}
            for n, b in enumerate(blks):
                base = 32 * b - 48
                psl = psb()
                MM(psl, s_['pmat'], zbt[b], start=True, stop=False,
                   skip_group_check=True)
                MM(psl, ones_row, s_['mb'][0:1, (b - 2) * 128:(b - 1) * 128],
                   start=False, stop=False, skip_group_check=True)
                for h in range(4):
                    MM(psl[32 * h:32 * h + 32, :],
                       q_sb[32 * h:32 * h + 32, 32 * b:32 * b + 32],
                       k_sb[32 * h:32 * h + 32, base:base + 128],
                       start=False, stop=(h == 3),
                       tile_position=(32 * h, 32 * h), skip_group_check=True)
                At = blkp.tile([128, 128], BF, tag='At', bufs=16)
                ACT(At, psl, AF.Exp, accum_out=dsum[:, n:n + 1])
                A_sb[b] = At
            if l == 0:
                dump('A2', A_sb[2])
            rd = abufp.tile([128, 16], F32, tag='rd')
            TS(rd[:, :nb], dsum[:, :nb], 1e-9, None, AADD)
            nc.vector.reciprocal(rd[:, :nb], rd[:, :nb])

            # pass 2: normalize A -> transpose -> AV -> O (ch-major)
            Ocm = abufp.tile([128, LOC], F32, tag='Ocm')
            for n, b in enumerate(blks):
                At = A_sb[b]
                TS(At, At, rd[:, n:n + 1], None, AMUL)
                pst = psp.tile([128, 128], BF, tag='psb', bufs=4, name='pstT')
                nc.tensor.transpose(pst, At, s_['ident'])
                ATs = blkp.tile([128, 128], BF, tag='ATs', bufs=8)
                TS(ATs, pst, 1.0, None, AMUL)
                pso = psb()
                MM(pso[:, 0:32], s_['ident'], zeros128[:, 0:32],
                   start=True, stop=False, skip_group_check=True)
                for h in range(4):
                    MM(pso[32 * h:32 * h + 32, 0:32],
                       ATs[:, 32 * h:32 * h + 32],
                       v_sb[b][:, 32 * h:32 * h + 32],
                       start=False, stop=(h == 3),
                       tile_position=(0, 32 * h), skip_group_check=True)
                nc.vector.transpose(Ocm[:, 32 * b:32 * b + 32], pso[:, 0:32])
            if l == 0:
                dump('Ocm0', Ocm)

            # epilogue: gated attn out + transition, next range only
            nr0, nr1 = 32 * blks[0], 32 * blks[-1] + 32
            go = abufp.tile([128, LOC], BF, tag='go')
            TT(go[:, nr0:nr1], g_sb[:, nr0:nr1], Ocm[:, nr0:nr1], AMUL)
            a_nxt = abufp.tile([128, LOC], BF, tag='anxt')
            nc.vector.memset(a_nxt, 0.0)
            for c0 in range(nr0, nr1, 512):
                c1 = min(c0 + 512, nr1)
                w = c1 - c0
                psa = ps512()
                MM(psa[:, :w], s_['Wo'][:, l * 128:(l + 1) * 128], go[:, c0:c1],
                   start=True, stop=True)
                ao = abufp.tile([128, 512], BF, tag='ao')
                TT(ao[:, :w], sgate[('sg', l)][:, c0:c1], psa[:, :w], AMUL)
                hh = []
                for t2 in range(2):
                    psh = ps512()
                    MM(psh[:, :w],
                       s_['trW1'][:, l * 256 + t2 * 128:l * 256 + (t2 + 1) * 128],
                       tn[:, c0:c1], start=True, stop=True)
                    ex = abufp.tile([128, 512], F32, tag='ex')
                    ACT(ex[:, :w], psh[:, :w], AF.Exp)
                    ACT(ex[:, :w], ex[:, :w], AF.Ln, bias=onef)
                    TT(ex[:, :w], psh[:, :w], ex[:, :w], ASUB)
                    ACT(ex[:, :w], ex[:, :w], AF.Exp)
                    s1 = abufp.tile([128, 512], BF, tag='s1')
                    TT(s1[:, :w], psh[:, :w], ex[:, :w], AMUL)
                    psh2 = ps512()
                    MM(psh2[:, :w],
                       s_['trW2'][:, l * 256 + t2 * 128:l * 256 + (t2 + 1) * 128],
                       tn[:, c0:c1], start=True, stop=True)
                    ht = abufp.tile([128, 512], BF, tag=f'hh{t2}')
                    TT(ht[:, :w], s1[:, :w], psh2[:, :w], AMUL)
                    hh.append(ht)
                pst2 = ps512()
                MM(pst2[:, :w], s_['trWout'][:, (l * 2) * 128:(l * 2 + 1) * 128],
                   hh[0][:, :w], start=True, stop=False)
                MM(pst2[:, :w], s_['trWout'][:, (l * 2 + 1) * 128:(l * 2 + 2) * 128],
                   hh[1][:, :w], start=False, stop=True)
                to = abufp.tile([128, 512], BF, tag='to')
                TT(to[:, :w], sgate[('og', l)][:, c0:c1], pst2[:, :w], AMUL)
                TT(a_nxt[:, c0:c1], ao[:, :w], to[:, :w], AADD)
            a_cur = a_nxt
        dump('a3', a_cur)

        # ---------------- final: al -> token partials ----------------
        al_rm = []
        for c in range(2):
            c0 = 192 + 128 * c
            psf = ps512()
            MM(psf[:, :N_TOK], a_cur[:, c0:c0 + 128], s_['Wot'],
               start=True, stop=True)
            alr = statep.tile([128, N_TOK], BF, name=f'alr{c}')
            TS(alr, psf[:, :N_TOK], 0.0, None, AMAX)
            al_rm.append(alr)
        partf = statep.tile([128, N_TOK], F32, name='partf')
        cntf = statep.tile([128, 3], F32, name='cntf')
        for tchunk in range(3):
            psp2 = ps512()
            for c in range(2):
                MM(psp2[:, :N_TOK],
                   s_['a2t'][:, c * N_TOK + tchunk * 128:
                             c * N_TOK + tchunk * 128 + 128],
                   al_rm[c], start=(c == 0), stop=(c == 1))
            TS(partf, psp2[:, :N_TOK], 1.0, None, AMUL)
            nc.sync.dma_start(out=out_part[tchunk * 128:(tchunk + 1) * 128, :],
                              in_=partf)
            psc = psb()
            for c in range(2):
                MM(psc[:, 0:1],
                   s_['a2t'][:, c * N_TOK + tchunk * 128:
                             c * N_TOK + tchunk * 128 + 128],
                   ones_col, start=(c == 0), stop=(c == 1))
            TS(cntf[:, tchunk:tchunk + 1], psc[:, 0:1], 1.0, None, AMUL)
        nc.sync.dma_start(out=out_cnt, in_=cntf)

    return nc


def _legalize_waits(nc, maxw=1):
    """The container's walrus accepts at most one sync-wait per instruction;
    Tile emits several. Split excess waits onto preceding same-engine NoOps
    (each wait is a >= threshold, so sequential waits are equivalent)."""
    for fn in nc.m.functions:
        for b in fn.blocks:
            out = []
            for i in b.instructions:
                si = i.sync_info
                if si is not None and len(si.on_wait) > maxw:
                    waits = list(si.on_wait)
                    k = 0
                    while len(waits) > maxw:
                        chunk, waits = waits[:maxw], waits[maxw:]
                        out.append(mybir.InstNoOp(
                            name=f"{i.name}-wsplit{k}", ins=[], outs=[],
                            engine=i.engine,
                            sync_info=mybir.SyncInfo(on_wait=chunk, on_update=[])))
                        k += 1
                    i.sync_info = mybir.SyncInfo(on_wait=waits,
                                                 on_update=list(si.on_update))
                out.append(i)
            b.instructions = out
    return nc


_CACHE = {}


def _get_nc(debug=()):
    key = tuple(sorted(debug))
    if key not in _CACHE:
        _CACHE[key] = _legalize_waits(build(key))
    return _CACHE[key]


def _maps(inputs):
    i = {k: np.asarray(v) for k, v in inputs.items()}
    shared = _prep_shared(i)
    maps = []
    for c in range(NCORES):
        m = dict(shared)
        m.update(_prep_core(c, i))
        maps.append(m)
    return maps


def kernel(**inputs):
    nc = _get_nc()
    res = run_bass_kernel_spmd(nc, _maps(inputs), list(range(NCORES))).results
    tot = np.zeros((N_TOK, N_TOK), np.float64)
    cnt = np.zeros(N_TOK, np.float64)
    for c in range(NCORES):
        tot += np.asarray(res[c]['part'], np.float64)
        cnt += np.asarray(res[c]['cnt'], np.float64).T.reshape(-1)
    out = tot / np.maximum(cnt, 1.0)[:, None]
    return out[None].astype(np.float32)


# revision 28
# speedup vs baseline: 1.5197x; 1.0103x over previous
"""AtomAttentionEncoder — hand-written Bass/Tile kernel for 8 trn2 NeuronCores.

Sequence-parallel over atoms: each core owns 192 atoms with a 192-atom halo on
each side (576 local atoms, zero inter-core collectives — the host sums the 8
per-core [384,384] token partials). Per core, bf16 channel-major throughout:

- atom activations [128 ch, 576 atoms]; LN stats via ones-matmuls (the PE
  broadcast comes for free), rstd = exp(-0.5*ln(var+eps)) so the whole main
  phase uses only the natural_log_exp activation-table set (one sigmoid-set
  phase precomputes all s-dependent gates, one Newton-rsqrt handles LN(s)).
- pair tensor [(gc,j)->(j,c) 128, rows=(block, q//4, q%4, k)] block-diagonal
  grouping so the 16-channel pair MLP and LN run as 128-wide matmuls; the
  whole pair pipeline is fused per 512-column chunk (one chunk == one block).
- 32x128 block-local attention on validity-shrinking blocks (14/10/6 per
  layer); no-max softmax (logits are tiny; masked keys get -1e4 before exp,
  exactly matching the reference's -1e9 since exp underflows to 0).
"""
import numpy as np
import ml_dtypes

import concourse.bass as bass
import concourse.mybir as mybir
import concourse.tile as tile
from concourse.alu_op_type import AluOpType
from concourse.bass_utils import run_bass_kernel_spmd

BF = mybir.dt.bfloat16
F32 = mybir.dt.float32
I32 = mybir.dt.int32
NPBF = ml_dtypes.bfloat16
AF = mybir.ActivationFunctionType

NCORES, OWN, MARGIN, LOC, PAD = 8, 192, 192, 576, 48
N_ATOM, N_TOK = 1536, 384
H, CH, CP = 4, 32, 16
NB0 = 14                      # layer-0 valid blocks (2..15)
ROWS = NB0 * 512              # pair rows (g, j, qq, k), q = 4j+qq
BIG = 1e4
EPS = 1e-5
L = 3
BLOCKS = [list(range(2, 16)), list(range(4, 14)), list(range(6, 12))]
RANGES = [(0, 576), (64, 512), (128, 448)]
AMUL, AADD, ASUB = AluOpType.mult, AluOpType.add, AluOpType.subtract
AMAX = AluOpType.max
RSQRT_MAGIC = 0x5F3759DF


# ------------------------------------------------------------------ host prep
def _prep_core(core, i):
    f32 = np.float32
    start = core * OWN - MARGIN
    idx = np.clip(start + np.arange(LOC), 0, N_ATOM - 1)
    pos = i['ref_pos'][0][idx].astype(f32)
    uid = i['ref_space_uid'][0][idx].astype(f32)
    feats = np.concatenate([
        i['ref_element'][0][idx],
        i['ref_atom_name_chars'][0].reshape(N_ATOM, -1)[idx],
        pos, i['ref_mask'][0][idx][:, None], i['ref_charge'][0][idx][:, None],
        uid[:, None], np.zeros((LOC, 122), f32)], axis=1)   # [LOC, 512]

    atom_mask = (i['atom_to_token_index'][0][idx] @ i['token_mask'][0]).astype(f32)
    mb = np.zeros((NB0, 128), f32)
    for n, b in enumerate(BLOCKS[0]):
        kl = 32 * b - PAD + np.arange(128)
        kg = start + kl
        ok = (kg >= 0) & (kg < N_ATOM) & (kl >= 0) & (kl < LOC)
        am = atom_mask[np.clip(kl, 0, LOC - 1)] > 0
        mb[n] = (np.where(ok & am, 1.0, 0.0) - 1.0) * BIG
    mb = mb.reshape(1, -1)

    qat = np.zeros((NB0, 8, 4), np.int64)          # local q-atom of (g, j, qq)
    for n, b in enumerate(BLOCKS[0]):
        qat[n] = 32 * b + (4 * np.arange(8)[:, None] + np.arange(4)[None, :])
    posq = np.zeros((24, NB0 * 4), f32)
    uidq = np.zeros((40, NB0 * 4), f32)
    for j in range(8):
        for gc in range(5):
            uidq[gc * 8 + j] = uid[qat[:, j, :]].reshape(-1)
            if gc < 3:
                posq[gc * 8 + j] = pos[qat[:, j, :], gc].reshape(-1)
    posw = np.zeros((24, LOC), f32)
    uidw = np.zeros((40, LOC), f32)
    for j in range(8):
        for gc in range(5):
            uidw[gc * 8 + j] = uid
            if gc < 3:
                posw[gc * 8 + j] = pos[:, gc]

    # a2t row-major, atoms padded 192 -> 2 chunks of 128 stacked on free axis
    a2t_own = i['atom_to_token_index'][0][core * OWN:(core + 1) * OWN].astype(f32)
    a2t_st = np.zeros((128, 2 * N_TOK), f32)
    a2t_st[:, :N_TOK] = a2t_own[0:128]
    a2t_st[0:64, N_TOK:] = a2t_own[128:192]
    d = {'feats': feats.T, 'posq': posq, 'uidq': uidq, 'posw': posw,
         'uidw': uidw, 'mb': mb, 'a2t': a2t_st}
    return {k: np.ascontiguousarray(v.astype(NPBF)) for k, v in d.items()}


def _prep_shared(i):
    f32 = np.float32
    inv_sqrt = 1.0 / np.sqrt(CH)
    Wf = np.asarray(i['W_feats'], f32)
    Wf2 = np.concatenate([Wf[4:132], Wf[133:389], Wf[0:3], Wf[3:4],
                          Wf[132:133], Wf[389:390],
                          np.zeros((122, 128), f32)], axis=0)       # [512,128]

    W5 = np.concatenate([np.asarray(i['W_ref_offset'], f32),
                         np.asarray(i['W_valid'], f32),
                         np.asarray(i['W_inv_sq'], f32)], 0)        # [5,16]
    bdW5 = np.zeros((40, 128), f32)
    for j in range(8):
        for gc in range(5):
            bdW5[gc * 8 + j, j * 16:(j + 1) * 16] = W5[gc]
    bdones3 = np.zeros((24, 8), f32)
    for j in range(8):
        for gc in range(3):
            bdones3[gc * 8 + j, j] = 1.0

    def bd8(w):
        n = w.shape[1]
        o = np.zeros((128, 8 * n), f32)
        for j in range(8):
            o[j * 16:(j + 1) * 16, j * n:(j + 1) * n] = w
        return o

    bdmlp = np.concatenate([bd8(np.asarray(i[f'W_mlp{k}'], f32)) for k in (1, 2, 3)], 1)
    bdones16 = bd8(np.full((16, 16), 1.0 / 16, f32))
    Wb_eff = np.asarray(i['lnz_g'], f32)[:, :, None] * np.asarray(i['Wb'], f32)
    bdWb = np.concatenate([bd8(Wb_eff[l]) for l in range(L)], 1)    # [128, 96]

    def stackL(w):
        return np.concatenate([np.asarray(w[l], f32) for l in range(L)], 1)

    d = {
        'Wf2': Wf2, 'bdW5': bdW5, 'bdones3': bdones3, 'bdones16': bdones16,
        'bdmlp': bdmlp, 'bdWb': bdWb,
        'Wlm': np.concatenate([np.asarray(i['W_l'], f32),
                               np.asarray(i['W_m'], f32)], 1),
        'Wq': stackL(np.asarray(i['Wq'], f32) * inv_sqrt),
        'Wk': stackL(i['Wk']), 'Wv': stackL(i['Wv']),
        'Wgate': stackL(i['Wgate']), 'Wo': stackL(i['Wo']),
        'aWg': stackL(np.asarray(i['attn_ada_Wg'], f32) *
                      np.asarray(i['attn_ada_gamma_s'], f32)[:, :, None]),
        'aWs': stackL(np.asarray(i['attn_ada_Ws'], f32) *
                      np.asarray(i['attn_ada_gamma_s'], f32)[:, :, None]),
        'tWg': stackL(np.asarray(i['tr_ada_Wg'], f32) *
                      np.asarray(i['tr_ada_gamma_s'], f32)[:, :, None]),
        'tWs': stackL(np.asarray(i['tr_ada_Ws'], f32) *
                      np.asarray(i['tr_ada_gamma_s'], f32)[:, :, None]),
        'Wsg': stackL(i['Wsg']), 'tWog': stackL(i['tr_Wog']),
        'trW1': np.concatenate([np.asarray(i['tr_W1'], f32)[l] for l in range(L)], 1),
        'trW2': np.concatenate([np.asarray(i['tr_W2'], f32)[l] for l in range(L)], 1),
        'trWout': np.concatenate(
            [np.asarray(i['tr_Wout'], f32)[l, h * 128:(h + 1) * 128]
             for l in range(L) for h in range(2)], 1),
        'Wot': i['W_out_tok'],
    }
    d = {k: np.ascontiguousarray(np.asarray(v, f32).astype(NPBF)) for k, v in d.items()}
    d['ident'] = np.eye(128, dtype=f32).astype(NPBF)
    pmat = np.zeros((128, 128), f32)       # zt row p'=16j+4h+qq -> m=32h+4j+qq
    for j in range(8):
        for h in range(4):
            for qq in range(4):
                pmat[16 * j + 4 * h + qq, 32 * h + 4 * j + qq] = 1.0
    d['pmat'] = pmat.astype(NPBF)
    d['bq'] = np.ascontiguousarray((np.asarray(i['bq'], f32) * inv_sqrt).T)
    d['bg_a'] = np.ascontiguousarray(np.asarray(i['attn_ada_bg'], f32).T)
    d['bg_t'] = np.ascontiguousarray(np.asarray(i['tr_ada_bg'], f32).T)
    d['bsg'] = np.ascontiguousarray(np.asarray(i['bsg'], f32).T)
    d['tbog'] = np.ascontiguousarray(np.asarray(i['tr_bog'], f32).T)
    return d


# ------------------------------------------------------------ program builder
def build(debug=()):
    nc = bass.Bass()
    TT = nc.vector.tensor_tensor
    TS = nc.vector.tensor_scalar
    ACT = nc.scalar.activation
    MM = nc.tensor.matmul

    def din(name, shape, dt=BF):
        return nc.dram_tensor(name, shape, dt, kind="ExternalInput")[:]

    ins = {}
    for nm, shp in [('posq', [24, NB0 * 4]),
                    ('uidq', [40, NB0 * 4]), ('posw', [24, LOC]),
                    ('uidw', [40, LOC]), ('mb', [1, NB0 * 128]),
                    ('a2t', [128, 2 * N_TOK]),
                    ('bdW5', [40, 128]), ('bdones3', [24, 8]),
                    ('bdones16', [128, 128]), ('bdmlp', [128, 384]),
                    ('bdWb', [128, 32 * L]), ('Wlm', [128, 32]),
                    ('Wq', [128, 128 * L]), ('Wk', [128, 128 * L]),
                    ('Wv', [128, 128 * L]), ('Wgate', [128, 128 * L]),
                    ('Wo', [128, 128 * L]), ('aWg', [128, 128 * L]),
                    ('aWs', [128, 128 * L]), ('tWg', [128, 128 * L]),
                    ('tWs', [128, 128 * L]), ('Wsg', [128, 128 * L]),
                    ('tWog', [128, 128 * L]), ('trW1', [128, 256 * L]),
                    ('trW2', [128, 256 * L]), ('trWout', [128, 128 * 2 * L]),
                    ('Wot', [128, N_TOK])]:
        ins[nm] = din(nm, shp)
    for nm in ('bq', 'bg_a', 'bg_t', 'bsg', 'tbog'):
        ins[nm] = din(nm, [128, L], F32)
    feats_d = din('feats', [512, LOC])
    wf2_d = din('Wf2', [512, 128])
    ins['ident'] = din('ident', [128, 128])
    ins['pmat'] = din('pmat', [128, 128])

    out_part = nc.dram_tensor('part', [N_TOK, N_TOK], F32, kind="ExternalOutput")[:]
    out_cnt = nc.dram_tensor('cnt', [128, 3], F32, kind="ExternalOutput")[:]
    dumps = []

    def dump(name, ap):
        if name not in debug:
            return
        t = nc.dram_tensor('dbg_' + name, list(ap.shape), ap.dtype,
                           kind="ExternalOutput")[:]
        nc.sync.dma_start(out=t, in_=ap)

    with tile.TileContext(nc) as tc, \
         tc.tile_pool(name="const", bufs=1) as constp, \
         tc.tile_pool(name="state", bufs=1) as statep, \
         tc.tile_pool(name="work", bufs=3) as workp, \
         tc.tile_pool(name="abuf", bufs=2) as abufp, \
         tc.tile_pool(name="blk", bufs=16) as blkp, \
         tc.tile_pool(name="psum", bufs=1, space="PSUM") as psp:

        ones_row = constp.tile([1, 128], BF)
        nc.vector.memset(ones_row, 1.0)
        ones_col = constp.tile([128, 1], BF)
        nc.vector.memset(ones_col, 1.0)
        inv128 = constp.tile([128, 128], BF)
        nc.vector.memset(inv128, 1.0 / 128.0)
        epsc = constp.tile([128, 1], F32)
        nc.vector.memset(epsc, EPS)
        onef = constp.tile([128, 1], F32)
        nc.vector.memset(onef, 1.0)
        zeros128 = constp.tile([128, 128], BF)
        nc.vector.memset(zeros128, 0.0)

        def load(ap, name, pool=constp):
            t = pool.tile(list(ap.shape), ap.dtype, name=name)
            nc.sync.dma_start(out=t, in_=ap)
            return t

        s_ = {nm: load(ap, 'w_' + nm) for nm, ap in ins.items()}
        s_feats = [load(feats_d[kk * 128:(kk + 1) * 128, :], f'feats{kk}')
                   for kk in range(4)]
        s_wf2 = [load(wf2_d[kk * 128:(kk + 1) * 128, :], f'wf2_{kk}')
                 for kk in range(4)]

        def ps512():
            return psp.tile([128, 512], F32, tag='ps', bufs=8, name='ps512')

        def psb(tag='psb'):
            return psp.tile([128, 128], F32, tag='ps', bufs=8, name='psb')

        # ---------------- embed: cl [128, LOC] ----------------
        cl = statep.tile([128, LOC], BF)
        for n0 in (0, 512):
            n1 = min(n0 + 512, LOC)
            ps = ps512()
            for kk in range(4):
                MM(ps[:, :n1 - n0], s_wf2[kk], s_feats[kk][:, n0:n1],
                   start=(kk == 0), stop=(kk == 3))
            TS(cl[:, n0:n1], ps[:, :n1 - n0], 1.0, None, AMUL)
        dump('cl', cl)

        # crelu -> [crl | crm] = crlm [32, LOC]; then crl_bd + crm_rep
        crelu = workp.tile([128, LOC], BF, tag='crelu', bufs=1)
        TS(crelu, cl, 0.0, None, AMAX)
        crlm = workp.tile([32, LOC], BF, tag='crlm', bufs=1)
        for n0 in (0, 512):
            n1 = min(n0 + 512, LOC)
            ps = ps512()
            MM(ps[0:32, :n1 - n0], s_['Wlm'], crelu[:, n0:n1], start=True, stop=True)
            TS(crlm[:, n0:n1], ps[0:32, :n1 - n0], 1.0, None, AMUL)
        crl_bd = statep.tile([128, NB0 * 4], BF)
        crm_rep = statep.tile([128, LOC], BF)
        pz = crlm.ap[0][0]
        for j in range(8):
            nc.sync.dma_start(
                out=crl_bd[j * 16:(j + 1) * 16, :],
                in_=bass.AP(tensor=crlm.tensor, offset=crlm.offset + 64 + 4 * j,
                            ap=[[pz, 16], [32, NB0], [1, 4]]))
            nc.sync.dma_start(out=crm_rep[j * 16:(j + 1) * 16, :],
                              in_=bass.AP(tensor=crlm.tensor,
                                          offset=crlm.offset + 16 * pz,
                                          ap=[[pz, 16], [1, LOC]]))
        dump('crl_bd', crl_bd)

        # ---------------- LN(s) via Newton rsqrt (table-free) ----------------
        sh = statep.tile([128, LOC], BF)
        xcs = workp.tile([128, LOC], BF, tag='xcs', bufs=1)
        vps = workp.tile([128, LOC], F32, tag='vps', bufs=1)   # var+eps
        y0 = workp.tile([128, LOC], F32, tag='nr1', bufs=1)
        t1 = workp.tile([128, LOC], F32, tag='nr2', bufs=1)
        for n0 in (0, 512):
            n1 = min(n0 + 512, LOC)
            w = n1 - n0
            ps = ps512()
            MM(ps[:, :w], inv128, cl[:, n0:n1], start=True, stop=True)
            TT(xcs[:, n0:n1], cl[:, n0:n1], ps[:, :w], ASUB)
            sq = workp.tile([128, 512], BF, tag='sqs')
            TT(sq[:, :w], xcs[:, n0:n1], xcs[:, n0:n1], AMUL)
            ps2 = ps512()
            MM(ps2[:, :w], inv128, sq[:, :w], start=True, stop=True)
            TS(vps[:, n0:n1], ps2[:, :w], EPS, None, AADD)
        v32 = vps.bitcast(I32)
        y32 = y0.bitcast(I32)
        t32 = t1.bitcast(I32)
        nc.vector.memset(t32, RSQRT_MAGIC)
        TS(y32, v32, 1, None, AluOpType.logical_shift_right)
        TT(y32, t32, y32, ASUB)                      # magic - (v>>1)
        for _ in range(2):                           # y *= 1.5 - 0.5*v*y*y
            TT(t1, y0, y0, AMUL)
            TT(t1, vps, t1, AMUL)
            TS(t1, t1, -0.5, 1.5, AMUL, AADD)
            TT(y0, y0, t1, AMUL)
        TT(sh, xcs, y0, AMUL)
        dump('sh', sh)

        # ---------------- s-gate precompute (sigmoid table set) ----------------
        sgate, sa = {}, {}
        for nm, src, bias in (('ga', 'aWg', 'bg_a'), ('gt', 'tWg', 'bg_t'),
                              ('sg', 'Wsg', 'bsg'), ('og', 'tWog', 'tbog')):
            x = sh if nm in ('ga', 'gt') else cl
            for l in range(L):
                g = statep.tile([128, LOC], BF, name=f'{nm}{l}')
                for n0 in (0, 512):
                    n1 = min(n0 + 512, LOC)
                    ps = ps512()
                    MM(ps[:, :n1 - n0], s_[src][:, l * 128:(l + 1) * 128],
                       x[:, n0:n1], start=True, stop=True)
                    ACT(g[:, n0:n1], ps[:, :n1 - n0], AF.Sigmoid,
                        bias=s_[bias][:, l:l + 1])
                sgate[(nm, l)] = g
        for nm, src in (('sa', 'aWs'), ('st', 'tWs')):
            for l in range(L):
                g = statep.tile([128, LOC], BF, name=f'{nm}{l}')
                for n0 in (0, 512):
                    n1 = min(n0 + 512, LOC)
                    ps = ps512()
                    MM(ps[:, :n1 - n0], s_[src][:, l * 128:(l + 1) * 128],
                       sh[:, n0:n1], start=True, stop=True)
                    TS(g[:, n0:n1], ps[:, :n1 - n0], 1.0, None, AMUL)
                sa[(nm, l)] = g
        dump('ga0', sgate[('ga', 0)])

        # ---------------- pair pipeline (fused per 512-chunk) ----------------
        plm = statep.tile([128, ROWS], BF)

        def winap(t, base, rows):
            return bass.AP(tensor=t.tensor, offset=t.offset + base,
                           ap=[[t.ap[0][0], rows], [0, 4], [1, 128]])

        def qapx(t, rows, goff):
            return bass.AP(tensor=t.tensor, offset=t.offset + goff * 4,
                           ap=[[t.ap[0][0], rows], [1, 4], [0, 128]])

        for n in range(NB0):
            base = 32 * (n + 2) - 48
            G = workp.tile([40, 512], BF, tag='G')
            d_raw = workp.tile([24, 512], BF, tag='draw')
            TT(d_raw, winap(s_['posw'], base, 24), qapx(s_['posq'], 24, n), ASUB)
            TT(G, winap(s_['uidw'], base, 40), qapx(s_['uidq'], 40, n),
               AluOpType.is_equal)
            TT(G[0:24, :], d_raw, G[0:24, :], AMUL)
            d2 = workp.tile([24, 512], BF, tag='d2')
            TT(d2, d_raw, d_raw, AMUL)
            psd = ps512()
            MM(psd[32:40, :], s_['bdones3'], d2, start=True, stop=True,
               tile_position=(0, 32))
            lni = workp.tile([40, 512], F32, tag='lni')
            ACT(lni[32:40, :], psd[32:40, :], AF.Ln, bias=onef[32:40, :])
            inv = workp.tile([40, 512], BF, tag='inv')
            ACT(inv[32:40, :], lni[32:40, :], AF.Exp, scale=-1.0)
            TT(G[32:40, :], inv[32:40, :], G[32:40, :], AMUL)

            ps = ps512()
            MM(ps, s_['bdW5'], G, start=True, stop=False)
            MM(ps, s_['ident'], qapx(crl_bd, 128, n), start=False, stop=False,
               skip_group_check=True)
            MM(ps, s_['ident'], winap(crm_rep, base, 128), start=False, stop=True,
               skip_group_check=True)
            ppre = workp.tile([128, 512], BF, tag='ppre')
            r = workp.tile([128, 512], BF, tag='rmlp')
            TS(ppre, ps, 1.0, None, AMUL)
            TS(r, ps, 0.0, None, AMAX)
            pf = None
            for k in range(3):
                psm = ps512()
                MM(psm, s_['bdmlp'][:, k * 128:(k + 1) * 128], r,
                   start=True, stop=(k == 2))
                if k < 2:
                    r = workp.tile([128, 512], BF, tag='rmlp')
                    TS(r, psm, 0.0, None, AMAX)
                else:
                    MM(psm, s_['ident'], ppre, start=False, stop=True,
                       skip_group_check=True)
                    pf = workp.tile([128, 512], BF, tag='pfull')
                    TS(pf, psm, 1.0, None, AMUL)
            psmu = ps512()
            MM(psmu, s_['bdones16'], pf, start=True, stop=True)
            xc = workp.tile([128, 512], BF, tag='xc')
            TT(xc, pf, psmu, ASUB)
            sq2 = workp.tile([128, 512], BF, tag='sq2')
            TT(sq2, xc, xc, AMUL)
            psv = ps512()
            MM(psv, s_['bdones16'], sq2, start=True, stop=True)
            lnv = workp.tile([128, 512], F32, tag='lnv')
            ACT(lnv, psv, AF.Ln, bias=epsc)
            rstd = workp.tile([128, 512], BF, tag='rstdp')
            ACT(rstd, lnv, AF.Exp, scale=-0.5)
            TT(plm[:, n * 512:(n + 1) * 512], xc, rstd, AMUL)
        dump('plm', plm)

        # ---------------- layer loop ----------------
        a_cur = cl
        for l in range(L):
            r0, r1 = RANGES[l]
            blks = BLOCKS[l]
            nb = len(blks)

            ahat = abufp.tile([128, LOC], BF, tag='ahat')
            for c0 in range(r0, r1, 512):
                c1 = min(c0 + 512, r1)
                w = c1 - c0
                ps = ps512()
                MM(ps[:, :w], inv128, a_cur[:, c0:c1], start=True, stop=True)
                xca = abufp.tile([128, 512], BF, tag='xca')
                TT(xca[:, :w], a_cur[:, c0:c1], ps[:, :w], ASUB)
                sqa = abufp.tile([128, 512], BF, tag='sqa')
                TT(sqa[:, :w], xca[:, :w], xca[:, :w], AMUL)
                ps2 = ps512()
                MM(ps2[:, :w], inv128, sqa[:, :w], start=True, stop=True)
                lnva = abufp.tile([128, 512], F32, tag='lnva')
                ACT(lnva[:, :w], ps2[:, :w], AF.Ln, bias=epsc)
                rstda = abufp.tile([128, 512], BF, tag='rstda')
                ACT(rstda[:, :w], lnva[:, :w], AF.Exp, scale=-0.5)
                TT(ahat[:, c0:c1], xca[:, :w], rstda[:, :w], AMUL)
            an = abufp.tile([128, LOC], BF, tag='an')
            tn = abufp.tile([128, LOC], BF, tag='tn')
            TT(an[:, r0:r1], sgate[('ga', l)][:, r0:r1], ahat[:, r0:r1], AMUL)
            TT(an[:, r0:r1], an[:, r0:r1], sa[('sa', l)][:, r0:r1], AADD)
            TT(tn[:, r0:r1], sgate[('gt', l)][:, r0:r1], ahat[:, r0:r1], AMUL)
            TT(tn[:, r0:r1], tn[:, r0:r1], sa[('st', l)][:, r0:r1], AADD)
            if l == 0:
                dump('an0', an)

            q_sb = abufp.tile([128, LOC], BF, tag='q')
            k_sb = abufp.tile([128, LOC], BF, tag='k')
            g_sb = abufp.tile([128, LOC], BF, tag='g')
            for c0 in range(r0, r1, 512):
                c1 = min(c0 + 512, r1)
                w = c1 - c0
                psq = ps512()
                MM(psq[:, :w], s_['Wq'][:, l * 128:(l + 1) * 128], an[:, c0:c1],
                   start=True, stop=True)
                ACT(q_sb[:, c0:c1], psq[:, :w], AF.Identity,
                    bias=s_['bq'][:, l:l + 1])
                psk = ps512()
                MM(psk[:, :w], s_['Wk'][:, l * 128:(l + 1) * 128], an[:, c0:c1],
                   start=True, stop=True)
                TS(k_sb[:, c0:c1], psk[:, :w], 1.0, None, AMUL)
                psg = ps512()
                MM(psg[:, :w], s_['Wgate'][:, l * 128:(l + 1) * 128], an[:, c0:c1],
                   start=True, stop=True)
                ex0 = abufp.tile([128, 512], F32, tag='ex0')
                ACT(ex0[:, :w], psg[:, :w], AF.Exp)
                ACT(ex0[:, :w], ex0[:, :w], AF.Ln, bias=onef)
                TT(ex0[:, :w], psg[:, :w], ex0[:, :w], ASUB)
                ACT(g_sb[:, c0:c1], ex0[:, :w], AF.Exp)

            zbt = {}
            for b in blks:
                ch = b - 2
                psz = ps512()
                MM(psz[0:32, :], s_['bdWb'][:, l * 32:(l + 1) * 32],
                   plm[:, ch * 512:(ch + 1) * 512], start=True, stop=True)
                zs = blkp.tile([32, 512], BF, tag='zs', bufs=4)
                nc.scalar.copy(zs, psz[0:32, :])
                zt = blkp.tile([128, 128], BF, tag='zt', bufs=16)
                pzt = zt.ap[0][0]
                for qq in range(4):
                    nc.sync.dma_start(
                        out=bass.AP(tensor=zt.tensor, offset=zt.offset + qq * pzt,
                                    ap=[[4 * pzt, 32], [1, 128]]),
                        in_=zs[:, 128 * qq:128 * qq + 128])
                zbt[b] = zt
            if l == 0:
                dump('zbt2', zbt[2])

            v_sb = {}
            for b in blks:
                base = 32 * b - 48
                psv2 = psb()
                MM(psv2, an[:, base:base + 128],
                   s_['Wv'][:, l * 128:(l + 1) * 128], start=True, stop=True)
                vt = blkp.tile([128, 128], BF, tag='vt', bufs=16)
                nc.scalar.copy(vt, psv2)
                v_sb[b] = vt

            # pass 1: logits -> exp -> A (accumulate row sums)
            dsum = abufp.tile([128, 16], F32, tag='dsum')
            A_sb = {}
            for n, b in enumerate(blks):
                base = 32 * b - 48
                psl = psb()
                MM(psl, s_['pmat'], zbt[b], start=True, stop=False,
                   skip_group_check=True)
                MM(psl, ones_row, s_['mb'][0:1, (b - 2) * 128:(b - 1) * 128],
                   start=False, stop=False, skip_group_check=True)
                for h in range(4):
                    MM(psl[32 * h:32 * h + 32, :],
                       q_sb[32 * h:32 * h + 32, 32 * b:32 * b + 32],
                       k_sb[32 * h:32 * h + 32, base:base + 128],
                       start=False, stop=(h == 3),
                       tile_position=(32 * h, 32 * h), skip_group_check=True)
                At = blkp.tile([128, 128], BF, tag='At', bufs=16)
                ACT(At, psl, AF.Exp, accum_out=dsum[:, n:n + 1])
                A_sb[b] = At
            if l == 0:
                dump('A2', A_sb[2])
            rd = abufp.tile([128, 16], F32, tag='rd')
            TS(rd[:, :nb], dsum[:, :nb], 1e-9, None, AADD)
            nc.vector.reciprocal(rd[:, :nb], rd[:, :nb])

            # pass 2: normalize A -> transpose -> AV -> O (ch-major)
            Ocm = abufp.tile([128, LOC], F32, tag='Ocm')
            for n, b in enumerate(blks):
                At = A_sb[b]
                TS(At, At, rd[:, n:n + 1], None, AMUL)
                pst = psp.tile([128, 128], BF, tag='ps', bufs=8, name='pstT')
                nc.tensor.transpose(pst, At, s_['ident'])
                ATs = blkp.tile([128, 128], BF, tag='ATs', bufs=8)
                TS(ATs, pst, 1.0, None, AMUL)
                pso = psb()
                MM(pso[:, 0:32], s_['ident'], zeros128[:, 0:32],
                   start=True, stop=False, skip_group_check=True)
                for h in range(4):
                    MM(pso[32 * h:32 * h + 32, 0:32],
                       ATs[:, 32 * h:32 * h + 32],
                       v_sb[b][:, 32 * h:32 * h + 32],
                       start=False, stop=(h == 3),
                       tile_position=(0, 32 * h), skip_group_check=True)
                nc.vector.transpose(Ocm[:, 32 * b:32 * b + 32], pso[:, 0:32])
            if l == 0:
                dump('Ocm0', Ocm)

            # epilogue: gated attn out + transition, next range only
            nr0, nr1 = 32 * blks[0], 32 * blks[-1] + 32
            go = abufp.tile([128, LOC], BF, tag='go')
            TT(go[:, nr0:nr1], g_sb[:, nr0:nr1], Ocm[:, nr0:nr1], AMUL)
            a_nxt = abufp.tile([128, LOC], BF, tag='anxt')
            nc.vector.memset(a_nxt, 0.0)
            for c0 in range(nr0, nr1, 512):
                c1 = min(c0 + 512, nr1)
                w = c1 - c0
                psa = ps512()
                MM(psa[:, :w], s_['Wo'][:, l * 128:(l + 1) * 128], go[:, c0:c1],
                   start=True, stop=True)
                ao = abufp.tile([128, 512], BF, tag='ao')
                TT(ao[:, :w], sgate[('sg', l)][:, c0:c1], psa[:, :w], AMUL)
                hh = []
                for t2 in range(2):
                    psh = ps512()
                    MM(psh[:, :w],
                       s_['trW1'][:, l * 256 + t2 * 128:l * 256 + (t2 + 1) * 128],
                       tn[:, c0:c1], start=True, stop=True)
                    ex = abufp.tile([128, 512], F32, tag='ex')
                    ACT(ex[:, :w], psh[:, :w], AF.Exp)
                    ACT(ex[:, :w], ex[:, :w], AF.Ln, bias=onef)
                    TT(ex[:, :w], psh[:, :w], ex[:, :w], ASUB)
                    ACT(ex[:, :w], ex[:, :w], AF.Exp)
                    s1 = abufp.tile([128, 512], BF, tag='s1')
                    TT(s1[:, :w], psh[:, :w], ex[:, :w], AMUL)
                    psh2 = ps512()
                    MM(psh2[:, :w],
                       s_['trW2'][:, l * 256 + t2 * 128:l * 256 + (t2 + 1) * 128],
                       tn[:, c0:c1], start=True, stop=True)
                    ht = abufp.tile([128, 512], BF, tag=f'hh{t2}')
                    TT(ht[:, :w], s1[:, :w], psh2[:, :w], AMUL)
                    hh.append(ht)
                pst2 = ps512()
                MM(pst2[:, :w], s_['trWout'][:, (l * 2) * 128:(l * 2 + 1) * 128],
                   hh[0][:, :w], start=True, stop=False)
                MM(pst2[:, :w], s_['trWout'][:, (l * 2 + 1) * 128:(l * 2 + 2) * 128],
                   hh[1][:, :w], start=False, stop=True)
                to = abufp.tile([128, 512], BF, tag='to')
                TT(to[:, :w], sgate[('og', l)][:, c0:c1], pst2[:, :w], AMUL)
                TT(a_nxt[:, c0:c1], ao[:, :w], to[:, :w], AADD)
            a_cur = a_nxt
        dump('a3', a_cur)

        # ---------------- final: al -> token partials ----------------
        al_rm = []
        for c in range(2):
            c0 = 192 + 128 * c
            psf = ps512()
            MM(psf[:, :N_TOK], a_cur[:, c0:c0 + 128], s_['Wot'],
               start=True, stop=True)
            alr = statep.tile([128, N_TOK], BF, name=f'alr{c}')
            TS(alr, psf[:, :N_TOK], 0.0, None, AMAX)
            al_rm.append(alr)
        partf = statep.tile([128, N_TOK], F32, name='partf')
        cntf = statep.tile([128, 3], F32, name='cntf')
        for tchunk in range(3):
            psp2 = ps512()
            for c in range(2):
                MM(psp2[:, :N_TOK],
                   s_['a2t'][:, c * N_TOK + tchunk * 128:
                             c * N_TOK + tchunk * 128 + 128],
                   al_rm[c], start=(c == 0), stop=(c == 1))
            TS(partf, psp2[:, :N_TOK], 1.0, None, AMUL)
            nc.sync.dma_start(out=out_part[tchunk * 128:(tchunk + 1) * 128, :],
                              in_=partf)
            psc = psb()
            for c in range(2):
                MM(psc[:, 0:1],
                   s_['a2t'][:, c * N_TOK + tchunk * 128:
                             c * N_TOK + tchunk * 128 + 128],
                   ones_col, start=(c == 0), stop=(c == 1))
            TS(cntf[:, tchunk:tchunk + 1], psc[:, 0:1], 1.0, None, AMUL)
        nc.sync.dma_start(out=out_cnt, in_=cntf)

    return nc


def _legalize_waits(nc, maxw=1):
    """The container's walrus accepts at most one sync-wait per instruction;
    Tile emits several. Split excess waits onto preceding same-engine NoOps
    (each wait is a >= threshold, so sequential waits are equivalent)."""
    for fn in nc.m.functions:
        for b in fn.blocks:
            out = []
            for i in b.instructions:
                si = i.sync_info
                if si is not None and len(si.on_wait) > maxw:
                    waits = list(si.on_wait)
                    k = 0
                    while len(waits) > maxw:
                        chunk, waits = waits[:maxw], waits[maxw:]
                        out.append(mybir.InstNoOp(
                            name=f"{i.name}-wsplit{k}", ins=[], outs=[],
                            engine=i.engine,
                            sync_info=mybir.SyncInfo(on_wait=chunk, on_update=[])))
                        k += 1
                    i.sync_info = mybir.SyncInfo(on_wait=waits,
                                                 on_update=list(si.on_update))
                out.append(i)
            b.instructions = out
    return nc


_CACHE = {}


def _get_nc(debug=()):
    key = tuple(sorted(debug))
    if key not in _CACHE:
        _CACHE[key] = _legalize_waits(build(key))
    return _CACHE[key]


def _maps(inputs):
    i = {k: np.asarray(v) for k, v in inputs.items()}
    shared = _prep_shared(i)
    maps = []
    for c in range(NCORES):
        m = dict(shared)
        m.update(_prep_core(c, i))
        maps.append(m)
    return maps


def kernel(**inputs):
    nc = _get_nc()
    res = run_bass_kernel_spmd(nc, _maps(inputs), list(range(NCORES))).results
    tot = np.zeros((N_TOK, N_TOK), np.float64)
    cnt = np.zeros(N_TOK, np.float64)
    for c in range(NCORES):
        tot += np.asarray(res[c]['part'], np.float64)
        cnt += np.asarray(res[c]['cnt'], np.float64).T.reshape(-1)
    out = tot / np.maximum(cnt, 1.0)[:, None]
    return out[None].astype(np.float32)


# revision 29
# speedup vs baseline: 1.5772x; 1.0379x over previous
"""AtomAttentionEncoder — hand-written Bass/Tile kernel for 8 trn2 NeuronCores.

Sequence-parallel over atoms: each core owns 192 atoms with a 192-atom halo on
each side (576 local atoms, zero inter-core collectives — the host sums the 8
per-core [384,384] token partials). Per core, bf16 channel-major throughout:

- atom activations [128 ch, 576 atoms]; LN stats via ones-matmuls (the PE
  broadcast comes for free), rstd = exp(-0.5*ln(var+eps)) so the whole main
  phase uses only the natural_log_exp activation-table set (one sigmoid-set
  phase precomputes all s-dependent gates, one Newton-rsqrt handles LN(s)).
- pair tensor [(gc,j)->(j,c) 128, rows=(block, q//4, q%4, k)] block-diagonal
  grouping so the 16-channel pair MLP and LN run as 128-wide matmuls; the
  whole pair pipeline is fused per 512-column chunk (one chunk == one block).
- 32x128 block-local attention on validity-shrinking blocks (14/10/6 per
  layer); no-max softmax (logits are tiny; masked keys get -1e4 before exp,
  exactly matching the reference's -1e9 since exp underflows to 0).
"""
import numpy as np
import ml_dtypes

import concourse.bass as bass
import concourse.mybir as mybir
import concourse.tile as tile
from concourse.alu_op_type import AluOpType
from concourse.bass_utils import run_bass_kernel_spmd

BF = mybir.dt.bfloat16
F32 = mybir.dt.float32
I32 = mybir.dt.int32
NPBF = ml_dtypes.bfloat16
AF = mybir.ActivationFunctionType

NCORES, OWN, MARGIN, LOC, PAD = 8, 192, 192, 576, 48
N_ATOM, N_TOK = 1536, 384
H, CH, CP = 4, 32, 16
NB0 = 14                      # layer-0 valid blocks (2..15)
ROWS = NB0 * 512              # pair rows (g, j, qq, k), q = 4j+qq
BIG = 1e4
EPS = 1e-5
L = 3
BLOCKS = [list(range(2, 16)), list(range(4, 14)), list(range(6, 12))]
RANGES = [(0, 576), (64, 512), (128, 448)]
AMUL, AADD, ASUB = AluOpType.mult, AluOpType.add, AluOpType.subtract
AMAX = AluOpType.max
RSQRT_MAGIC = 0x5F3759DF


# ------------------------------------------------------------------ host prep
def _prep_core(core, i):
    f32 = np.float32
    start = core * OWN - MARGIN
    idx = np.clip(start + np.arange(LOC), 0, N_ATOM - 1)
    pos = i['ref_pos'][0][idx].astype(f32)
    uid = i['ref_space_uid'][0][idx].astype(f32)
    feats = np.concatenate([
        i['ref_element'][0][idx],
        i['ref_atom_name_chars'][0].reshape(N_ATOM, -1)[idx],
        pos, i['ref_mask'][0][idx][:, None], i['ref_charge'][0][idx][:, None],
        uid[:, None], np.zeros((LOC, 122), f32)], axis=1)   # [LOC, 512]

    atom_mask = (i['atom_to_token_index'][0][idx] @ i['token_mask'][0]).astype(f32)
    mb = np.zeros((NB0, 128), f32)
    for n, b in enumerate(BLOCKS[0]):
        kl = 32 * b - PAD + np.arange(128)
        kg = start + kl
        ok = (kg >= 0) & (kg < N_ATOM) & (kl >= 0) & (kl < LOC)
        am = atom_mask[np.clip(kl, 0, LOC - 1)] > 0
        mb[n] = (np.where(ok & am, 1.0, 0.0) - 1.0) * BIG
    mb = mb.reshape(1, -1)

    qat = np.zeros((NB0, 8, 4), np.int64)          # local q-atom of (g, j, qq)
    for n, b in enumerate(BLOCKS[0]):
        qat[n] = 32 * b + (4 * np.arange(8)[:, None] + np.arange(4)[None, :])
    posq = np.zeros((24, NB0 * 4), f32)
    uidq = np.zeros((40, NB0 * 4), f32)
    for j in range(8):
        for gc in range(5):
            uidq[gc * 8 + j] = uid[qat[:, j, :]].reshape(-1)
            if gc < 3:
                posq[gc * 8 + j] = pos[qat[:, j, :], gc].reshape(-1)
    posw = np.zeros((24, LOC), f32)
    uidw = np.zeros((40, LOC), f32)
    for j in range(8):
        for gc in range(5):
            uidw[gc * 8 + j] = uid
            if gc < 3:
                posw[gc * 8 + j] = pos[:, gc]

    # a2t row-major, atoms padded 192 -> 2 chunks of 128 stacked on free axis
    a2t_own = i['atom_to_token_index'][0][core * OWN:(core + 1) * OWN].astype(f32)
    a2t_st = np.zeros((128, 2 * N_TOK), f32)
    a2t_st[:, :N_TOK] = a2t_own[0:128]
    a2t_st[0:64, N_TOK:] = a2t_own[128:192]
    d = {'feats': feats.T, 'posq': posq, 'uidq': uidq, 'posw': posw,
         'uidw': uidw, 'mb': mb, 'a2t': a2t_st}
    return {k: np.ascontiguousarray(v.astype(NPBF)) for k, v in d.items()}


def _prep_shared(i):
    f32 = np.float32
    inv_sqrt = 1.0 / np.sqrt(CH)
    Wf = np.asarray(i['W_feats'], f32)
    Wf2 = np.concatenate([Wf[4:132], Wf[133:389], Wf[0:3], Wf[3:4],
                          Wf[132:133], Wf[389:390],
                          np.zeros((122, 128), f32)], axis=0)       # [512,128]

    W5 = np.concatenate([np.asarray(i['W_ref_offset'], f32),
                         np.asarray(i['W_valid'], f32),
                         np.asarray(i['W_inv_sq'], f32)], 0)        # [5,16]
    bdW5 = np.zeros((40, 128), f32)
    for j in range(8):
        for gc in range(5):
            bdW5[gc * 8 + j, j * 16:(j + 1) * 16] = W5[gc]
    bdones3 = np.zeros((24, 8), f32)
    for j in range(8):
        for gc in range(3):
            bdones3[gc * 8 + j, j] = 1.0

    def bd8(w):
        n = w.shape[1]
        o = np.zeros((128, 8 * n), f32)
        for j in range(8):
            o[j * 16:(j + 1) * 16, j * n:(j + 1) * n] = w
        return o

    bdmlp = np.concatenate([bd8(np.asarray(i[f'W_mlp{k}'], f32)) for k in (1, 2, 3)], 1)
    bdones16 = bd8(np.full((16, 16), 1.0 / 16, f32))
    Wb_eff = np.asarray(i['lnz_g'], f32)[:, :, None] * np.asarray(i['Wb'], f32)
    bdWb = np.concatenate([bd8(Wb_eff[l]) for l in range(L)], 1)    # [128, 96]

    def stackL(w):
        return np.concatenate([np.asarray(w[l], f32) for l in range(L)], 1)

    d = {
        'Wf2': Wf2, 'bdW5': bdW5, 'bdones3': bdones3, 'bdones16': bdones16,
        'bdmlp': bdmlp, 'bdWb': bdWb,
        'Wlm': np.concatenate([np.asarray(i['W_l'], f32),
                               np.asarray(i['W_m'], f32)], 1),
        'Wq': stackL(np.asarray(i['Wq'], f32) * inv_sqrt),
        'Wk': stackL(i['Wk']), 'Wv': stackL(i['Wv']),
        'Wgate': stackL(i['Wgate']), 'Wo': stackL(i['Wo']),
        'aWg': stackL(np.asarray(i['attn_ada_Wg'], f32) *
                      np.asarray(i['attn_ada_gamma_s'], f32)[:, :, None]),
        'aWs': stackL(np.asarray(i['attn_ada_Ws'], f32) *
                      np.asarray(i['attn_ada_gamma_s'], f32)[:, :, None]),
        'tWg': stackL(np.asarray(i['tr_ada_Wg'], f32) *
                      np.asarray(i['tr_ada_gamma_s'], f32)[:, :, None]),
        'tWs': stackL(np.asarray(i['tr_ada_Ws'], f32) *
                      np.asarray(i['tr_ada_gamma_s'], f32)[:, :, None]),
        'Wsg': stackL(i['Wsg']), 'tWog': stackL(i['tr_Wog']),
        'trW1': np.concatenate([np.asarray(i['tr_W1'], f32)[l] for l in range(L)], 1),
        'trW2': np.concatenate([np.asarray(i['tr_W2'], f32)[l] for l in range(L)], 1),
        'trWout': np.concatenate(
            [np.asarray(i['tr_Wout'], f32)[l, h * 128:(h + 1) * 128]
             for l in range(L) for h in range(2)], 1),
        'Wot': i['W_out_tok'],
    }
    d = {k: np.ascontiguousarray(np.asarray(v, f32).astype(NPBF)) for k, v in d.items()}
    d['ident'] = np.eye(128, dtype=f32).astype(NPBF)
    pmat = np.zeros((128, 128), f32)       # zt row p'=16j+4h+qq -> m=32h+4j+qq
    for j in range(8):
        for h in range(4):
            for qq in range(4):
                pmat[16 * j + 4 * h + qq, 32 * h + 4 * j + qq] = 1.0
    d['pmat'] = pmat.astype(NPBF)
    d['bq'] = np.ascontiguousarray((np.asarray(i['bq'], f32) * inv_sqrt).T)
    d['bg_a'] = np.ascontiguousarray(np.asarray(i['attn_ada_bg'], f32).T)
    d['bg_t'] = np.ascontiguousarray(np.asarray(i['tr_ada_bg'], f32).T)
    d['bsg'] = np.ascontiguousarray(np.asarray(i['bsg'], f32).T)
    d['tbog'] = np.ascontiguousarray(np.asarray(i['tr_bog'], f32).T)
    return d


# ------------------------------------------------------------ program builder
def build(debug=()):
    nc = bass.Bass()
    TT = nc.vector.tensor_tensor
    TS = nc.vector.tensor_scalar
    ACT = nc.scalar.activation
    MM = nc.tensor.matmul

    def din(name, shape, dt=BF):
        return nc.dram_tensor(name, shape, dt, kind="ExternalInput")[:]

    ins = {}
    for nm, shp in [('posq', [24, NB0 * 4]),
                    ('uidq', [40, NB0 * 4]), ('posw', [24, LOC]),
                    ('uidw', [40, LOC]), ('mb', [1, NB0 * 128]),
                    ('a2t', [128, 2 * N_TOK]),
                    ('bdW5', [40, 128]), ('bdones3', [24, 8]),
                    ('bdones16', [128, 128]), ('bdmlp', [128, 384]),
                    ('bdWb', [128, 32 * L]), ('Wlm', [128, 32]),
                    ('Wq', [128, 128 * L]), ('Wk', [128, 128 * L]),
                    ('Wv', [128, 128 * L]), ('Wgate', [128, 128 * L]),
                    ('Wo', [128, 128 * L]), ('aWg', [128, 128 * L]),
                    ('aWs', [128, 128 * L]), ('tWg', [128, 128 * L]),
                    ('tWs', [128, 128 * L]), ('Wsg', [128, 128 * L]),
                    ('tWog', [128, 128 * L]), ('trW1', [128, 256 * L]),
                    ('trW2', [128, 256 * L]), ('trWout', [128, 128 * 2 * L]),
                    ('Wot', [128, N_TOK])]:
        ins[nm] = din(nm, shp)
    for nm in ('bq', 'bg_a', 'bg_t', 'bsg', 'tbog'):
        ins[nm] = din(nm, [128, L], F32)
    feats_d = din('feats', [512, LOC])
    wf2_d = din('Wf2', [512, 128])
    ins['ident'] = din('ident', [128, 128])
    ins['pmat'] = din('pmat', [128, 128])

    out_part = nc.dram_tensor('part', [N_TOK, N_TOK], F32, kind="ExternalOutput")[:]
    out_cnt = nc.dram_tensor('cnt', [128, 3], F32, kind="ExternalOutput")[:]
    dumps = []

    def dump(name, ap):
        if name not in debug:
            return
        t = nc.dram_tensor('dbg_' + name, list(ap.shape), ap.dtype,
                           kind="ExternalOutput")[:]
        nc.sync.dma_start(out=t, in_=ap)

    with tile.TileContext(nc) as tc, \
         tc.tile_pool(name="const", bufs=1) as constp, \
         tc.tile_pool(name="state", bufs=1) as statep, \
         tc.tile_pool(name="work", bufs=3) as workp, \
         tc.tile_pool(name="abuf", bufs=2) as abufp, \
         tc.tile_pool(name="blk", bufs=16) as blkp, \
         tc.tile_pool(name="psum", bufs=1, space="PSUM") as psp:

        ones_row = constp.tile([1, 128], BF)
        nc.vector.memset(ones_row, 1.0)
        ones_col = constp.tile([128, 1], BF)
        nc.vector.memset(ones_col, 1.0)
        inv128 = constp.tile([128, 128], BF)
        nc.vector.memset(inv128, 1.0 / 128.0)
        epsc = constp.tile([128, 1], F32)
        nc.vector.memset(epsc, EPS)
        onef = constp.tile([128, 1], F32)
        nc.vector.memset(onef, 1.0)
        zeros128 = constp.tile([128, 128], BF)
        nc.vector.memset(zeros128, 0.0)

        def load(ap, name, pool=constp):
            t = pool.tile(list(ap.shape), ap.dtype, name=name)
            nc.sync.dma_start(out=t, in_=ap)
            return t

        s_feats = [load(feats_d[kk * 128:(kk + 1) * 128, :], f'feats{kk}')
                   for kk in range(4)]
        s_wf2 = [load(wf2_d[kk * 128:(kk + 1) * 128, :], f'wf2_{kk}')
                 for kk in range(4)]
        _order = ['Wlm', 'posw', 'posq', 'uidw', 'uidq', 'bdW5', 'bdones3',
                  'ident', 'pmat', 'bdmlp', 'bdones16', 'bdWb', 'mb',
                  'aWg', 'aWs', 'tWg', 'tWs', 'Wsg', 'tWog',
                  'bg_a', 'bg_t', 'bsg', 'tbog', 'bq',
                  'Wq', 'Wk', 'Wv', 'Wgate', 'Wo', 'trW1', 'trW2', 'trWout',
                  'Wot', 'a2t']
        assert set(_order) == set(ins), set(ins) ^ set(_order)
        s_ = {nm: load(ins[nm], 'w_' + nm) for nm in _order}

        def ps512():
            return psp.tile([128, 512], F32, tag='ps', bufs=8, name='ps512')

        def psb(tag='psb'):
            return psp.tile([128, 128], F32, tag='ps', bufs=8, name='psb')

        # ---------------- embed: cl [128, LOC] ----------------
        cl = statep.tile([128, LOC], BF)
        for n0 in (0, 512):
            n1 = min(n0 + 512, LOC)
            ps = ps512()
            for kk in range(4):
                MM(ps[:, :n1 - n0], s_wf2[kk], s_feats[kk][:, n0:n1],
                   start=(kk == 0), stop=(kk == 3))
            TS(cl[:, n0:n1], ps[:, :n1 - n0], 1.0, None, AMUL)
        dump('cl', cl)

        # crelu -> [crl | crm] = crlm [32, LOC]; then crl_bd + crm_rep
        crelu = workp.tile([128, LOC], BF, tag='crelu', bufs=1)
        TS(crelu, cl, 0.0, None, AMAX)
        crlm = workp.tile([32, LOC], BF, tag='crlm', bufs=1)
        for n0 in (0, 512):
            n1 = min(n0 + 512, LOC)
            ps = ps512()
            MM(ps[0:32, :n1 - n0], s_['Wlm'], crelu[:, n0:n1], start=True, stop=True)
            TS(crlm[:, n0:n1], ps[0:32, :n1 - n0], 1.0, None, AMUL)
        crl_bd = statep.tile([128, NB0 * 4], BF)
        crm_rep = statep.tile([128, LOC], BF)
        pz = crlm.ap[0][0]
        for j in range(8):
            nc.sync.dma_start(
                out=crl_bd[j * 16:(j + 1) * 16, :],
                in_=bass.AP(tensor=crlm.tensor, offset=crlm.offset + 64 + 4 * j,
                            ap=[[pz, 16], [32, NB0], [1, 4]]))
            nc.sync.dma_start(out=crm_rep[j * 16:(j + 1) * 16, :],
                              in_=bass.AP(tensor=crlm.tensor,
                                          offset=crlm.offset + 16 * pz,
                                          ap=[[pz, 16], [1, LOC]]))
        dump('crl_bd', crl_bd)

        # ---------------- LN(s) via Newton rsqrt (table-free) ----------------
        sh = statep.tile([128, LOC], BF)
        xcs = workp.tile([128, LOC], BF, tag='xcs', bufs=1)
        vps = workp.tile([128, LOC], F32, tag='vps', bufs=1)   # var+eps
        y0 = workp.tile([128, LOC], F32, tag='nr1', bufs=1)
        t1 = workp.tile([128, LOC], F32, tag='nr2', bufs=1)
        for n0 in (0, 512):
            n1 = min(n0 + 512, LOC)
            w = n1 - n0
            ps = ps512()
            MM(ps[:, :w], inv128, cl[:, n0:n1], start=True, stop=True)
            TT(xcs[:, n0:n1], cl[:, n0:n1], ps[:, :w], ASUB)
            sq = workp.tile([128, 512], BF, tag='sqs')
            TT(sq[:, :w], xcs[:, n0:n1], xcs[:, n0:n1], AMUL)
            ps2 = ps512()
            MM(ps2[:, :w], inv128, sq[:, :w], start=True, stop=True)
            TS(vps[:, n0:n1], ps2[:, :w], EPS, None, AADD)
        v32 = vps.bitcast(I32)
        y32 = y0.bitcast(I32)
        t32 = t1.bitcast(I32)
        nc.vector.memset(t32, RSQRT_MAGIC)
        TS(y32, v32, 1, None, AluOpType.logical_shift_right)
        TT(y32, t32, y32, ASUB)                      # magic - (v>>1)
        for _ in range(2):                           # y *= 1.5 - 0.5*v*y*y
            TT(t1, y0, y0, AMUL)
            TT(t1, vps, t1, AMUL)
            TS(t1, t1, -0.5, 1.5, AMUL, AADD)
            TT(y0, y0, t1, AMUL)
        TT(sh, xcs, y0, AMUL)
        dump('sh', sh)

        # ---------------- s-gate precompute (sigmoid table set) ----------------
        sgate, sa = {}, {}
        for nm, src, bias in (('ga', 'aWg', 'bg_a'), ('gt', 'tWg', 'bg_t'),
                              ('sg', 'Wsg', 'bsg'), ('og', 'tWog', 'tbog')):
            x = sh if nm in ('ga', 'gt') else cl
            for l in range(L):
                g = statep.tile([128, LOC], BF, name=f'{nm}{l}')
                for n0 in (0, 512):
                    n1 = min(n0 + 512, LOC)
                    ps = ps512()
                    MM(ps[:, :n1 - n0], s_[src][:, l * 128:(l + 1) * 128],
                       x[:, n0:n1], start=True, stop=True)
                    ACT(g[:, n0:n1], ps[:, :n1 - n0], AF.Sigmoid,
                        bias=s_[bias][:, l:l + 1])
                sgate[(nm, l)] = g
        for nm, src in (('sa', 'aWs'), ('st', 'tWs')):
            for l in range(L):
                g = statep.tile([128, LOC], BF, name=f'{nm}{l}')
                for n0 in (0, 512):
                    n1 = min(n0 + 512, LOC)
                    ps = ps512()
                    MM(ps[:, :n1 - n0], s_[src][:, l * 128:(l + 1) * 128],
                       sh[:, n0:n1], start=True, stop=True)
                    TS(g[:, n0:n1], ps[:, :n1 - n0], 1.0, None, AMUL)
                sa[(nm, l)] = g
        dump('ga0', sgate[('ga', 0)])

        # ---------------- pair pipeline (fused per 512-chunk) ----------------
        plm = statep.tile([128, ROWS], BF)

        def winap(t, base, rows):
            return bass.AP(tensor=t.tensor, offset=t.offset + base,
                           ap=[[t.ap[0][0], rows], [0, 4], [1, 128]])

        def qapx(t, rows, goff):
            return bass.AP(tensor=t.tensor, offset=t.offset + goff * 4,
                           ap=[[t.ap[0][0], rows], [1, 4], [0, 128]])

        for n in range(NB0):
            base = 32 * (n + 2) - 48
            G = workp.tile([40, 512], BF, tag='G')
            d_raw = workp.tile([24, 512], BF, tag='draw')
            TT(d_raw, winap(s_['posw'], base, 24), qapx(s_['posq'], 24, n), ASUB)
            TT(G, winap(s_['uidw'], base, 40), qapx(s_['uidq'], 40, n),
               AluOpType.is_equal)
            TT(G[0:24, :], d_raw, G[0:24, :], AMUL)
            d2 = workp.tile([24, 512], BF, tag='d2')
            TT(d2, d_raw, d_raw, AMUL)
            psd = ps512()
            MM(psd[32:40, :], s_['bdones3'], d2, start=True, stop=True,
               tile_position=(0, 32))
            lni = workp.tile([40, 512], F32, tag='lni')
            ACT(lni[32:40, :], psd[32:40, :], AF.Ln, bias=onef[32:40, :])
            inv = workp.tile([40, 512], BF, tag='inv')
            ACT(inv[32:40, :], lni[32:40, :], AF.Exp, scale=-1.0)
            TT(G[32:40, :], inv[32:40, :], G[32:40, :], AMUL)

            ps = ps512()
            MM(ps, s_['bdW5'], G, start=True, stop=False)
            MM(ps, s_['ident'], qapx(crl_bd, 128, n), start=False, stop=False,
               skip_group_check=True)
            MM(ps, s_['ident'], winap(crm_rep, base, 128), start=False, stop=True,
               skip_group_check=True)
            ppre = workp.tile([128, 512], BF, tag='ppre')
            r = workp.tile([128, 512], BF, tag='rmlp')
            TS(ppre, ps, 1.0, None, AMUL)
            TS(r, ps, 0.0, None, AMAX)
            pf = None
            for k in range(3):
                psm = ps512()
                MM(psm, s_['bdmlp'][:, k * 128:(k + 1) * 128], r,
                   start=True, stop=(k == 2))
                if k < 2:
                    r = workp.tile([128, 512], BF, tag='rmlp')
                    TS(r, psm, 0.0, None, AMAX)
                else:
                    MM(psm, s_['ident'], ppre, start=False, stop=True,
                       skip_group_check=True)
                    pf = workp.tile([128, 512], BF, tag='pfull')
                    TS(pf, psm, 1.0, None, AMUL)
            psmu = ps512()
            MM(psmu, s_['bdones16'], pf, start=True, stop=True)
            xc = workp.tile([128, 512], BF, tag='xc')
            TT(xc, pf, psmu, ASUB)
            sq2 = workp.tile([128, 512], BF, tag='sq2')
            TT(sq2, xc, xc, AMUL)
            psv = ps512()
            MM(psv, s_['bdones16'], sq2, start=True, stop=True)
            lnv = workp.tile([128, 512], F32, tag='lnv')
            ACT(lnv, psv, AF.Ln, bias=epsc)
            rstd = workp.tile([128, 512], BF, tag='rstdp')
            ACT(rstd, lnv, AF.Exp, scale=-0.5)
            TT(plm[:, n * 512:(n + 1) * 512], xc, rstd, AMUL)
        dump('plm', plm)

        # ---------------- layer loop ----------------
        a_cur = cl
        for l in range(L):
            r0, r1 = RANGES[l]
            blks = BLOCKS[l]
            nb = len(blks)

            ahat = abufp.tile([128, LOC], BF, tag='ahat')
            for c0 in range(r0, r1, 512):
                c1 = min(c0 + 512, r1)
                w = c1 - c0
                ps = ps512()
                MM(ps[:, :w], inv128, a_cur[:, c0:c1], start=True, stop=True)
                xca = abufp.tile([128, 512], BF, tag='xca')
                TT(xca[:, :w], a_cur[:, c0:c1], ps[:, :w], ASUB)
                sqa = abufp.tile([128, 512], BF, tag='sqa')
                TT(sqa[:, :w], xca[:, :w], xca[:, :w], AMUL)
                ps2 = ps512()
                MM(ps2[:, :w], inv128, sqa[:, :w], start=True, stop=True)
                lnva = abufp.tile([128, 512], F32, tag='lnva')
                ACT(lnva[:, :w], ps2[:, :w], AF.Ln, bias=epsc)
                rstda = abufp.tile([128, 512], BF, tag='rstda')
                ACT(rstda[:, :w], lnva[:, :w], AF.Exp, scale=-0.5)
                TT(ahat[:, c0:c1], xca[:, :w], rstda[:, :w], AMUL)
            an = abufp.tile([128, LOC], BF, tag='an')
            tn = abufp.tile([128, LOC], BF, tag='tn')
            TT(an[:, r0:r1], sgate[('ga', l)][:, r0:r1], ahat[:, r0:r1], AMUL)
            TT(an[:, r0:r1], an[:, r0:r1], sa[('sa', l)][:, r0:r1], AADD)
            TT(tn[:, r0:r1], sgate[('gt', l)][:, r0:r1], ahat[:, r0:r1], AMUL)
            TT(tn[:, r0:r1], tn[:, r0:r1], sa[('st', l)][:, r0:r1], AADD)
            if l == 0:
                dump('an0', an)

            q_sb = abufp.tile([128, LOC], BF, tag='q')
            k_sb = abufp.tile([128, LOC], BF, tag='k')
            g_sb = abufp.tile([128, LOC], BF, tag='g')
            for c0 in range(r0, r1, 512):
                c1 = min(c0 + 512, r1)
                w = c1 - c0
                psq = ps512()
                MM(psq[:, :w], s_['Wq'][:, l * 128:(l + 1) * 128], an[:, c0:c1],
                   start=True, stop=True)
                ACT(q_sb[:, c0:c1], psq[:, :w], AF.Identity,
                    bias=s_['bq'][:, l:l + 1])
                psk = ps512()
                MM(psk[:, :w], s_['Wk'][:, l * 128:(l + 1) * 128], an[:, c0:c1],
                   start=True, stop=True)
                TS(k_sb[:, c0:c1], psk[:, :w], 1.0, None, AMUL)
                psg = ps512()
                MM(psg[:, :w], s_['Wgate'][:, l * 128:(l + 1) * 128], an[:, c0:c1],
                   start=True, stop=True)
                ex0 = abufp.tile([128, 512], F32, tag='ex0')
                ACT(ex0[:, :w], psg[:, :w], AF.Exp)
                ACT(ex0[:, :w], ex0[:, :w], AF.Ln, bias=onef)
                TT(ex0[:, :w], psg[:, :w], ex0[:, :w], ASUB)
                ACT(g_sb[:, c0:c1], ex0[:, :w], AF.Exp)

            zbt = {}
            for b in blks:
                ch = b - 2
                psz = ps512()
                MM(psz[0:32, :], s_['bdWb'][:, l * 32:(l + 1) * 32],
                   plm[:, ch * 512:(ch + 1) * 512], start=True, stop=True)
                zs = blkp.tile([32, 512], BF, tag='zs', bufs=4)
                nc.scalar.copy(zs, psz[0:32, :])
                zt = blkp.tile([128, 128], BF, tag='zt', bufs=16)
                pzt = zt.ap[0][0]
                for qq in range(4):
                    nc.sync.dma_start(
                        out=bass.AP(tensor=zt.tensor, offset=zt.offset + qq * pzt,
                                    ap=[[4 * pzt, 32], [1, 128]]),
                        in_=zs[:, 128 * qq:128 * qq + 128])
                zbt[b] = zt
            if l == 0:
                dump('zbt2', zbt[2])

            v_sb = {}
            for b in blks:
                base = 32 * b - 48
                psv2 = psb()
                MM(psv2, an[:, base:base + 128],
                   s_['Wv'][:, l * 128:(l + 1) * 128], start=True, stop=True)
                vt = blkp.tile([128, 128], BF, tag='vt', bufs=16)
                nc.scalar.copy(vt, psv2)
                v_sb[b] = vt

            # pass 1: logits -> exp -> A (accumulate row sums)
            dsum = abufp.tile([128, 16], F32, tag='dsum')
            A_sb = {}
            for n, b in enumerate(blks):
                base = 32 * b - 48
                psl = psb()
                MM(psl, s_['pmat'], zbt[b], start=True, stop=False,
                   skip_group_check=True)
                MM(psl, ones_row, s_['mb'][0:1, (b - 2) * 128:(b - 1) * 128],
                   start=False, stop=False, skip_group_check=True)
                for h in range(4):
                    MM(psl[32 * h:32 * h + 32, :],
                       q_sb[32 * h:32 * h + 32, 32 * b:32 * b + 32],
                       k_sb[32 * h:32 * h + 32, base:base + 128],
                       start=False, stop=(h == 3),
                       tile_position=(32 * h, 32 * h), skip_group_check=True)
                At = blkp.tile([128, 128], BF, tag='At', bufs=16)
                ACT(At, psl, AF.Exp, accum_out=dsum[:, n:n + 1])
                A_sb[b] = At
            if l == 0:
                dump('A2', A_sb[2])
            rd = abufp.tile([128, 16], F32, tag='rd')
            TS(rd[:, :nb], dsum[:, :nb], 1e-9, None, AADD)
            nc.vector.reciprocal(rd[:, :nb], rd[:, :nb])

            # pass 2: normalize A -> transpose -> AV -> O (ch-major)
            Ocm = abufp.tile([128, LOC], F32, tag='Ocm')
            for n, b in enumerate(blks):
                At = A_sb[b]
                TS(At, At, rd[:, n:n + 1], None, AMUL)
                pst = psp.tile([128, 128], BF, tag='ps', bufs=8, name='pstT')
                nc.tensor.transpose(pst, At, s_['ident'])
                ATs = blkp.tile([128, 128], BF, tag='ATs', bufs=8)
                TS(ATs, pst, 1.0, None, AMUL)
                pso = psb()
                MM(pso[:, 0:32], s_['ident'], zeros128[:, 0:32],
                   start=True, stop=False, skip_group_check=True)
                for h in range(4):
                    MM(pso[32 * h:32 * h + 32, 0:32],
                       ATs[:, 32 * h:32 * h + 32],
                       v_sb[b][:, 32 * h:32 * h + 32],
                       start=False, stop=(h == 3),
                       tile_position=(0, 32 * h), skip_group_check=True)
                nc.vector.transpose(Ocm[:, 32 * b:32 * b + 32], pso[:, 0:32])
            if l == 0:
                dump('Ocm0', Ocm)

            # epilogue: gated attn out + transition, next range only
            nr0, nr1 = 32 * blks[0], 32 * blks[-1] + 32
            go = abufp.tile([128, LOC], BF, tag='go')
            TT(go[:, nr0:nr1], g_sb[:, nr0:nr1], Ocm[:, nr0:nr1], AMUL)
            a_nxt = abufp.tile([128, LOC], BF, tag='anxt')
            nc.vector.memset(a_nxt, 0.0)
            for c0 in range(nr0, nr1, 512):
                c1 = min(c0 + 512, nr1)
                w = c1 - c0
                psa = ps512()
                MM(psa[:, :w], s_['Wo'][:, l * 128:(l + 1) * 128], go[:, c0:c1],
                   start=True, stop=True)
                ao = abufp.tile([128, 512], BF, tag='ao')
                TT(ao[:, :w], sgate[('sg', l)][:, c0:c1], psa[:, :w], AMUL)
                hh = []
                for t2 in range(2):
                    psh = ps512()
                    MM(psh[:, :w],
                       s_['trW1'][:, l * 256 + t2 * 128:l * 256 + (t2 + 1) * 128],
                       tn[:, c0:c1], start=True, stop=True)
                    ex = abufp.tile([128, 512], F32, tag='ex')
                    ACT(ex[:, :w], psh[:, :w], AF.Exp)
                    ACT(ex[:, :w], ex[:, :w], AF.Ln, bias=onef)
                    TT(ex[:, :w], psh[:, :w], ex[:, :w], ASUB)
                    ACT(ex[:, :w], ex[:, :w], AF.Exp)
                    s1 = abufp.tile([128, 512], BF, tag='s1')
                    TT(s1[:, :w], psh[:, :w], ex[:, :w], AMUL)
                    psh2 = ps512()
                    MM(psh2[:, :w],
                       s_['trW2'][:, l * 256 + t2 * 128:l * 256 + (t2 + 1) * 128],
                       tn[:, c0:c1], start=True, stop=True)
                    ht = abufp.tile([128, 512], BF, tag=f'hh{t2}')
                    TT(ht[:, :w], s1[:, :w], psh2[:, :w], AMUL)
                    hh.append(ht)
                pst2 = ps512()
                MM(pst2[:, :w], s_['trWout'][:, (l * 2) * 128:(l * 2 + 1) * 128],
                   hh[0][:, :w], start=True, stop=False)
                MM(pst2[:, :w], s_['trWout'][:, (l * 2 + 1) * 128:(l * 2 + 2) * 128],
                   hh[1][:, :w], start=False, stop=True)
                to = abufp.tile([128, 512], BF, tag='to')
                TT(to[:, :w], sgate[('og', l)][:, c0:c1], pst2[:, :w], AMUL)
                TT(a_nxt[:, c0:c1], ao[:, :w], to[:, :w], AADD)
            a_cur = a_nxt
        dump('a3', a_cur)

        # ---------------- final: al -> token partials ----------------
        al_rm = []
        for c in range(2):
            c0 = 192 + 128 * c
            psf = ps512()
            MM(psf[:, :N_TOK], a_cur[:, c0:c0 + 128], s_['Wot'],
               start=True, stop=True)
            alr = statep.tile([128, N_TOK], BF, name=f'alr{c}')
            TS(alr, psf[:, :N_TOK], 0.0, None, AMAX)
            al_rm.append(alr)
        partf = statep.tile([128, N_TOK], F32, name='partf')
        cntf = statep.tile([128, 3], F32, name='cntf')
        for tchunk in range(3):
            psp2 = ps512()
            for c in range(2):
                MM(psp2[:, :N_TOK],
                   s_['a2t'][:, c * N_TOK + tchunk * 128:
                             c * N_TOK + tchunk * 128 + 128],
                   al_rm[c], start=(c == 0), stop=(c == 1))
            TS(partf, psp2[:, :N_TOK], 1.0, None, AMUL)
            nc.sync.dma_start(out=out_part[tchunk * 128:(tchunk + 1) * 128, :],
                              in_=partf)
            psc = psb()
            for c in range(2):
                MM(psc[:, 0:1],
                   s_['a2t'][:, c * N_TOK + tchunk * 128:
                             c * N_TOK + tchunk * 128 + 128],
                   ones_col, start=(c == 0), stop=(c == 1))
            TS(cntf[:, tchunk:tchunk + 1], psc[:, 0:1], 1.0, None, AMUL)
        nc.sync.dma_start(out=out_cnt, in_=cntf)

    return nc


def _legalize_waits(nc, maxw=1):
    """The container's walrus accepts at most one sync-wait per instruction;
    Tile emits several. Split excess waits onto preceding same-engine NoOps
    (each wait is a >= threshold, so sequential waits are equivalent)."""
    for fn in nc.m.functions:
        for b in fn.blocks:
            out = []
            for i in b.instructions:
                si = i.sync_info
                if si is not None and len(si.on_wait) > maxw:
                    waits = list(si.on_wait)
                    k = 0
                    while len(waits) > maxw:
                        chunk, waits = waits[:maxw], waits[maxw:]
                        out.append(mybir.InstNoOp(
                            name=f"{i.name}-wsplit{k}", ins=[], outs=[],
                            engine=i.engine,
                            sync_info=mybir.SyncInfo(on_wait=chunk, on_update=[])))
                        k += 1
                    i.sync_info = mybir.SyncInfo(on_wait=waits,
                                                 on_update=list(si.on_update))
                out.append(i)
            b.instructions = out
    return nc


_CACHE = {}


def _get_nc(debug=()):
    key = tuple(sorted(debug))
    if key not in _CACHE:
        _CACHE[key] = _legalize_waits(build(key))
    return _CACHE[key]


def _maps(inputs):
    i = {k: np.asarray(v) for k, v in inputs.items()}
    shared = _prep_shared(i)
    maps = []
    for c in range(NCORES):
        m = dict(shared)
        m.update(_prep_core(c, i))
        maps.append(m)
    return maps


def kernel(**inputs):
    nc = _get_nc()
    res = run_bass_kernel_spmd(nc, _maps(inputs), list(range(NCORES))).results
    tot = np.zeros((N_TOK, N_TOK), np.float64)
    cnt = np.zeros(N_TOK, np.float64)
    for c in range(NCORES):
        tot += np.asarray(res[c]['part'], np.float64)
        cnt += np.asarray(res[c]['cnt'], np.float64).T.reshape(-1)
    out = tot / np.maximum(cnt, 1.0)[:, None]
    return out[None].astype(np.float32)


# revision 31
# speedup vs baseline: 1.5917x; 1.0091x over previous
"""AtomAttentionEncoder — hand-written Bass/Tile kernel for 8 trn2 NeuronCores.

Sequence-parallel over atoms: each core owns 192 atoms with a 192-atom halo on
each side (576 local atoms, zero inter-core collectives — the host sums the 8
per-core [384,384] token partials). Per core, bf16 channel-major throughout:

- atom activations [128 ch, 576 atoms]; LN stats via ones-matmuls (the PE
  broadcast comes for free), rstd = exp(-0.5*ln(var+eps)) so the whole main
  phase uses only the natural_log_exp activation-table set (one sigmoid-set
  phase precomputes all s-dependent gates, one Newton-rsqrt handles LN(s)).
- pair tensor [(gc,j)->(j,c) 128, rows=(block, q//4, q%4, k)] block-diagonal
  grouping so the 16-channel pair MLP and LN run as 128-wide matmuls; the
  whole pair pipeline is fused per 512-column chunk (one chunk == one block).
- 32x128 block-local attention on validity-shrinking blocks (14/10/6 per
  layer); no-max softmax (logits are tiny; masked keys get -1e4 before exp,
  exactly matching the reference's -1e9 since exp underflows to 0).
"""
import numpy as np
import ml_dtypes

import concourse.bass as bass
import concourse.mybir as mybir
import concourse.tile as tile
from concourse.alu_op_type import AluOpType
from concourse.bass_utils import run_bass_kernel_spmd

BF = mybir.dt.bfloat16
F32 = mybir.dt.float32
I32 = mybir.dt.int32
NPBF = ml_dtypes.bfloat16
AF = mybir.ActivationFunctionType

NCORES, OWN, MARGIN, LOC, PAD = 8, 192, 192, 576, 48
N_ATOM, N_TOK = 1536, 384
H, CH, CP = 4, 32, 16
NB0 = 14                      # layer-0 valid blocks (2..15)
ROWS = NB0 * 512              # pair rows (g, j, qq, k), q = 4j+qq
BIG = 1e4
EPS = 1e-5
L = 3
BLOCKS = [list(range(2, 16)), list(range(4, 14)), list(range(6, 12))]
RANGES = [(0, 576), (64, 512), (128, 448)]
AMUL, AADD, ASUB = AluOpType.mult, AluOpType.add, AluOpType.subtract
AMAX = AluOpType.max
RSQRT_MAGIC = 0x5F3759DF


# ------------------------------------------------------------------ host prep
def _prep_core(core, i):
    f32 = np.float32
    start = core * OWN - MARGIN
    idx = np.clip(start + np.arange(LOC), 0, N_ATOM - 1)
    pos = i['ref_pos'][0][idx].astype(f32)
    uid = i['ref_space_uid'][0][idx].astype(f32)
    feats = np.concatenate([
        i['ref_element'][0][idx],
        i['ref_atom_name_chars'][0].reshape(N_ATOM, -1)[idx],
        pos, i['ref_mask'][0][idx][:, None], i['ref_charge'][0][idx][:, None],
        uid[:, None], np.zeros((LOC, 122), f32)], axis=1)   # [LOC, 512]

    atom_mask = (i['atom_to_token_index'][0][idx] @ i['token_mask'][0]).astype(f32)
    mb = np.zeros((NB0, 128), f32)
    for n, b in enumerate(BLOCKS[0]):
        kl = 32 * b - PAD + np.arange(128)
        kg = start + kl
        ok = (kg >= 0) & (kg < N_ATOM) & (kl >= 0) & (kl < LOC)
        am = atom_mask[np.clip(kl, 0, LOC - 1)] > 0
        mb[n] = (np.where(ok & am, 1.0, 0.0) - 1.0) * BIG
    mb = mb.reshape(1, -1)

    qat = np.zeros((NB0, 8, 4), np.int64)          # local q-atom of (g, j, qq)
    for n, b in enumerate(BLOCKS[0]):
        qat[n] = 32 * b + (4 * np.arange(8)[:, None] + np.arange(4)[None, :])
    posq = np.zeros((24, NB0 * 4), f32)
    uidq = np.zeros((40, NB0 * 4), f32)
    for j in range(8):
        for gc in range(5):
            uidq[gc * 8 + j] = uid[qat[:, j, :]].reshape(-1)
            if gc < 3:
                posq[gc * 8 + j] = pos[qat[:, j, :], gc].reshape(-1)
    posw = np.zeros((24, LOC), f32)
    uidw = np.zeros((40, LOC), f32)
    for j in range(8):
        for gc in range(5):
            uidw[gc * 8 + j] = uid
            if gc < 3:
                posw[gc * 8 + j] = pos[:, gc]

    # a2t row-major, atoms padded 192 -> 2 chunks of 128 stacked on free axis
    a2t_own = i['atom_to_token_index'][0][core * OWN:(core + 1) * OWN].astype(f32)
    a2t_st = np.zeros((128, 2 * N_TOK), f32)
    a2t_st[:, :N_TOK] = a2t_own[0:128]
    a2t_st[0:64, N_TOK:] = a2t_own[128:192]
    d = {'feats': feats.T, 'posq': posq, 'uidq': uidq, 'posw': posw,
         'uidw': uidw, 'mb': mb, 'a2t': a2t_st}
    return {k: np.ascontiguousarray(v.astype(NPBF)) for k, v in d.items()}


def _prep_shared(i):
    f32 = np.float32
    inv_sqrt = 1.0 / np.sqrt(CH)
    Wf = np.asarray(i['W_feats'], f32)
    Wf2 = np.concatenate([Wf[4:132], Wf[133:389], Wf[0:3], Wf[3:4],
                          Wf[132:133], Wf[389:390],
                          np.zeros((122, 128), f32)], axis=0)       # [512,128]

    W5 = np.concatenate([np.asarray(i['W_ref_offset'], f32),
                         np.asarray(i['W_valid'], f32),
                         np.asarray(i['W_inv_sq'], f32)], 0)        # [5,16]
    bdW5 = np.zeros((40, 128), f32)
    for j in range(8):
        for gc in range(5):
            bdW5[gc * 8 + j, j * 16:(j + 1) * 16] = W5[gc]
    bdones3 = np.zeros((24, 8), f32)
    for j in range(8):
        for gc in range(3):
            bdones3[gc * 8 + j, j] = 1.0

    def bd8(w):
        n = w.shape[1]
        o = np.zeros((128, 8 * n), f32)
        for j in range(8):
            o[j * 16:(j + 1) * 16, j * n:(j + 1) * n] = w
        return o

    bdmlp = np.concatenate([bd8(np.asarray(i[f'W_mlp{k}'], f32)) for k in (1, 2, 3)], 1)
    bdones16 = bd8(np.full((16, 16), 1.0 / 16, f32))
    Wb_eff = np.asarray(i['lnz_g'], f32)[:, :, None] * np.asarray(i['Wb'], f32)
    bdWb = np.concatenate([bd8(Wb_eff[l]) for l in range(L)], 1)    # [128, 96]

    def stackL(w):
        return np.concatenate([np.asarray(w[l], f32) for l in range(L)], 1)

    d = {
        'Wf2': Wf2, 'bdW5': bdW5, 'bdones3': bdones3, 'bdones16': bdones16,
        'bdmlp': bdmlp, 'bdWb': bdWb,
        'Wlm': np.concatenate([np.asarray(i['W_l'], f32),
                               np.asarray(i['W_m'], f32)], 1),
        'Wq': stackL(np.asarray(i['Wq'], f32) * inv_sqrt),
        'Wk': stackL(i['Wk']), 'Wv': stackL(i['Wv']),
        'Wgate': stackL(i['Wgate']), 'Wo': stackL(i['Wo']),
        'aWg': stackL(np.asarray(i['attn_ada_Wg'], f32) *
                      np.asarray(i['attn_ada_gamma_s'], f32)[:, :, None]),
        'aWs': stackL(np.asarray(i['attn_ada_Ws'], f32) *
                      np.asarray(i['attn_ada_gamma_s'], f32)[:, :, None]),
        'tWg': stackL(np.asarray(i['tr_ada_Wg'], f32) *
                      np.asarray(i['tr_ada_gamma_s'], f32)[:, :, None]),
        'tWs': stackL(np.asarray(i['tr_ada_Ws'], f32) *
                      np.asarray(i['tr_ada_gamma_s'], f32)[:, :, None]),
        'Wsg': stackL(i['Wsg']), 'tWog': stackL(i['tr_Wog']),
        'trW1': np.concatenate([np.asarray(i['tr_W1'], f32)[l] for l in range(L)], 1),
        'trW2': np.concatenate([np.asarray(i['tr_W2'], f32)[l] for l in range(L)], 1),
        'trWout': np.concatenate(
            [np.asarray(i['tr_Wout'], f32)[l, h * 128:(h + 1) * 128]
             for l in range(L) for h in range(2)], 1),
        'Wot': i['W_out_tok'],
    }
    d = {k: np.ascontiguousarray(np.asarray(v, f32).astype(NPBF)) for k, v in d.items()}
    d['ident'] = np.eye(128, dtype=f32).astype(NPBF)
    pmat = np.zeros((128, 128), f32)       # zt row p'=16j+4h+qq -> m=32h+4j+qq
    for j in range(8):
        for h in range(4):
            for qq in range(4):
                pmat[16 * j + 4 * h + qq, 32 * h + 4 * j + qq] = 1.0
    d['pmat'] = pmat.astype(NPBF)
    d['bq'] = np.ascontiguousarray((np.asarray(i['bq'], f32) * inv_sqrt).T)
    d['bg_a'] = np.ascontiguousarray(np.asarray(i['attn_ada_bg'], f32).T)
    d['bg_t'] = np.ascontiguousarray(np.asarray(i['tr_ada_bg'], f32).T)
    d['bsg'] = np.ascontiguousarray(np.asarray(i['bsg'], f32).T)
    d['tbog'] = np.ascontiguousarray(np.asarray(i['tr_bog'], f32).T)
    return d


# ------------------------------------------------------------ program builder
def build(debug=()):
    nc = bass.Bass()
    TT = nc.vector.tensor_tensor
    TS = nc.vector.tensor_scalar
    ACT = nc.scalar.activation
    MM = nc.tensor.matmul

    def din(name, shape, dt=BF):
        return nc.dram_tensor(name, shape, dt, kind="ExternalInput")[:]

    ins = {}
    for nm, shp in [('posq', [24, NB0 * 4]),
                    ('uidq', [40, NB0 * 4]), ('posw', [24, LOC]),
                    ('uidw', [40, LOC]), ('mb', [1, NB0 * 128]),
                    ('a2t', [128, 2 * N_TOK]),
                    ('bdW5', [40, 128]), ('bdones3', [24, 8]),
                    ('bdones16', [128, 128]), ('bdmlp', [128, 384]),
                    ('bdWb', [128, 32 * L]), ('Wlm', [128, 32]),
                    ('Wq', [128, 128 * L]), ('Wk', [128, 128 * L]),
                    ('Wv', [128, 128 * L]), ('Wgate', [128, 128 * L]),
                    ('Wo', [128, 128 * L]), ('aWg', [128, 128 * L]),
                    ('aWs', [128, 128 * L]), ('tWg', [128, 128 * L]),
                    ('tWs', [128, 128 * L]), ('Wsg', [128, 128 * L]),
                    ('tWog', [128, 128 * L]), ('trW1', [128, 256 * L]),
                    ('trW2', [128, 256 * L]), ('trWout', [128, 128 * 2 * L]),
                    ('Wot', [128, N_TOK])]:
        ins[nm] = din(nm, shp)
    for nm in ('bq', 'bg_a', 'bg_t', 'bsg', 'tbog'):
        ins[nm] = din(nm, [128, L], F32)
    feats_d = din('feats', [512, LOC])
    wf2_d = din('Wf2', [512, 128])
    ins['ident'] = din('ident', [128, 128])
    ins['pmat'] = din('pmat', [128, 128])

    out_part = nc.dram_tensor('part', [N_TOK, N_TOK], F32, kind="ExternalOutput")[:]
    out_cnt = nc.dram_tensor('cnt', [128, 3], F32, kind="ExternalOutput")[:]
    dumps = []

    def dump(name, ap):
        if name not in debug:
            return
        t = nc.dram_tensor('dbg_' + name, list(ap.shape), ap.dtype,
                           kind="ExternalOutput")[:]
        nc.sync.dma_start(out=t, in_=ap)

    with tile.TileContext(nc) as tc, \
         tc.tile_pool(name="const", bufs=1) as constp, \
         tc.tile_pool(name="state", bufs=1) as statep, \
         tc.tile_pool(name="work", bufs=3) as workp, \
         tc.tile_pool(name="abuf", bufs=2) as abufp, \
         tc.tile_pool(name="blk", bufs=16) as blkp, \
         tc.tile_pool(name="psum", bufs=1, space="PSUM") as psp:

        ones_row = constp.tile([1, 128], BF)
        nc.vector.memset(ones_row, 1.0)
        ones_col = constp.tile([128, 1], BF)
        nc.vector.memset(ones_col, 1.0)
        inv128 = constp.tile([128, 128], BF)
        nc.vector.memset(inv128, 1.0 / 128.0)
        epsc = constp.tile([128, 1], F32)
        nc.vector.memset(epsc, EPS)
        onef = constp.tile([128, 1], F32)
        nc.vector.memset(onef, 1.0)
        zeros128 = constp.tile([128, 128], BF)
        nc.vector.memset(zeros128, 0.0)

        def load(ap, name, pool=constp):
            t = pool.tile(list(ap.shape), ap.dtype, name=name)
            nc.sync.dma_start(out=t, in_=ap)
            return t

        s_feats = [load(feats_d[kk * 128:(kk + 1) * 128, :], f'feats{kk}')
                   for kk in range(4)]
        s_wf2 = [load(wf2_d[kk * 128:(kk + 1) * 128, :], f'wf2_{kk}')
                 for kk in range(4)]
        _order = ['Wlm', 'posw', 'posq', 'uidw', 'uidq', 'bdW5', 'bdones3',
                  'ident', 'pmat', 'bdmlp', 'bdones16', 'bdWb', 'mb',
                  'aWg', 'aWs', 'tWg', 'tWs', 'Wsg', 'tWog',
                  'bg_a', 'bg_t', 'bsg', 'tbog', 'bq',
                  'Wq', 'Wk', 'Wv', 'Wgate', 'Wo', 'trW1', 'trW2', 'trWout',
                  'Wot', 'a2t']
        assert set(_order) == set(ins), set(ins) ^ set(_order)
        s_ = {nm: load(ins[nm], 'w_' + nm) for nm in _order}

        def ps512():
            return psp.tile([128, 512], F32, tag='ps', bufs=8, name='ps512')

        def psb(tag='psb'):
            return psp.tile([128, 128], F32, tag='ps', bufs=8, name='psb')

        # ---------------- embed: cl [128, LOC] ----------------
        cl = statep.tile([128, LOC], BF)
        for n0 in (0, 512):
            n1 = min(n0 + 512, LOC)
            ps = ps512()
            for kk in range(4):
                MM(ps[:, :n1 - n0], s_wf2[kk], s_feats[kk][:, n0:n1],
                   start=(kk == 0), stop=(kk == 3))
            TS(cl[:, n0:n1], ps[:, :n1 - n0], 1.0, None, AMUL)
        dump('cl', cl)

        # crelu -> [crl | crm] = crlm [32, LOC]; then crl_bd + crm_rep
        crelu = workp.tile([128, LOC], BF, tag='crelu', bufs=1)
        TS(crelu, cl, 0.0, None, AMAX)
        crlm = workp.tile([32, LOC], BF, tag='crlm', bufs=1)
        for n0 in (0, 512):
            n1 = min(n0 + 512, LOC)
            ps = ps512()
            MM(ps[0:32, :n1 - n0], s_['Wlm'], crelu[:, n0:n1], start=True, stop=True)
            TS(crlm[:, n0:n1], ps[0:32, :n1 - n0], 1.0, None, AMUL)
        crl_bd = statep.tile([128, NB0 * 4], BF)
        crm_rep = statep.tile([128, LOC], BF)
        pz = crlm.ap[0][0]
        for j in range(8):
            nc.sync.dma_start(
                out=crl_bd[j * 16:(j + 1) * 16, :],
                in_=bass.AP(tensor=crlm.tensor, offset=crlm.offset + 64 + 4 * j,
                            ap=[[pz, 16], [32, NB0], [1, 4]]))
            nc.sync.dma_start(out=crm_rep[j * 16:(j + 1) * 16, :],
                              in_=bass.AP(tensor=crlm.tensor,
                                          offset=crlm.offset + 16 * pz,
                                          ap=[[pz, 16], [1, LOC]]))
        dump('crl_bd', crl_bd)

        # ---------------- LN(s) via Newton rsqrt (table-free) ----------------
        sh = statep.tile([128, LOC], BF)
        xcs = workp.tile([128, LOC], BF, tag='xcs', bufs=1)
        vps = workp.tile([128, LOC], F32, tag='vps', bufs=1)   # var+eps
        y0 = workp.tile([128, LOC], F32, tag='nr1', bufs=1)
        t1 = workp.tile([128, LOC], F32, tag='nr2', bufs=1)
        for n0 in (0, 512):
            n1 = min(n0 + 512, LOC)
            w = n1 - n0
            ps = ps512()
            MM(ps[:, :w], inv128, cl[:, n0:n1], start=True, stop=True)
            TT(xcs[:, n0:n1], cl[:, n0:n1], ps[:, :w], ASUB)
            sq = workp.tile([128, 512], BF, tag='sqs')
            TT(sq[:, :w], xcs[:, n0:n1], xcs[:, n0:n1], AMUL)
            ps2 = ps512()
            MM(ps2[:, :w], inv128, sq[:, :w], start=True, stop=True)
            TS(vps[:, n0:n1], ps2[:, :w], EPS, None, AADD)
        v32 = vps.bitcast(I32)
        y32 = y0.bitcast(I32)
        t32 = t1.bitcast(I32)
        nc.vector.memset(t32, RSQRT_MAGIC)
        TS(y32, v32, 1, None, AluOpType.logical_shift_right)
        TT(y32, t32, y32, ASUB)                      # magic - (v>>1)
        for _ in range(2):                           # y *= 1.5 - 0.5*v*y*y
            TT(t1, y0, y0, AMUL)
            TT(t1, vps, t1, AMUL)
            TS(t1, t1, -0.5, 1.5, AMUL, AADD)
            TT(y0, y0, t1, AMUL)
        TT(sh, xcs, y0, AMUL)
        dump('sh', sh)

        # ---------------- s-gate precompute (sigmoid table set) ----------------
        sgate, sa = {}, {}
        for nm, src, bias in (('ga', 'aWg', 'bg_a'), ('gt', 'tWg', 'bg_t'),
                              ('sg', 'Wsg', 'bsg'), ('og', 'tWog', 'tbog')):
            x = sh if nm in ('ga', 'gt') else cl
            for l in range(L):
                g = statep.tile([128, LOC], BF, name=f'{nm}{l}')
                for n0 in (0, 512):
                    n1 = min(n0 + 512, LOC)
                    ps = ps512()
                    MM(ps[:, :n1 - n0], s_[src][:, l * 128:(l + 1) * 128],
                       x[:, n0:n1], start=True, stop=True)
                    ACT(g[:, n0:n1], ps[:, :n1 - n0], AF.Sigmoid,
                        bias=s_[bias][:, l:l + 1])
                sgate[(nm, l)] = g
        for nm, src in (('sa', 'aWs'), ('st', 'tWs')):
            for l in range(L):
                g = statep.tile([128, LOC], BF, name=f'{nm}{l}')
                for n0 in (0, 512):
                    n1 = min(n0 + 512, LOC)
                    ps = ps512()
                    MM(ps[:, :n1 - n0], s_[src][:, l * 128:(l + 1) * 128],
                       sh[:, n0:n1], start=True, stop=True)
                    TS(g[:, n0:n1], ps[:, :n1 - n0], 1.0, None, AMUL)
                sa[(nm, l)] = g
        dump('ga0', sgate[('ga', 0)])

        # ---------------- pair pipeline (fused per 512-chunk) ----------------
        plm = statep.tile([128, ROWS], BF)

        def winap(t, base, rows):
            return bass.AP(tensor=t.tensor, offset=t.offset + base,
                           ap=[[t.ap[0][0], rows], [0, 4], [1, 128]])

        def qapx(t, rows, goff):
            return bass.AP(tensor=t.tensor, offset=t.offset + goff * 4,
                           ap=[[t.ap[0][0], rows], [1, 4], [0, 128]])

        for n in range(NB0):
            base = 32 * (n + 2) - 48
            G = workp.tile([40, 512], BF, tag='G')
            d_raw = workp.tile([24, 512], BF, tag='draw')
            TT(d_raw, winap(s_['posw'], base, 24), qapx(s_['posq'], 24, n), ASUB)
            TT(G, winap(s_['uidw'], base, 40), qapx(s_['uidq'], 40, n),
               AluOpType.is_equal)
            TT(G[0:24, :], d_raw, G[0:24, :], AMUL)
            d2 = workp.tile([24, 512], BF, tag='d2')
            TT(d2, d_raw, d_raw, AMUL)
            psd = ps512()
            MM(psd[32:40, :], s_['bdones3'], d2, start=True, stop=True,
               tile_position=(0, 32))
            lni = workp.tile([40, 512], F32, tag='lni')
            ACT(lni[32:40, :], psd[32:40, :], AF.Ln, bias=onef[32:40, :])
            inv = workp.tile([40, 512], BF, tag='inv')
            ACT(inv[32:40, :], lni[32:40, :], AF.Exp, scale=-1.0)
            TT(G[32:40, :], inv[32:40, :], G[32:40, :], AMUL)

            ps = ps512()
            MM(ps, s_['bdW5'], G, start=True, stop=False)
            MM(ps, s_['ident'], qapx(crl_bd, 128, n), start=False, stop=False,
               skip_group_check=True)
            MM(ps, s_['ident'], winap(crm_rep, base, 128), start=False, stop=True,
               skip_group_check=True)
            ppre = workp.tile([128, 512], BF, tag='ppre')
            r = workp.tile([128, 512], BF, tag='rmlp')
            nc.scalar.copy(ppre, ps)
            TS(r, ps, 0.0, None, AMAX)
            pf = None
            for k in range(3):
                psm = ps512()
                MM(psm, s_['bdmlp'][:, k * 128:(k + 1) * 128], r,
                   start=True, stop=(k == 2))
                if k < 2:
                    r = workp.tile([128, 512], BF, tag='rmlp')
                    TS(r, psm, 0.0, None, AMAX)
                else:
                    MM(psm, s_['ident'], ppre, start=False, stop=True,
                       skip_group_check=True)
                    pf = workp.tile([128, 512], BF, tag='pfull')
                    nc.scalar.copy(pf, psm)
            psmu = ps512()
            MM(psmu, s_['bdones16'], pf, start=True, stop=True)
            xc = workp.tile([128, 512], BF, tag='xc')
            TT(xc, pf, psmu, ASUB)
            sq2 = workp.tile([128, 512], BF, tag='sq2')
            TT(sq2, xc, xc, AMUL)
            psv = ps512()
            MM(psv, s_['bdones16'], sq2, start=True, stop=True)
            lnv = workp.tile([128, 512], F32, tag='lnv')
            ACT(lnv, psv, AF.Ln, bias=epsc)
            rstd = workp.tile([128, 512], BF, tag='rstdp')
            ACT(rstd, lnv, AF.Exp, scale=-0.5)
            TT(plm[:, n * 512:(n + 1) * 512], xc, rstd, AMUL)
        dump('plm', plm)

        # ---------------- layer loop ----------------
        a_cur = cl
        for l in range(L):
            r0, r1 = RANGES[l]
            blks = BLOCKS[l]
            nb = len(blks)

            ahat = abufp.tile([128, LOC], BF, tag='ahat')
            for c0 in range(r0, r1, 512):
                c1 = min(c0 + 512, r1)
                w = c1 - c0
                ps = ps512()
                MM(ps[:, :w], inv128, a_cur[:, c0:c1], start=True, stop=True)
                xca = abufp.tile([128, 512], BF, tag='xca')
                TT(xca[:, :w], a_cur[:, c0:c1], ps[:, :w], ASUB)
                sqa = abufp.tile([128, 512], BF, tag='sqa')
                TT(sqa[:, :w], xca[:, :w], xca[:, :w], AMUL)
                ps2 = ps512()
                MM(ps2[:, :w], inv128, sqa[:, :w], start=True, stop=True)
                lnva = abufp.tile([128, 512], F32, tag='lnva')
                ACT(lnva[:, :w], ps2[:, :w], AF.Ln, bias=epsc)
                rstda = abufp.tile([128, 512], BF, tag='rstda')
                ACT(rstda[:, :w], lnva[:, :w], AF.Exp, scale=-0.5)
                TT(ahat[:, c0:c1], xca[:, :w], rstda[:, :w], AMUL)
            an = abufp.tile([128, LOC], BF, tag='an')
            tn = abufp.tile([128, LOC], BF, tag='tn')
            TT(an[:, r0:r1], sgate[('ga', l)][:, r0:r1], ahat[:, r0:r1], AMUL)
            TT(an[:, r0:r1], an[:, r0:r1], sa[('sa', l)][:, r0:r1], AADD)
            TT(tn[:, r0:r1], sgate[('gt', l)][:, r0:r1], ahat[:, r0:r1], AMUL)
            TT(tn[:, r0:r1], tn[:, r0:r1], sa[('st', l)][:, r0:r1], AADD)
            if l == 0:
                dump('an0', an)

            q_sb = abufp.tile([128, LOC], BF, tag='q')
            k_sb = abufp.tile([128, LOC], BF, tag='k')
            g_sb = abufp.tile([128, LOC], BF, tag='g')
            for c0 in range(r0, r1, 512):
                c1 = min(c0 + 512, r1)
                w = c1 - c0
                psq = ps512()
                MM(psq[:, :w], s_['Wq'][:, l * 128:(l + 1) * 128], an[:, c0:c1],
                   start=True, stop=True)
                ACT(q_sb[:, c0:c1], psq[:, :w], AF.Identity,
                    bias=s_['bq'][:, l:l + 1])
                psk = ps512()
                MM(psk[:, :w], s_['Wk'][:, l * 128:(l + 1) * 128], an[:, c0:c1],
                   start=True, stop=True)
                TS(k_sb[:, c0:c1], psk[:, :w], 1.0, None, AMUL)
                psg = ps512()
                MM(psg[:, :w], s_['Wgate'][:, l * 128:(l + 1) * 128], an[:, c0:c1],
                   start=True, stop=True)
                ex0 = abufp.tile([128, 512], F32, tag='ex0')
                ACT(ex0[:, :w], psg[:, :w], AF.Exp)
                ACT(ex0[:, :w], ex0[:, :w], AF.Ln, bias=onef)
                TT(ex0[:, :w], psg[:, :w], ex0[:, :w], ASUB)
                ACT(g_sb[:, c0:c1], ex0[:, :w], AF.Exp)

            zbt = {}
            for b in blks:
                ch = b - 2
                psz = ps512()
                MM(psz[0:32, :], s_['bdWb'][:, l * 32:(l + 1) * 32],
                   plm[:, ch * 512:(ch + 1) * 512], start=True, stop=True)
                zs = blkp.tile([32, 512], BF, tag='zs', bufs=4)
                nc.scalar.copy(zs, psz[0:32, :])
                zt = blkp.tile([128, 128], BF, tag='zt', bufs=16)
                pzt = zt.ap[0][0]
                for qq in range(4):
                    nc.sync.dma_start(
                        out=bass.AP(tensor=zt.tensor, offset=zt.offset + qq * pzt,
                                    ap=[[4 * pzt, 32], [1, 128]]),
                        in_=zs[:, 128 * qq:128 * qq + 128])
                zbt[b] = zt
            if l == 0:
                dump('zbt2', zbt[2])

            v_sb = {}
            for b in blks:
                base = 32 * b - 48
                psv2 = psb()
                MM(psv2, an[:, base:base + 128],
                   s_['Wv'][:, l * 128:(l + 1) * 128], start=True, stop=True)
                vt = blkp.tile([128, 128], BF, tag='vt', bufs=16)
                nc.scalar.copy(vt, psv2)
                v_sb[b] = vt

            # pass 1: logits -> exp -> A (accumulate row sums)
            dsum = abufp.tile([128, 16], F32, tag='dsum')
            A_sb = {}
            for n, b in enumerate(blks):
                base = 32 * b - 48
                psl = psb()
                MM(psl, s_['pmat'], zbt[b], start=True, stop=False,
                   skip_group_check=True)
                MM(psl, ones_row, s_['mb'][0:1, (b - 2) * 128:(b - 1) * 128],
                   start=False, stop=False, skip_group_check=True)
                for h in range(4):
                    MM(psl[32 * h:32 * h + 32, :],
                       q_sb[32 * h:32 * h + 32, 32 * b:32 * b + 32],
                       k_sb[32 * h:32 * h + 32, base:base + 128],
                       start=False, stop=(h == 3),
                       tile_position=(32 * h, 32 * h), skip_group_check=True)
                At = blkp.tile([128, 128], BF, tag='At', bufs=16)
                ACT(At, psl, AF.Exp, accum_out=dsum[:, n:n + 1])
                A_sb[b] = At
            if l == 0:
                dump('A2', A_sb[2])
            rd = abufp.tile([128, 16], F32, tag='rd')
            TS(rd[:, :nb], dsum[:, :nb], 1e-9, None, AADD)
            nc.vector.reciprocal(rd[:, :nb], rd[:, :nb])

            # pass 2: normalize A -> transpose -> AV -> O (ch-major)
            Ocm = abufp.tile([128, LOC], F32, tag='Ocm')
            for n, b in enumerate(blks):
                At = A_sb[b]
                TS(At, At, rd[:, n:n + 1], None, AMUL)
                pst = psp.tile([128, 128], BF, tag='ps', bufs=8, name='pstT')
                nc.tensor.transpose(pst, At, s_['ident'])
                ATs = blkp.tile([128, 128], BF, tag='ATs', bufs=8)
                TS(ATs, pst, 1.0, None, AMUL)
                pso = psb()
                MM(pso[:, 0:32], s_['ident'], zeros128[:, 0:32],
                   start=True, stop=False, skip_group_check=True)
                for h in range(4):
                    MM(pso[32 * h:32 * h + 32, 0:32],
                       ATs[:, 32 * h:32 * h + 32],
                       v_sb[b][:, 32 * h:32 * h + 32],
                       start=False, stop=(h == 3),
                       tile_position=(0, 32 * h), skip_group_check=True)
                nc.vector.transpose(Ocm[:, 32 * b:32 * b + 32], pso[:, 0:32])
            if l == 0:
                dump('Ocm0', Ocm)

            # epilogue: gated attn out + transition, next range only
            nr0, nr1 = 32 * blks[0], 32 * blks[-1] + 32
            go = abufp.tile([128, LOC], BF, tag='go')
            TT(go[:, nr0:nr1], g_sb[:, nr0:nr1], Ocm[:, nr0:nr1], AMUL)
            a_nxt = abufp.tile([128, LOC], BF, tag='anxt')
            nc.vector.memset(a_nxt, 0.0)
            for c0 in range(nr0, nr1, 512):
                c1 = min(c0 + 512, nr1)
                w = c1 - c0
                psa = ps512()
                MM(psa[:, :w], s_['Wo'][:, l * 128:(l + 1) * 128], go[:, c0:c1],
                   start=True, stop=True)
                ao = abufp.tile([128, 512], BF, tag='ao')
                TT(ao[:, :w], sgate[('sg', l)][:, c0:c1], psa[:, :w], AMUL)
                hh = []
                for t2 in range(2):
                    psh = ps512()
                    MM(psh[:, :w],
                       s_['trW1'][:, l * 256 + t2 * 128:l * 256 + (t2 + 1) * 128],
                       tn[:, c0:c1], start=True, stop=True)
                    ex = abufp.tile([128, 512], F32, tag='ex')
                    ACT(ex[:, :w], psh[:, :w], AF.Exp)
                    ACT(ex[:, :w], ex[:, :w], AF.Ln, bias=onef)
                    TT(ex[:, :w], psh[:, :w], ex[:, :w], ASUB)
                    ACT(ex[:, :w], ex[:, :w], AF.Exp)
                    s1 = abufp.tile([128, 512], BF, tag='s1')
                    TT(s1[:, :w], psh[:, :w], ex[:, :w], AMUL)
                    psh2 = ps512()
                    MM(psh2[:, :w],
                       s_['trW2'][:, l * 256 + t2 * 128:l * 256 + (t2 + 1) * 128],
                       tn[:, c0:c1], start=True, stop=True)
                    ht = abufp.tile([128, 512], BF, tag=f'hh{t2}')
                    TT(ht[:, :w], s1[:, :w], psh2[:, :w], AMUL)
                    hh.append(ht)
                pst2 = ps512()
                MM(pst2[:, :w], s_['trWout'][:, (l * 2) * 128:(l * 2 + 1) * 128],
                   hh[0][:, :w], start=True, stop=False)
                MM(pst2[:, :w], s_['trWout'][:, (l * 2 + 1) * 128:(l * 2 + 2) * 128],
                   hh[1][:, :w], start=False, stop=True)
                to = abufp.tile([128, 512], BF, tag='to')
                TT(to[:, :w], sgate[('og', l)][:, c0:c1], pst2[:, :w], AMUL)
                TT(a_nxt[:, c0:c1], ao[:, :w], to[:, :w], AADD)
            a_cur = a_nxt
        dump('a3', a_cur)

        # ---------------- final: al -> token partials ----------------
        al_rm = []
        for c in range(2):
            c0 = 192 + 128 * c
            psf = ps512()
            MM(psf[:, :N_TOK], a_cur[:, c0:c0 + 128], s_['Wot'],
               start=True, stop=True)
            alr = statep.tile([128, N_TOK], BF, name=f'alr{c}')
            TS(alr, psf[:, :N_TOK], 0.0, None, AMAX)
            al_rm.append(alr)
        partf = statep.tile([128, N_TOK], F32, name='partf')
        cntf = statep.tile([128, 3], F32, name='cntf')
        for tchunk in range(3):
            psp2 = ps512()
            for c in range(2):
                MM(psp2[:, :N_TOK],
                   s_['a2t'][:, c * N_TOK + tchunk * 128:
                             c * N_TOK + tchunk * 128 + 128],
                   al_rm[c], start=(c == 0), stop=(c == 1))
            TS(partf, psp2[:, :N_TOK], 1.0, None, AMUL)
            nc.sync.dma_start(out=out_part[tchunk * 128:(tchunk + 1) * 128, :],
                              in_=partf)
            psc = psb()
            for c in range(2):
                MM(psc[:, 0:1],
                   s_['a2t'][:, c * N_TOK + tchunk * 128:
                             c * N_TOK + tchunk * 128 + 128],
                   ones_col, start=(c == 0), stop=(c == 1))
            TS(cntf[:, tchunk:tchunk + 1], psc[:, 0:1], 1.0, None, AMUL)
        nc.sync.dma_start(out=out_cnt, in_=cntf)

    return nc


def _legalize_waits(nc, maxw=1):
    """The container's walrus accepts at most one sync-wait per instruction;
    Tile emits several. Split excess waits onto preceding same-engine NoOps
    (each wait is a >= threshold, so sequential waits are equivalent)."""
    for fn in nc.m.functions:
        for b in fn.blocks:
            out = []
            for i in b.instructions:
                si = i.sync_info
                if si is not None and len(si.on_wait) > maxw:
                    waits = list(si.on_wait)
                    k = 0
                    while len(waits) > maxw:
                        chunk, waits = waits[:maxw], waits[maxw:]
                        out.append(mybir.InstNoOp(
                            name=f"{i.name}-wsplit{k}", ins=[], outs=[],
                            engine=i.engine,
                            sync_info=mybir.SyncInfo(on_wait=chunk, on_update=[])))
                        k += 1
                    i.sync_info = mybir.SyncInfo(on_wait=waits,
                                                 on_update=list(si.on_update))
                out.append(i)
            b.instructions = out
    return nc


_CACHE = {}


def _get_nc(debug=()):
    key = tuple(sorted(debug))
    if key not in _CACHE:
        _CACHE[key] = _legalize_waits(build(key))
    return _CACHE[key]


def _maps(inputs):
    i = {k: np.asarray(v) for k, v in inputs.items()}
    shared = _prep_shared(i)
    maps = []
    for c in range(NCORES):
        m = dict(shared)
        m.update(_prep_core(c, i))
        maps.append(m)
    return maps


def kernel(**inputs):
    nc = _get_nc()
    res = run_bass_kernel_spmd(nc, _maps(inputs), list(range(NCORES))).results
    tot = np.zeros((N_TOK, N_TOK), np.float64)
    cnt = np.zeros(N_TOK, np.float64)
    for c in range(NCORES):
        tot += np.asarray(res[c]['part'], np.float64)
        cnt += np.asarray(res[c]['cnt'], np.float64).T.reshape(-1)
    out = tot / np.maximum(cnt, 1.0)[:, None]
    return out[None].astype(np.float32)
